# revision 18
# baseline (speedup 1.0000x reference)
"""Trainium2 Bass kernel for nn_AttentionalCopula (sparse_attention).

Sharding: data-parallel over batch (B=8 -> 8 cores); per-head K/V MLP stacks
computed locally per core. Weights replicated. No collectives.

Key optimizations over the v1 baseline:
  * Row compaction: the attention only gathers K/V rows from
    union_s({left} u ({arange} if s<i else {right})) -- ~1350 of 2048 rows.
    The MLP chains run only on those rows.
  * K chains in fp8 (float8e4) with MatmulPerfMode.DoubleRow: K_eff=256 per
    matmul at 0.5 cyc/col (4x bf16). V chains stay bf16 (V values feed the
    output directly; fp8 there fails the accuracy gate -- measured).
  * The u-row (feature 257) enters L1 psum via a K=1 bf16 rank-1 matmul
    (tile_position-paired across the two f-tiles).
  * Relu/copy work is round-robined across ACT/DVE/Pool engines.
  * l=0 K/V staged interleaved in DRAM so one indirect gather per slot
    fetches both; contiguous slots (n=1, s<i) use direct DMA.
"""

from contextlib import ExitStack

import ml_dtypes
import numpy as np

import concourse.bass as bass
import concourse.mybir as mybir
import concourse.tile as tile
from concourse import bacc
from concourse.bass_utils import run_bass_kernel_spmd
from concourse.masks import make_identity

B, S, T, D = 8, 8, 256, 256
H, DK = 8, 64
HD = H * DK            # 512
L = 2
F = 256
R = 512
M = 128
EPS = 1e-5
NCORES = 8
P = 128

F32 = mybir.dt.float32
F32R = mybir.dt.float32r
BF16 = mybir.dt.bfloat16
FP8 = mybir.dt.float8e4
I32 = mybir.dt.int32
E4NP = ml_dtypes.float8_e4m3
BFNP = ml_dtypes.bfloat16
DRMODE = mybir.MatmulPerfMode.DoubleRow

NCH = L * H            # 16 chains per kv side

# fp8 scale plan (K side):  X*16, W1*64 -> psum 1024*pre1 ; h1'=psum/64 (16*h1)
# W2*2 -> psum2 32*pre2 ; h2' = max(psum2 + 32*b2, 0) (32*h2)
# W3*64 -> psum3 2048*kv ; stag = psum3/2048
XS = 16.0
W1S = 64.0
H1DIV = 64.0
W2S = 2.0
B2KS = 32.0
W3S = 64.0
L3KDIV = 2048.0
# V side: X*16 (bf16), W1*64 (bf16) -> psum 1024*pre1 ; h1 = psum/1024
V1DIV = 1024.0

# PBIAS layout (per-partition f32 scalars)
PB_FFB1 = 0                      # 4 per l
PB_DEB1 = PB_FFB1 + 4 * L
PB_DEB2 = PB_DEB1 + 2
PB_B2KP = PB_DEB2 + 2            # +32*b2k per (c, gt)
PB_B2KN = PB_B2KP + 2 * NCH      # -32*b2k
PB_B2VP = PB_B2KN + 2 * NCH      # +b2v
PB_B2VN = PB_B2VP + 2 * NCH      # -b2v
PB_COLS = PB_B2VN + 2 * NCH

# BCAST layout (free-dim vectors, bf16, replicated across partitions)
BC_DSB = 0
BC_B3V8 = BC_DSB + HD            # S * vb3 per l
BC_FFB2 = BC_B3V8 + L * HD
BC_DEB3 = BC_FFB2 + L * HD
BC_LN1G = BC_DEB3 + R
BC_LN1B = BC_LN1G + L * HD
BC_LN2G = BC_LN1B + L * HD
BC_LN2B = BC_LN2G + L * HD
BC_COLS = BC_LN2B + L * HD


def _chunks(nrowc):
    out = []
    col = 0
    while col < nrowc:
        w = min(512, nrowc - col)
        out.append((col, w))
        col += w
    return out


class _RR:
    def __init__(self, pattern):
        self.pattern = pattern
        self.i = 0

    def next(self):
        e = self.pattern[self.i % len(self.pattern)]
        self.i += 1
        return e


def _emit(nc, tc, t, i_val, nrowc, sec_starts, has_b2):
    (XT8, XTB, XTUB, PREDT, DSW, W1K8, W2K8, W3K8, W1LB,
     W1V, W2V, W3V, FFW1, FFW2, DEW1, DEW2, DEW3,
     PBIAS, BCAST, IDX, OUT, KV0D, KD1, VD1A, VD1B) = t

    NRT = nrowc // P
    chunks = _chunks(nrowc)
    relu_rr = _RR("AADAD")      # Pool cannot read PSUM on TRN2
    copy_rr = _RR("AD")

    with ExitStack() as ctx:
        cp = ctx.enter_context(tc.tile_pool(name="const", bufs=1))
        wp = ctx.enter_context(tc.tile_pool(name="w", bufs=3))
        sp = ctx.enter_context(tc.tile_pool(name="stag", bufs=2))
        hp = ctx.enter_context(tc.tile_pool(name="h", bufs=2))
        gp0 = ctx.enter_context(tc.tile_pool(name="gath0", bufs=1))
        gp1 = ctx.enter_context(tc.tile_pool(name="gath1", bufs=2))
        ap = ctx.enter_context(tc.tile_pool(name="attn", bufs=1))
        pp = ctx.enter_context(tc.tile_pool(name="ps", bufs=4, space="PSUM"))
        pq3 = ctx.enter_context(tc.tile_pool(name="ps3", bufs=2, space="PSUM"))
        pqa = ctx.enter_context(tc.tile_pool(name="pa", bufs=2, space="PSUM"))

        # ---- resident loads; chain-0 critical path first ----
        xt8 = cp.tile([P, 2, nrowc], FP8)
        nc.sync.dma_start(xt8[:, :, 0:256], XT8.ap()[:, :, 0:256])
        nc.scalar.dma_start(xt8[:, :, 256:512], XT8.ap()[:, :, 256:512])
        xtub = cp.tile([97, nrowc], BF16)
        nc.scalar.dma_start(xtub[0:97:32, :], XTUB.ap())
        pbias = cp.tile([P, PB_COLS], F32)
        nc.gpsimd.dma_start(pbias[:], PBIAS.ap())
        for col, w in chunks:
            if col == 0:
                continue
            nc.scalar.dma_start(xt8[:, :, col:col + w], XT8.ap()[:, :, col:col + w])
        idx = cp.tile([P, 16], I32)
        nc.gpsimd.dma_start(idx[:], IDX.ap())
        xtb = cp.tile([P, 2, nrowc], BF16)
        for col, w in chunks:
            nc.scalar.dma_start(xtb[:, :, col:col + w], XTB.ap()[:, :, col:col + w])
        predt = cp.tile([P, 2, M], F32R)
        nc.gpsimd.dma_start(predt[:], PREDT.ap())
        dsw = cp.tile([P, 2, HD], F32R)
        nc.gpsimd.dma_start(dsw[:], DSW.ap())
        bcast = cp.tile([P, BC_COLS], BF16)
        nc.gpsimd.dma_start(bcast[:], BCAST.ap())
        ffw1 = cp.tile([P, L, 4, HD], BF16)
        nc.gpsimd.dma_start(ffw1[:], FFW1.ap().rearrange("l p a h -> p l a h"))
        ffw2 = cp.tile([P, L, 4, HD], BF16)
        nc.gpsimd.dma_start(ffw2[:], FFW2.ap().rearrange("l p a h -> p l a h"))
        dew1 = cp.tile([P, 4, F], BF16)
        nc.gpsimd.dma_start(dew1[:], DEW1.ap())
        dew2 = cp.tile([P, 2, F], BF16)
        nc.gpsimd.dma_start(dew2[:], DEW2.ap())
        dew3 = cp.tile([P, 2, R], BF16)
        nc.gpsimd.dma_start(dew3[:], DEW3.ap())
        ident = cp.tile([P, P], F32)
        make_identity(nc, ident[:])
        epst = cp.tile([P, 1], F32)
        nc.vector.memset(epst[:], EPS)

        def relu_out(ps_ap, out_ap, scale=None, bias=None):
            e = relu_rr.next()
            if e == "A":
                if bias is None:
                    nc.scalar.activation(out=out_ap, in_=ps_ap,
                                         func=mybir.ActivationFunctionType.Relu,
                                         scale=scale)
                else:
                    nc.scalar.activation(out=out_ap, in_=ps_ap,
                                         func=mybir.ActivationFunctionType.Relu,
                                         bias=pbias[:, bias[0]:bias[0] + 1])
            else:
                eng = nc.vector if e == "D" else nc.gpsimd
                if bias is None:
                    eng.tensor_scalar(out=out_ap, in0=ps_ap,
                                      scalar1=float(scale), scalar2=0.0,
                                      op0=mybir.AluOpType.mult,
                                      op1=mybir.AluOpType.max)
                else:
                    eng.tensor_scalar(out=out_ap, in0=ps_ap,
                                      scalar1=pbias[:, bias[1]:bias[1] + 1],
                                      scalar2=pbias[:, bias[0]:bias[0] + 1],
                                      op0=mybir.AluOpType.max,
                                      op1=mybir.AluOpType.add)

        def copy_out(ps_ap, out_ap, scale):
            e = copy_rr.next()
            if e == "A":
                nc.scalar.activation(out=out_ap, in_=ps_ap,
                                     func=mybir.ActivationFunctionType.Copy,
                                     scale=scale)
            else:
                eng = nc.vector if e == "D" else nc.gpsimd
                eng.tensor_scalar(out=out_ap, in0=ps_ap,
                                  scalar1=float(scale), scalar2=0.0,
                                  op0=mybir.AluOpType.mult,
                                  op1=mybir.AluOpType.add)

        def k_chain(c, h, stag):
            """fp8 DoubleRow chain, software-pipelined:
            L1(ci) -> L2(ci-1) -> L3(ci-2) so relus hide under matmuls."""
            w1 = wp.tile([P, 2, F], FP8, tag="w1k")
            nc.sync.dma_start(w1[:], W1K8.ap()[c])
            w2 = wp.tile([P, 2, F], FP8, tag="w2k")
            nc.sync.dma_start(w2[:], W2K8.ap()[c])
            w3 = wp.tile([P, 2, DK], FP8, tag="w3k")
            nc.sync.dma_start(w3[:], W3K8.ap()[c])
            w1l = wp.tile([97, F], BF16, tag="w1lk")
            nc.sync.dma_start(w1l[0:97:32, :], W1LB.ap()[c])

            st = {}

            def l1(ci):
                col, cw = chunks[ci]
                h1t = hp.tile([P, 2, 512], FP8, tag="h1k")
                # rank-1 u-terms OPEN the full psum region (a start=False
                # accumulate spanning two DR-opened regions corrupts psum, so
                # open wide first, then let the DR mains accumulate densely)
                ps = []
                for ft in range(2):
                    fsl = slice(ft * P, (ft + 1) * P)
                    rg = 32 * ft
                    ps1 = pp.tile([P, 512], F32, tag="ps")
                    ps.append(ps1)
                    nc.tensor.matmul(ps1[:, 0:cw], w1l[rg:rg + 1, fsl],
                                     xtub[rg:rg + 1, col:col + cw],
                                     start=True, stop=False, tile_position=(rg, 0))
                for ft in range(2):
                    fsl = slice(ft * P, (ft + 1) * P)
                    for cc in range(0, cw, 256):
                        ccw = min(256, cw - cc)
                        csl = slice(col + cc, col + cc + ccw)
                        nc.tensor.matmul(ps[ft][:, cc:cc + ccw], w1[:, :, fsl],
                                         xt8[:, :, csl], start=False,
                                         stop=(cc + 256 >= cw),
                                         perf_mode=DRMODE, skip_group_check=True)
                for ft in range(2):
                    relu_out(ps[ft][:, 0:cw], h1t[:, ft, 0:cw], scale=1.0 / H1DIV)
                st[ci] = [h1t, None]

            def l2(ci):
                col, cw = chunks[ci]
                h1t = st[ci][0]
                h2t = hp.tile([P, 2, 512], FP8, tag="h2k")
                ps = []
                for gt in range(2):
                    gsl = slice(gt * P, (gt + 1) * P)
                    ps2 = pp.tile([P, 512], F32, tag="ps")
                    ps.append(ps2)
                    for cc in range(0, cw, 256):
                        ccw = min(256, cw - cc)
                        nc.tensor.matmul(ps2[:, cc:cc + ccw], w2[:, :, gsl],
                                         h1t[:, :, cc:cc + ccw], start=True,
                                         stop=True, perf_mode=DRMODE)
                for gt in range(2):
                    relu_out(ps[gt][:, 0:cw], h2t[:, gt, 0:cw], scale=1.0,
                             bias=((PB_B2KP + 2 * c + gt, PB_B2KN + 2 * c + gt)
                                   if has_b2 else None))
                st[ci][1] = h2t

            def l3(ci):
                col, cw = chunks[ci]
                nrti = cw // P
                h2t = st[ci][1]
                ps3 = pq3.tile([P, 256], F32, tag="ps3")
                for rti in range(nrti):
                    rsl = slice(rti * P, (rti + 1) * P)
                    nc.tensor.matmul(ps3[:, rti * DK:(rti + 1) * DK],
                                     h2t[:, :, rsl], w3[:, :, :],
                                     start=True, stop=True, perf_mode=DRMODE)
                copy_out(ps3[:, 0:nrti * DK].rearrange("p (r d) -> p r d", d=DK),
                         stag[:, col // P:col // P + nrti, h * DK:(h + 1) * DK],
                         1.0 / L3KDIV)

            _pipeline(l1, l2, l3)

        def _pipeline(l1, l2, l3):
            n = len(chunks)
            for ci in range(n):
                l1(ci)
                if ci >= 1:
                    l2(ci - 1)
                if ci >= 2:
                    l3(ci - 2)
            l2(n - 1)
            if n >= 2:
                l3(n - 2)
            l3(n - 1)

        def v_chain(c, h, stag):
            w1 = wp.tile([P, 2, F], BF16, tag="w1v")
            nc.scalar.dma_start(w1[:], W1V.ap()[c])
            w2 = wp.tile([P, 2, F], BF16, tag="w2v")
            nc.scalar.dma_start(w2[:], W2V.ap()[c])
            w3 = wp.tile([P, 2, DK], BF16, tag="w3v")
            nc.scalar.dma_start(w3[:], W3V.ap()[c])
            w1l = wp.tile([97, F], BF16, tag="w1lv")
            nc.scalar.dma_start(w1l[0:97:32, :], W1LB.ap()[NCH + c])

            st = {}

            def l1(ci):
                col, cw = chunks[ci]
                h1t = hp.tile([P, 2, 512], BF16, tag="h1v")
                ps = []
                for ft in range(2):
                    fsl = slice(ft * P, (ft + 1) * P)
                    rg = 32 * ft
                    ps1 = pp.tile([P, 512], F32, tag="ps")
                    ps.append(ps1)
                    nc.tensor.matmul(ps1[:, 0:cw], w1l[rg:rg + 1, fsl],
                                     xtub[rg:rg + 1, col:col + cw],
                                     start=True, stop=False, tile_position=(rg, 0))
                for ft in range(2):
                    fsl = slice(ft * P, (ft + 1) * P)
                    nc.tensor.matmul(ps[ft][:, 0:cw], w1[:, 0, fsl],
                                     xtb[:, 0, col:col + cw], start=False, stop=False,
                                     skip_group_check=True)
                    nc.tensor.matmul(ps[ft][:, 0:cw], w1[:, 1, fsl],
                                     xtb[:, 1, col:col + cw], start=False, stop=True,
                                     skip_group_check=True)
                for ft in range(2):
                    relu_out(ps[ft][:, 0:cw], h1t[:, ft, 0:cw], scale=1.0 / V1DIV)
                st[ci] = [h1t, None]

            def l2(ci):
                col, cw = chunks[ci]
                h1t = st[ci][0]
                h2t = hp.tile([P, 2, 512], BF16, tag="h2v")
                ps = []
                for gt in range(2):
                    gsl = slice(gt * P, (gt + 1) * P)
                    ps2 = pp.tile([P, 512], F32, tag="ps")
                    ps.append(ps2)
                    nc.tensor.matmul(ps2[:, 0:cw], w2[:, 0, gsl],
                                     h1t[:, 0, 0:cw], start=True, stop=False)
                    nc.tensor.matmul(ps2[:, 0:cw], w2[:, 1, gsl],
                                     h1t[:, 1, 0:cw], start=False, stop=True)
                for gt in range(2):
                    relu_out(ps[gt][:, 0:cw], h2t[:, gt, 0:cw], scale=1.0,
                             bias=((PB_B2VP + 2 * c + gt, PB_B2VN + 2 * c + gt)
                                   if has_b2 else None))
                st[ci][1] = h2t

            def l3(ci):
                col, cw = chunks[ci]
                nrti = cw // P
                h2t = st[ci][1]
                ps3 = pq3.tile([P, 256], F32, tag="ps3")
                for rti in range(nrti):
                    rsl = slice(rti * P, (rti + 1) * P)
                    nc.tensor.matmul(ps3[:, rti * DK:(rti + 1) * DK],
                                     h2t[:, 0, rsl], w3[:, 0, :],
                                     start=True, stop=False)
                    nc.tensor.matmul(ps3[:, rti * DK:(rti + 1) * DK],
                                     h2t[:, 1, rsl], w3[:, 1, :],
                                     start=False, stop=True)
                copy_out(ps3[:, 0:nrti * DK].rearrange("p (r d) -> p r d", d=DK),
                         stag[:, col // P:col // P + nrti, h * DK:(h + 1) * DK],
                         1.0)

            _pipeline(l1, l2, l3)

        def ds_project():
            ps = pqa.tile([P, HD], F32, tag="pa")
            nc.tensor.matmul(ps[:], predt[:, 0, :], dsw[:, 0, :], start=True, stop=False)
            nc.tensor.matmul(ps[:], predt[:, 1, :], dsw[:, 1, :], start=False, stop=True)
            av0 = ap.tile([P, HD], F32, tag="av")
            nc.vector.tensor_tensor(out=av0[:], in0=ps[:],
                                    in1=bcast[:, BC_DSB:BC_DSB + HD],
                                    op=mybir.AluOpType.add)
            return av0

        def layer_norm(src, gcol, bcol, out):
            stats = ap.tile([P, 6], F32, tag="lnstat")
            nc.vector.bn_stats(stats[:], src[:])
            mv = ap.tile([P, 2], F32, tag="lnmv")
            nc.vector.bn_aggr(mv[:], stats[:])
            rstd = ap.tile([P, 1], F32, tag="lnrstd")
            nc.scalar.activation(out=rstd[:], in_=mv[:, 1:2],
                                 func=mybir.ActivationFunctionType.Sqrt,
                                 bias=epst[:, 0:1])
            nc.vector.reciprocal(rstd[:], rstd[:])
            nc.vector.tensor_scalar(out=out[:], in0=src[:], scalar1=mv[:, 0:1],
                                    scalar2=rstd[:, 0:1],
                                    op0=mybir.AluOpType.subtract,
                                    op1=mybir.AluOpType.mult)
            nc.vector.tensor_tensor(out=out[:], in0=out[:],
                                    in1=bcast[:, gcol:gcol + HD],
                                    op=mybir.AluOpType.mult)
            nc.vector.tensor_tensor(out=out[:], in0=out[:],
                                    in1=bcast[:, bcol:bcol + HD],
                                    op=mybir.AluOpType.add)

        def transpose_to(src, dst):
            for kt in range(4):
                tp_ = pqa.tile([P, P], F32, tag="pa")
                nc.tensor.transpose(tp_[:], src[:, kt * P:(kt + 1) * P], ident[:])
                nc.scalar.activation(out=dst[:, kt, :], in_=tp_[:],
                                     func=mybir.ActivationFunctionType.Copy)

        def gather0(gall):
            """l=0: combined K+V rows from KV0D into [P, 16, 2, HD]."""
            for j in range(16):
                n, s = j // 8, j % 8
                if n == 1 and s < i_val:
                    sec = sec_starts[s]
                    nc.sync.dma_start(gall[:, j, :, :], KV0D.ap()[sec:sec + P, :, :])
                else:
                    # indirect DMA requires 2D src/dst APs (3D reads garbage)
                    nc.gpsimd.indirect_dma_start(
                        out=gall[:, j, :, :].rearrange("p a hd -> p (a hd)"),
                        out_offset=None,
                        in_=KV0D.ap().rearrange("r a hd -> r (a hd)"),
                        in_offset=bass.IndirectOffsetOnAxis(ap=idx[:, j:j + 1], axis=0))

        def gather1(src, gall):
            """l=1: one of KD1/VD1 into [P, 16, HD]."""
            for j in range(16):
                n, s = j // 8, j % 8
                if n == 1 and s < i_val:
                    sec = sec_starts[s]
                    nc.sync.dma_start(gall[:, j, :], src.ap()[sec:sec + P, :])
                else:
                    nc.gpsimd.indirect_dma_start(
                        out=gall[:, j, :], out_offset=None, in_=src.ap(),
                        in_offset=bass.IndirectOffsetOnAxis(ap=idx[:, j:j + 1], axis=0))

        def attn_phase1(l, av_in, wall, kg):
            """kg: callable js_slice -> [P, 4, HD] bf16 K rows."""
            avbf = ap.tile([P, HD], BF16, tag="avbf")
            nc.vector.tensor_copy(out=avbf[:], in_=av_in[:])
            scrb = ap.tile([P, 4, HD], BF16, tag="scrb")
            logits = ap.tile([P, 16, 8], BF16, tag="logits")
            avb = avbf[:, None, :].to_broadcast([P, 4, HD])
            for js in range(4):
                jsl = slice(js * 4, (js + 1) * 4)
                nc.vector.tensor_tensor(out=scrb[:], in0=kg(jsl),
                                        in1=avb, op=mybir.AluOpType.mult)
                with nc.allow_low_precision(reason="logits bf16; sigmoid smooths"):
                    nc.vector.tensor_reduce(
                        out=logits[:, jsl, :],
                        in_=scrb[:].rearrange("p a (h d) -> p (a h) d", d=DK),
                        axis=mybir.AxisListType.X, op=mybir.AluOpType.add)
            delta = ap.tile([P, 8, 8], BF16, tag="delta")
            nc.vector.tensor_tensor(out=delta[:], in0=logits[:, 0:8, :],
                                    in1=logits[:, 8:16, :],
                                    op=mybir.AluOpType.subtract)
            scale = DK ** (-0.5)
            dflat = delta[:].rearrange("p a b -> p (a b)")
            nc.scalar.activation(out=wall[:, 0:64], in_=dflat,
                                 func=mybir.ActivationFunctionType.Sigmoid,
                                 scale=scale)
            nc.scalar.activation(out=wall[:, 64:128], in_=dflat,
                                 func=mybir.ActivationFunctionType.Sigmoid,
                                 scale=-scale)

        def phase2_step(l, wall, vg, js, att, part, scr, half=None):
            jsl = slice(js * 4, (js + 1) * 4)
            wv = wall[:].rearrange("p (j h) -> p j h", h=8)
            if half is None:
                hsl, nh = slice(0, HD), 8
            else:
                hsl, nh = slice(half * (HD // 2), (half + 1) * (HD // 2)), 4
            wvh = wv[:, jsl, half * 4:half * 4 + 4] if half is not None \
                else wv[:, jsl, :]
            nc.vector.tensor_tensor(
                out=scr[:, :, hsl].rearrange("p a (h d) -> p a h d", d=DK),
                in0=vg(jsl).rearrange("p a (h d) -> p a h d", d=DK),
                in1=wvh[:, :, :, None].to_broadcast([P, 4, nh, DK]),
                op=mybir.AluOpType.mult)
            dst = att if js == 0 else part
            nc.vector.tensor_reduce(
                out=dst[:, hsl],
                in_=scr[:, :, hsl].rearrange("p a hd -> p hd a"),
                axis=mybir.AxisListType.X, op=mybir.AluOpType.add)
            if js > 0:
                nc.vector.tensor_tensor(out=att[:, hsl], in0=att[:, hsl],
                                        in1=part[:, hsl],
                                        op=mybir.AluOpType.add)

        def attn_phase2_tail(l, avp, att):
            nc.vector.tensor_tensor(out=att[:], in0=att[:], in1=avp[:],
                                    op=mybir.AluOpType.add)
            xn = ap.tile([P, HD], F32, tag="xn")
            layer_norm(att, BC_LN1G + l * HD, BC_LN1B + l * HD, xn)
            xT = ap.tile([P, 4, P], BF16, tag="xT")
            transpose_to(xn, xT)
            ff1 = ap.tile([P, 4, P], BF16, tag="ff1")
            for ft in range(4):
                psf = pqa.tile([P, P], F32, tag="pa")
                for kt in range(4):
                    nc.tensor.matmul(psf[:], ffw1[:, l, kt, ft * P:(ft + 1) * P],
                                     xT[:, kt, :], start=(kt == 0), stop=(kt == 3))
                nc.scalar.activation(
                    out=ff1[:, ft, :], in_=psf[:],
                    func=mybir.ActivationFunctionType.Relu,
                    bias=pbias[:, PB_FFB1 + 4 * l + ft:PB_FFB1 + 4 * l + ft + 1])
            ps2 = pqa.tile([P, HD], F32, tag="pa")
            for kt in range(4):
                nc.tensor.matmul(ps2[:], ff1[:, kt, :], ffw2[:, l, kt, :],
                                 start=(kt == 0), stop=(kt == 3))
            ffx = ap.tile([P, HD], F32, tag="ffx")
            nc.vector.tensor_tensor(out=ffx[:], in0=ps2[:],
                                    in1=bcast[:, BC_FFB2 + l * HD:BC_FFB2 + (l + 1) * HD],
                                    op=mybir.AluOpType.add)
            nc.vector.tensor_tensor(out=ffx[:], in0=ffx[:], in1=xn[:],
                                    op=mybir.AluOpType.add)
            av_out = ap.tile([P, HD], F32, tag="av")
            layer_norm(ffx, BC_LN2G + l * HD, BC_LN2B + l * HD, av_out)
            return av_out

        def make_avp(l, av_in):
            avp = ap.tile([P, HD], F32, tag="avp")
            nc.vector.tensor_tensor(
                out=avp[:], in0=av_in[:],
                in1=bcast[:, BC_B3V8 + l * HD:BC_B3V8 + (l + 1) * HD],
                op=mybir.AluOpType.add)
            return avp

        # ================= schedule =================
        stag = sp.tile([P, NRT, HD], BF16, tag="stag")
        av = None
        for h in range(H):
            k_chain(0 * 8 + h, h, stag)
            if h == 0:
                av = ds_project()
        nc.sync.dma_start(
            KV0D.ap()[:, 0, :].rearrange("(rt p) hd -> p rt hd", p=P), stag[:])

        stag = sp.tile([P, NRT, HD], BF16, tag="stag")
        for h in range(H):
            v_chain(0 * 8 + h, h, stag)
        nc.sync.dma_start(
            KV0D.ap()[:, 1, :].rearrange("(rt p) hd -> p rt hd", p=P), stag[:])
        kv0gall = gp0.tile([P, 16, 2, HD], BF16, tag="gall0")
        gather0(kv0gall)

        # l=1 K chains; l=0 attention interleaved (phase2 spread over chains)
        stag = sp.tile([P, NRT, HD], BF16, tag="stag")
        wall0 = ap.tile([P, 128], F32, tag="wall")
        att0 = ap.tile([P, HD], F32, tag="att")
        part0 = ap.tile([P, HD], F32, tag="part")
        scr0 = ap.tile([P, 4, HD], F32, tag="scr")
        vg0 = lambda jsl: kv0gall[:, jsl, 1, :]
        avp0 = None
        for h in range(H):
            k_chain(1 * 8 + h, h, stag)
            if h == 1:
                attn_phase1(0, av, wall0, lambda jsl: kv0gall[:, jsl, 0, :])
                avp0 = make_avp(0, av)
            elif 2 <= h <= 5:
                phase2_step(0, wall0, vg0, h - 2, att0, part0, scr0)
        nc.sync.dma_start(
            KD1.ap().rearrange("(rt p) hd -> p rt hd", p=P), stag[:])
        k1gall = gp1.tile([P, 16, HD], BF16, tag="gall1")
        gather1(KD1, k1gall)
        av = attn_phase2_tail(0, avp0, att0)

        # l=1 V chains; l=1 phase1 interleaved. The stag is written to DRAM
        # in two head-halves so V gathers + weighted sums start while the
        # last four chains still compute.
        stag = sp.tile([P, NRT, HD], BF16, tag="stag")
        wall1 = ap.tile([P, 128], F32, tag="wall")
        att1 = ap.tile([P, HD], F32, tag="att")
        part1 = ap.tile([P, HD], F32, tag="part")
        scr1 = ap.tile([P, 4, HD], F32, tag="scr")
        v1gall = gp1.tile([P, 16, HD], BF16, tag="gall1")
        avp1 = None

        def v1_half(half, src_t):
            hsl = slice(half * (HD // 2), (half + 1) * (HD // 2))
            nc.sync.dma_start(
                src_t.ap().rearrange("(rt p) hd -> p rt hd", p=P),
                stag[:, :, hsl])
            for js in range(4):
                for j in range(js * 4, js * 4 + 4):
                    n, s = j // 8, j % 8
                    if n == 1 and s < i_val:
                        nc.sync.dma_start(
                            v1gall[:, j, hsl],
                            src_t.ap()[sec_starts[s]:sec_starts[s] + P, :])
                    else:
                        nc.gpsimd.indirect_dma_start(
                            out=v1gall[:, j, hsl], out_offset=None,
                            in_=src_t.ap(),
                            in_offset=bass.IndirectOffsetOnAxis(
                                ap=idx[:, j:j + 1], axis=0))
                phase2_step(1, wall1, lambda jsl: v1gall[:, jsl, hsl], js,
                            att1, part1, scr1, half=half)

        for h in range(H):
            v_chain(1 * 8 + h, h, stag)
            if h == 1:
                attn_phase1(1, av, wall1, lambda jsl: k1gall[:, jsl, :])
                avp1 = make_avp(1, av)
            elif h == 4:
                v1_half(0, VD1A)
        v1_half(1, VD1B)
        av = attn_phase2_tail(1, avp1, att1)

        # ---- dist extractor ----
        avT = ap.tile([P, 4, P], BF16, tag="avT")
        transpose_to(av, avT)
        h1 = ap.tile([P, 2, P], BF16, tag="deh1")
        for ft in range(2):
            psd = pqa.tile([P, P], F32, tag="pa")
            for kt in range(4):
                nc.tensor.matmul(psd[:], dew1[:, kt, ft * P:(ft + 1) * P],
                                 avT[:, kt, :], start=(kt == 0), stop=(kt == 3))
            nc.scalar.activation(out=h1[:, ft, :], in_=psd[:],
                                 func=mybir.ActivationFunctionType.Relu,
                                 bias=pbias[:, PB_DEB1 + ft:PB_DEB1 + ft + 1])
        h2 = ap.tile([P, 2, P], BF16, tag="deh2")
        for ft in range(2):
            psd = pqa.tile([P, P], F32, tag="pa")
            for kt in range(2):
                nc.tensor.matmul(psd[:], dew2[:, kt, ft * P:(ft + 1) * P],
                                 h1[:, kt, :], start=(kt == 0), stop=(kt == 1))
            nc.scalar.activation(out=h2[:, ft, :], in_=psd[:],
                                 func=mybir.ActivationFunctionType.Relu,
                                 bias=pbias[:, PB_DEB2 + ft:PB_DEB2 + ft + 1])
        pso = pqa.tile([P, R], F32, tag="pa")
        for kt in range(2):
            nc.tensor.matmul(pso[:], h2[:, kt, :], dew3[:, kt, :],
                             start=(kt == 0), stop=(kt == 1))
        o = ap.tile([P, R], F32, tag="out")
        nc.vector.tensor_tensor(out=o[:], in0=pso[:],
                                in1=bcast[:, BC_DEB3:BC_DEB3 + R],
                                op=mybir.AluOpType.add)
        nc.sync.dma_start(OUT.ap()[:, 0:R // 2], o[:, 0:R // 2])
        nc.scalar.dma_start(OUT.ap()[:, R // 2:], o[:, R // 2:])


_BUILD_CACHE = {}


def _build(i_val, nrowc, sec_starts, has_b2=False):
    key = (i_val, nrowc, tuple(sec_starts), has_b2)
    if key in _BUILD_CACHE:
        return _BUILD_CACHE[key]
    nc = bacc.Bacc("TRN2", target_bir_lowering=False, debug=False)
    t = []
    t.append(nc.dram_tensor("XT8", [P, 2, nrowc], FP8, kind="ExternalInput"))
    t.append(nc.dram_tensor("XTB", [P, 2, nrowc], BF16, kind="ExternalInput"))
    t.append(nc.dram_tensor("XTUB", [4, nrowc], BF16, kind="ExternalInput"))
    t.append(nc.dram_tensor("PREDT", [P, 2, M], F32R, kind="ExternalInput"))
    t.append(nc.dram_tensor("DSW", [P, 2, HD], F32R, kind="ExternalInput"))
    t.append(nc.dram_tensor("W1K8", [NCH, P, 2, F], FP8, kind="ExternalInput"))
    t.append(nc.dram_tensor("W2K8", [NCH, P, 2, F], FP8, kind="ExternalInput"))
    t.append(nc.dram_tensor("W3K8", [NCH, P, 2, DK], FP8, kind="ExternalInput"))
    t.append(nc.dram_tensor("W1LB", [2 * NCH, 4, F], BF16, kind="ExternalInput"))
    t.append(nc.dram_tensor("W1V", [NCH, P, 2, F], BF16, kind="ExternalInput"))
    t.append(nc.dram_tensor("W2V", [NCH, P, 2, F], BF16, kind="ExternalInput"))
    t.append(nc.dram_tensor("W3V", [NCH, P, 2, DK], BF16, kind="ExternalInput"))
    t.append(nc.dram_tensor("FFW1", [L, P, 4, HD], BF16, kind="ExternalInput"))
    t.append(nc.dram_tensor("FFW2", [L, P, 4, HD], BF16, kind="ExternalInput"))
    t.append(nc.dram_tensor("DEW1", [P, 4, F], BF16, kind="ExternalInput"))
    t.append(nc.dram_tensor("DEW2", [P, 2, F], BF16, kind="ExternalInput"))
    t.append(nc.dram_tensor("DEW3", [P, 2, R], BF16, kind="ExternalInput"))
    t.append(nc.dram_tensor("PBIAS", [P, PB_COLS], F32, kind="ExternalInput"))
    t.append(nc.dram_tensor("BCAST", [P, BC_COLS], BF16, kind="ExternalInput"))
    t.append(nc.dram_tensor("IDX", [P, 16], I32, kind="ExternalInput"))
    t.append(nc.dram_tensor("OUT", [M, R], F32, kind="ExternalOutput"))
    t.append(nc.dram_tensor("KV0D", [nrowc, 2, HD], BF16))
    t.append(nc.dram_tensor("KD1", [nrowc, HD], BF16))
    t.append(nc.dram_tensor("VD1A", [nrowc, HD // 2], BF16))
    t.append(nc.dram_tensor("VD1B", [nrowc, HD // 2], BF16))
    with tile.TileContext(nc) as tc:
        _emit(nc, tc, t, i_val, nrowc, sec_starts, has_b2)
    nc.compile()
    _BUILD_CACHE[key] = nc
    return nc


def _fp8(x, scale):
    return np.clip(np.asarray(x, np.float32) * scale, -240.0, 240.0).astype(E4NP)


def plan_compaction(ins):
    i = int(ins["i"])
    left = np.asarray(ins["left_idx"], np.int64)
    right = np.asarray(ins["right_idx"], np.int64)
    m_ar = np.arange(M, dtype=np.int64)
    tsets = []
    for s in range(S):
        tset = np.union1d(left, m_ar) if s < i else np.union1d(left, right)
        tsets.append(np.sort(tset))
    sec_starts = []
    rows = []
    rowmap = {}
    for s in range(S):
        sec_starts.append(len(rows))
        for tt in tsets[s]:
            rowmap[(s, int(tt))] = len(rows)
            rows.append((s, int(tt)))
    nraw = len(rows)
    nrowc = ((nraw + P - 1) // P) * P
    idxm = np.zeros((P, 16), np.int32)
    for s in range(S):
        for mm in range(M):
            idxm[mm, 0 * 8 + s] = rowmap[(s, int(left[mm]))]
            t1 = mm if s < i else int(right[mm])
            idxm[mm, 1 * 8 + s] = rowmap[(s, t1)]
    return {
        "i": i, "rows": rows, "nrowc": nrowc,
        "sec_starts": tuple(sec_starts[:i]), "idx": idxm,
    }


def _prep_shared(ins, plan):
    f32 = np.float32
    kW1, kW2, kW3 = ins["kW1"], ins["kW2"], ins["kW3"]
    vW1, vW2, vW3 = ins["vW1"], ins["vW2"], ins["vW3"]
    kb1, kb2 = ins["kb1"], ins["kb2"]
    vb1, vb2, vb3 = ins["vb1"], ins["vb2"], ins["vb3"]
    if np.any(np.asarray(kb1, np.float32)) or np.any(np.asarray(vb1, np.float32)):
        raise NotImplementedError("nonzero kb1/vb1 not folded (u-row rank-1 "
                                  "carries no bias term)")

    W1K8 = np.empty((NCH, P, 2, F), E4NP)
    W2K8 = np.empty((NCH, P, 2, F), E4NP)
    W3K8 = np.empty((NCH, P, 2, DK), E4NP)
    W1V = np.empty((NCH, P, 2, F), BFNP)
    W2V = np.empty((NCH, P, 2, F), BFNP)
    W3V = np.empty((NCH, P, 2, DK), BFNP)
    W1LB = np.empty((2 * NCH, 4, F), BFNP)
    PB = np.zeros((P, PB_COLS), f32)
    BC = np.zeros((BC_COLS,), f32)

    def pack2(w, ncols):
        return np.asarray(w, f32).reshape(2, P, ncols).transpose(1, 0, 2)

    for l in range(L):
        for h in range(H):
            c = l * 8 + h
            W1K8[c] = _fp8(pack2(kW1[l, h][:256], F), W1S)
            W2K8[c] = _fp8(pack2(kW2[l, h], F), W2S)
            W3K8[c] = _fp8(pack2(kW3[l, h], DK), W3S)
            W1LB[c] = np.broadcast_to(
                (np.asarray(kW1[l, h][256], f32) * W1S).astype(BFNP), (4, F))
            W1V[c] = (pack2(vW1[l, h][:256], F) * 64.0).astype(BFNP)
            W2V[c] = pack2(vW2[l, h], F).astype(BFNP)
            W3V[c] = pack2(vW3[l, h], DK).astype(BFNP)
            W1LB[NCH + c] = np.broadcast_to(
                (np.asarray(vW1[l, h][256], f32) * W1S).astype(BFNP), (4, F))
            for gt in range(2):
                gsl = slice(gt * P, (gt + 1) * P)
                PB[:, PB_B2KP + 2 * c + gt] = B2KS * np.asarray(kb2[l, h][gsl], f32)
                PB[:, PB_B2KN + 2 * c + gt] = -B2KS * np.asarray(kb2[l, h][gsl], f32)
                PB[:, PB_B2VP + 2 * c + gt] = np.asarray(vb2[l, h][gsl], f32)
                PB[:, PB_B2VN + 2 * c + gt] = -np.asarray(vb2[l, h][gsl], f32)

    DSW = np.asarray(ins["ds_W"], f32).reshape(2, P, HD).transpose(1, 0, 2).copy()
    BC[BC_DSB:BC_DSB + HD] = np.asarray(ins["ds_b"], f32)
    for l in range(L):
        BC[BC_B3V8 + l * HD:BC_B3V8 + (l + 1) * HD] = \
            S * np.asarray(vb3[l], f32).reshape(HD)

    FFW1 = np.empty((L, P, 4, HD), BFNP)
    FFW2 = np.empty((L, P, 4, HD), BFNP)
    for l in range(L):
        FFW1[l] = np.asarray(ins["ffW1"][l], f32).reshape(4, P, HD).transpose(1, 0, 2).astype(BFNP)
        FFW2[l] = np.asarray(ins["ffW2"][l], f32).reshape(4, P, HD).transpose(1, 0, 2).astype(BFNP)
        for ft in range(4):
            PB[:, PB_FFB1 + 4 * l + ft] = np.asarray(ins["ffb1"][l][ft * P:(ft + 1) * P], f32)
        BC[BC_FFB2 + l * HD:BC_FFB2 + (l + 1) * HD] = np.asarray(ins["ffb2"][l], f32)
        BC[BC_LN1G + l * HD:BC_LN1G + (l + 1) * HD] = np.asarray(ins["ln1_g"][l], f32)
        BC[BC_LN1B + l * HD:BC_LN1B + (l + 1) * HD] = np.asarray(ins["ln1_b"][l], f32)
        BC[BC_LN2G + l * HD:BC_LN2G + (l + 1) * HD] = np.asarray(ins["ln2_g"][l], f32)
        BC[BC_LN2B + l * HD:BC_LN2B + (l + 1) * HD] = np.asarray(ins["ln2_b"][l], f32)

    DEW1 = np.asarray(ins["deW1"], f32).reshape(4, P, F).transpose(1, 0, 2).astype(BFNP)
    DEW2 = np.asarray(ins["deW2"], f32).reshape(2, P, F).transpose(1, 0, 2).astype(BFNP)
    DEW3 = np.asarray(ins["deW3"], f32).reshape(2, P, R).transpose(1, 0, 2).astype(BFNP)
    for ft in range(2):
        PB[:, PB_DEB1 + ft] = np.asarray(ins["deb1"][ft * P:(ft + 1) * P], f32)
        PB[:, PB_DEB2 + ft] = np.asarray(ins["deb2"][ft * P:(ft + 1) * P], f32)
    BC[BC_DEB3:BC_DEB3 + R] = np.asarray(ins["deb3"], f32)

    BCAST = np.broadcast_to(BC.astype(BFNP), (P, BC_COLS)).copy()

    return {
        "W1K8": W1K8, "W2K8": W2K8, "W3K8": W3K8, "W1LB": W1LB,
        "W1V": W1V, "W2V": W2V, "W3V": W3V,
        "DSW": DSW, "FFW1": FFW1, "FFW2": FFW2,
        "DEW1": DEW1, "DEW2": DEW2, "DEW3": DEW3,
        "PBIAS": PB, "BCAST": BCAST, "IDX": plan["idx"],
    }


def make_in_maps(ins, plan=None):
    if plan is None:
        plan = plan_compaction(ins)
    shared = _prep_shared(ins, plan)
    enc = np.asarray(ins["encoded"], np.float32)
    tu = np.asarray(ins["true_u"], np.float32)
    mid = np.asarray(ins["mid_idx"], np.int64)
    i = plan["i"]
    nrowc = plan["nrowc"]
    rows = plan["rows"]
    s_idx = np.array([r[0] for r in rows], np.int64)
    t_idx = np.array([r[1] for r in rows], np.int64)

    in_maps = []
    for b in range(B):
        xg = np.zeros((nrowc, D), np.float32)
        xg[:len(rows)] = enc[b][s_idx, t_idx]
        ug = np.zeros((nrowc,), np.float32)
        ug[:len(rows)] = tu[b][s_idx, t_idx]
        xs16 = xg.T.reshape(2, P, nrowc).transpose(1, 0, 2) * XS
        xub = np.broadcast_to((ug * XS).astype(BFNP), (4, nrowc))
        pred = enc[b, i][mid]
        predt = pred.T.reshape(2, P, M).transpose(1, 0, 2).copy()
        m = dict(shared)
        m["XT8"] = _fp8(xs16, 1.0)
        m["XTB"] = xs16.astype(BFNP)
        m["XTUB"] = np.ascontiguousarray(xub)
        m["PREDT"] = predt
        in_maps.append(m)
    return in_maps


def kernel(**inputs):
    ins = {k: np.asarray(v) for k, v in inputs.items()}
    plan = plan_compaction(ins)
    in_maps = make_in_maps(ins, plan)
    has_b2 = bool(np.any(np.asarray(ins["kb2"], np.float32))
                  or np.any(np.asarray(ins["vb2"], np.float32)))
    nc = _build(plan["i"], plan["nrowc"], plan["sec_starts"], has_b2)
    res = run_bass_kernel_spmd(nc, in_maps, core_ids=list(range(NCORES)))
    out = np.stack([res.results[c]["OUT"] for c in range(NCORES)])
    return out.astype(np.float32)


# revision 19
# speedup vs baseline: 1.0307x; 1.0307x over previous
"""Trainium2 Bass kernel for nn_AttentionalCopula (sparse_attention).

Sharding: data-parallel over batch (B=8 -> 8 cores); per-head K/V MLP stacks
computed locally per core. Weights replicated. No collectives.

Key optimizations over the v1 baseline:
  * Row compaction: the attention only gathers K/V rows from
    union_s({left} u ({arange} if s<i else {right})) -- ~1350 of 2048 rows.
    The MLP chains run only on those rows.
  * K chains in fp8 (float8e4) with MatmulPerfMode.DoubleRow: K_eff=256 per
    matmul at 0.5 cyc/col (4x bf16). V chains stay bf16 (V values feed the
    output directly; fp8 there fails the accuracy gate -- measured).
  * The u-row (feature 257) enters L1 psum via a K=1 bf16 rank-1 matmul
    (tile_position-paired across the two f-tiles).
  * Relu/copy work is round-robined across ACT/DVE/Pool engines.
  * l=0 K/V staged interleaved in DRAM so one indirect gather per slot
    fetches both; contiguous slots (n=1, s<i) use direct DMA.
"""

from contextlib import ExitStack

import ml_dtypes
import numpy as np

import concourse.bass as bass
import concourse.mybir as mybir
import concourse.tile as tile
from concourse import bacc
from concourse.bass_utils import run_bass_kernel_spmd
from concourse.masks import make_identity

B, S, T, D = 8, 8, 256, 256
H, DK = 8, 64
HD = H * DK            # 512
L = 2
F = 256
R = 512
M = 128
EPS = 1e-5
NCORES = 8
P = 128

F32 = mybir.dt.float32
F32R = mybir.dt.float32r
BF16 = mybir.dt.bfloat16
FP8 = mybir.dt.float8e4
I32 = mybir.dt.int32
E4NP = ml_dtypes.float8_e4m3
BFNP = ml_dtypes.bfloat16
DRMODE = mybir.MatmulPerfMode.DoubleRow

NCH = L * H            # 16 chains per kv side

# fp8 scale plan (K side):  X*16, W1*64 -> psum 1024*pre1 ; h1'=psum/64 (16*h1)
# W2*2 -> psum2 32*pre2 ; h2' = max(psum2 + 32*b2, 0) (32*h2)
# W3*64 -> psum3 2048*kv ; stag = psum3/2048
XS = 16.0
W1S = 64.0
H1DIV = 64.0
W2S = 2.0
B2KS = 32.0
W3S = 64.0
L3KDIV = 2048.0
# V side: X*16 (bf16), W1*64 (bf16) -> psum 1024*pre1 ; h1 = psum/1024
V1DIV = 1024.0

# PBIAS layout (per-partition f32 scalars)
PB_FFB1 = 0                      # 4 per l
PB_DEB1 = PB_FFB1 + 4 * L
PB_DEB2 = PB_DEB1 + 2
PB_B2KP = PB_DEB2 + 2            # +32*b2k per (c, gt)
PB_B2KN = PB_B2KP + 2 * NCH      # -32*b2k
PB_B2VP = PB_B2KN + 2 * NCH      # +b2v
PB_B2VN = PB_B2VP + 2 * NCH      # -b2v
PB_COLS = PB_B2VN + 2 * NCH

# BCAST layout (free-dim vectors, bf16, replicated across partitions)
BC_DSB = 0
BC_B3V8 = BC_DSB + HD            # S * vb3 per l
BC_FFB2 = BC_B3V8 + L * HD
BC_DEB3 = BC_FFB2 + L * HD
BC_LN1G = BC_DEB3 + R
BC_LN1B = BC_LN1G + L * HD
BC_LN2G = BC_LN1B + L * HD
BC_LN2B = BC_LN2G + L * HD
BC_COLS = BC_LN2B + L * HD


def _chunks(nrowc):
    out = []
    col = 0
    while col < nrowc:
        w = min(512, nrowc - col)
        out.append((col, w))
        col += w
    return out


class _RR:
    def __init__(self, pattern):
        self.pattern = pattern
        self.i = 0

    def next(self):
        e = self.pattern[self.i % len(self.pattern)]
        self.i += 1
        return e


def _emit(nc, tc, t, i_val, nrowc, sec_starts, has_b2):
    (XT8, XTB, XTUB, PREDT, DSW, W1K8, W2K8, W3K8, W1LB,
     W1V, W2V, W3V, FFW1, FFW2, DEW1, DEW2, DEW3,
     PBIAS, BCAST, IDX, OUT, KV0D, KD1, VD1A, VD1B) = t

    NRT = nrowc // P
    chunks = _chunks(nrowc)
    relu_rr = _RR("AADAD")      # Pool cannot read PSUM on TRN2
    copy_rr = _RR("AD")

    with ExitStack() as ctx:
        cp = ctx.enter_context(tc.tile_pool(name="const", bufs=1))
        wp = ctx.enter_context(tc.tile_pool(name="w", bufs=3))
        sp = ctx.enter_context(tc.tile_pool(name="stag", bufs=2))
        hp = ctx.enter_context(tc.tile_pool(name="h", bufs=2))
        gp0 = ctx.enter_context(tc.tile_pool(name="gath0", bufs=1))
        gp1 = ctx.enter_context(tc.tile_pool(name="gath1", bufs=2))
        ap = ctx.enter_context(tc.tile_pool(name="attn", bufs=1))
        pp = ctx.enter_context(tc.tile_pool(name="ps", bufs=5, space="PSUM"))
        pq3 = ctx.enter_context(tc.tile_pool(name="ps3", bufs=1, space="PSUM"))
        pqa = ctx.enter_context(tc.tile_pool(name="pa", bufs=2, space="PSUM"))

        # ---- resident loads; chain-0 critical path first ----
        xt8 = cp.tile([P, 2, nrowc], FP8)
        nc.sync.dma_start(xt8[:, :, 0:256], XT8.ap()[:, :, 0:256])
        nc.scalar.dma_start(xt8[:, :, 256:512], XT8.ap()[:, :, 256:512])
        xtub = cp.tile([97, nrowc], BF16)
        nc.scalar.dma_start(xtub[0:97:32, :], XTUB.ap())
        pbias = cp.tile([P, PB_COLS], F32)
        nc.gpsimd.dma_start(pbias[:], PBIAS.ap())
        for col, w in chunks:
            if col == 0:
                continue
            nc.scalar.dma_start(xt8[:, :, col:col + w], XT8.ap()[:, :, col:col + w])
        idx = cp.tile([P, 16], I32)
        nc.gpsimd.dma_start(idx[:], IDX.ap())
        xtb = cp.tile([P, 2, nrowc], BF16)
        for col, w in chunks:
            nc.scalar.dma_start(xtb[:, :, col:col + w], XTB.ap()[:, :, col:col + w])
        predt = cp.tile([P, 2, M], F32R)
        nc.gpsimd.dma_start(predt[:], PREDT.ap())
        dsw = cp.tile([P, 2, HD], F32R)
        nc.gpsimd.dma_start(dsw[:], DSW.ap())
        bcast = cp.tile([P, BC_COLS], BF16)
        nc.gpsimd.dma_start(bcast[:], BCAST.ap())
        ffw1 = cp.tile([P, L, 4, HD], BF16)
        nc.gpsimd.dma_start(ffw1[:], FFW1.ap().rearrange("l p a h -> p l a h"))
        ffw2 = cp.tile([P, L, 4, HD], BF16)
        nc.gpsimd.dma_start(ffw2[:], FFW2.ap().rearrange("l p a h -> p l a h"))
        dew1 = cp.tile([P, 4, F], BF16)
        nc.gpsimd.dma_start(dew1[:], DEW1.ap())
        dew2 = cp.tile([P, 2, F], BF16)
        nc.gpsimd.dma_start(dew2[:], DEW2.ap())
        dew3 = cp.tile([P, 2, R], BF16)
        nc.gpsimd.dma_start(dew3[:], DEW3.ap())
        ident = cp.tile([P, P], F32)
        make_identity(nc, ident[:])
        epst = cp.tile([P, 1], F32)
        nc.vector.memset(epst[:], EPS)

        def relu_out(ps_ap, out_ap, scale=None, bias=None):
            e = relu_rr.next()
            if e == "A":
                if bias is None:
                    nc.scalar.activation(out=out_ap, in_=ps_ap,
                                         func=mybir.ActivationFunctionType.Relu,
                                         scale=scale)
                else:
                    nc.scalar.activation(out=out_ap, in_=ps_ap,
                                         func=mybir.ActivationFunctionType.Relu,
                                         bias=pbias[:, bias[0]:bias[0] + 1])
            else:
                eng = nc.vector if e == "D" else nc.gpsimd
                if bias is None:
                    eng.tensor_scalar(out=out_ap, in0=ps_ap,
                                      scalar1=float(scale), scalar2=0.0,
                                      op0=mybir.AluOpType.mult,
                                      op1=mybir.AluOpType.max)
                else:
                    eng.tensor_scalar(out=out_ap, in0=ps_ap,
                                      scalar1=pbias[:, bias[1]:bias[1] + 1],
                                      scalar2=pbias[:, bias[0]:bias[0] + 1],
                                      op0=mybir.AluOpType.max,
                                      op1=mybir.AluOpType.add)

        def copy_out(ps_ap, out_ap, scale):
            e = copy_rr.next()
            if e == "A":
                nc.scalar.activation(out=out_ap, in_=ps_ap,
                                     func=mybir.ActivationFunctionType.Copy,
                                     scale=scale)
            else:
                eng = nc.vector if e == "D" else nc.gpsimd
                eng.tensor_scalar(out=out_ap, in0=ps_ap,
                                  scalar1=float(scale), scalar2=0.0,
                                  op0=mybir.AluOpType.mult,
                                  op1=mybir.AluOpType.add)

        def k_chain(c, h, stag):
            """fp8 DoubleRow chain, software-pipelined:
            L1(ci) -> L2(ci-1) -> L3(ci-2) so relus hide under matmuls."""
            w1 = wp.tile([P, 2, F], FP8, tag="w1k")
            nc.sync.dma_start(w1[:], W1K8.ap()[c])
            w2 = wp.tile([P, 2, F], FP8, tag="w2k")
            nc.sync.dma_start(w2[:], W2K8.ap()[c])
            w3 = wp.tile([P, 2, DK], FP8, tag="w3k")
            nc.sync.dma_start(w3[:], W3K8.ap()[c])
            w1l = wp.tile([97, F], BF16, tag="w1lk")
            nc.sync.dma_start(w1l[0:97:32, :], W1LB.ap()[c])

            st = {}

            def l1(ci):
                col, cw = chunks[ci]
                h1t = hp.tile([P, 2, 512], FP8, tag="h1k")
                # rank-1 u-terms OPEN the full psum region (a start=False
                # accumulate spanning two DR-opened regions corrupts psum, so
                # open wide first, then let the DR mains accumulate densely)
                ps = []
                for ft in range(2):
                    fsl = slice(ft * P, (ft + 1) * P)
                    rg = 32 * ft
                    ps1 = pp.tile([P, 512], F32, tag="ps")
                    ps.append(ps1)
                    nc.tensor.matmul(ps1[:, 0:cw], w1l[rg:rg + 1, fsl],
                                     xtub[rg:rg + 1, col:col + cw],
                                     start=True, stop=False, tile_position=(rg, 0))
                for ft in range(2):
                    fsl = slice(ft * P, (ft + 1) * P)
                    for cc in range(0, cw, 256):
                        ccw = min(256, cw - cc)
                        csl = slice(col + cc, col + cc + ccw)
                        nc.tensor.matmul(ps[ft][:, cc:cc + ccw], w1[:, :, fsl],
                                         xt8[:, :, csl], start=False,
                                         stop=(cc + 256 >= cw),
                                         perf_mode=DRMODE, skip_group_check=True)
                for ft in range(2):
                    relu_out(ps[ft][:, 0:cw], h1t[:, ft, 0:cw], scale=1.0 / H1DIV)
                st[ci] = [h1t, None]

            def l2(ci):
                col, cw = chunks[ci]
                h1t = st[ci][0]
                h2t = hp.tile([P, 2, 512], FP8, tag="h2k")
                ps = []
                for gt in range(2):
                    gsl = slice(gt * P, (gt + 1) * P)
                    ps2 = pp.tile([P, 512], F32, tag="ps")
                    ps.append(ps2)
                    for cc in range(0, cw, 256):
                        ccw = min(256, cw - cc)
                        nc.tensor.matmul(ps2[:, cc:cc + ccw], w2[:, :, gsl],
                                         h1t[:, :, cc:cc + ccw], start=True,
                                         stop=True, perf_mode=DRMODE)
                for gt in range(2):
                    relu_out(ps[gt][:, 0:cw], h2t[:, gt, 0:cw], scale=1.0,
                             bias=((PB_B2KP + 2 * c + gt, PB_B2KN + 2 * c + gt)
                                   if has_b2 else None))
                st[ci][1] = h2t

            def l3(ci):
                col, cw = chunks[ci]
                nrti = cw // P
                h2t = st[ci][1]
                ps3 = pq3.tile([P, 256], F32, tag="ps3")
                for rti in range(nrti):
                    rsl = slice(rti * P, (rti + 1) * P)
                    nc.tensor.matmul(ps3[:, rti * DK:(rti + 1) * DK],
                                     h2t[:, :, rsl], w3[:, :, :],
                                     start=True, stop=True, perf_mode=DRMODE)
                copy_out(ps3[:, 0:nrti * DK].rearrange("p (r d) -> p r d", d=DK),
                         stag[:, col // P:col // P + nrti, h * DK:(h + 1) * DK],
                         1.0 / L3KDIV)

            _pipeline(l1, l2, l3)

        def _pipeline(l1, l2, l3):
            n = len(chunks)
            for ci in range(n):
                l1(ci)
                if ci >= 1:
                    l2(ci - 1)
                if ci >= 2:
                    l3(ci - 2)
            l2(n - 1)
            if n >= 2:
                l3(n - 2)
            l3(n - 1)

        def v_chain(c, h, stag):
            w1 = wp.tile([P, 2, F], BF16, tag="w1v")
            nc.scalar.dma_start(w1[:], W1V.ap()[c])
            w2 = wp.tile([P, 2, F], BF16, tag="w2v")
            nc.scalar.dma_start(w2[:], W2V.ap()[c])
            w3 = wp.tile([P, 2, DK], BF16, tag="w3v")
            nc.scalar.dma_start(w3[:], W3V.ap()[c])
            w1l = wp.tile([97, F], BF16, tag="w1lv")
            nc.scalar.dma_start(w1l[0:97:32, :], W1LB.ap()[NCH + c])

            st = {}

            def l1(ci):
                col, cw = chunks[ci]
                h1t = hp.tile([P, 2, 512], BF16, tag="h1v")
                ps = []
                for ft in range(2):
                    fsl = slice(ft * P, (ft + 1) * P)
                    rg = 32 * ft
                    ps1 = pp.tile([P, 512], F32, tag="ps")
                    ps.append(ps1)
                    nc.tensor.matmul(ps1[:, 0:cw], w1l[rg:rg + 1, fsl],
                                     xtub[rg:rg + 1, col:col + cw],
                                     start=True, stop=False, tile_position=(rg, 0))
                for ft in range(2):
                    fsl = slice(ft * P, (ft + 1) * P)
                    nc.tensor.matmul(ps[ft][:, 0:cw], w1[:, 0, fsl],
                                     xtb[:, 0, col:col + cw], start=False, stop=False,
                                     skip_group_check=True)
                    nc.tensor.matmul(ps[ft][:, 0:cw], w1[:, 1, fsl],
                                     xtb[:, 1, col:col + cw], start=False, stop=True,
                                     skip_group_check=True)
                for ft in range(2):
                    relu_out(ps[ft][:, 0:cw], h1t[:, ft, 0:cw], scale=1.0 / V1DIV)
                st[ci] = [h1t, None]

            def l2(ci):
                col, cw = chunks[ci]
                h1t = st[ci][0]
                h2t = hp.tile([P, 2, 512], BF16, tag="h2v")
                ps = []
                for gt in range(2):
                    gsl = slice(gt * P, (gt + 1) * P)
                    ps2 = pp.tile([P, 512], F32, tag="ps")
                    ps.append(ps2)
                    nc.tensor.matmul(ps2[:, 0:cw], w2[:, 0, gsl],
                                     h1t[:, 0, 0:cw], start=True, stop=False)
                    nc.tensor.matmul(ps2[:, 0:cw], w2[:, 1, gsl],
                                     h1t[:, 1, 0:cw], start=False, stop=True)
                for gt in range(2):
                    relu_out(ps[gt][:, 0:cw], h2t[:, gt, 0:cw], scale=1.0,
                             bias=((PB_B2VP + 2 * c + gt, PB_B2VN + 2 * c + gt)
                                   if has_b2 else None))
                st[ci][1] = h2t

            def l3(ci):
                col, cw = chunks[ci]
                nrti = cw // P
                h2t = st[ci][1]
                ps3 = pq3.tile([P, 256], F32, tag="ps3")
                for rti in range(nrti):
                    rsl = slice(rti * P, (rti + 1) * P)
                    nc.tensor.matmul(ps3[:, rti * DK:(rti + 1) * DK],
                                     h2t[:, 0, rsl], w3[:, 0, :],
                                     start=True, stop=False)
                    nc.tensor.matmul(ps3[:, rti * DK:(rti + 1) * DK],
                                     h2t[:, 1, rsl], w3[:, 1, :],
                                     start=False, stop=True)
                copy_out(ps3[:, 0:nrti * DK].rearrange("p (r d) -> p r d", d=DK),
                         stag[:, col // P:col // P + nrti, h * DK:(h + 1) * DK],
                         1.0)

            _pipeline(l1, l2, l3)

        def ds_project():
            ps = pqa.tile([P, HD], F32, tag="pa")
            nc.tensor.matmul(ps[:], predt[:, 0, :], dsw[:, 0, :], start=True, stop=False)
            nc.tensor.matmul(ps[:], predt[:, 1, :], dsw[:, 1, :], start=False, stop=True)
            av0 = ap.tile([P, HD], F32, tag="av")
            nc.vector.tensor_tensor(out=av0[:], in0=ps[:],
                                    in1=bcast[:, BC_DSB:BC_DSB + HD],
                                    op=mybir.AluOpType.add)
            return av0

        def layer_norm(src, gcol, bcol, out):
            stats = ap.tile([P, 6], F32, tag="lnstat")
            nc.vector.bn_stats(stats[:], src[:])
            mv = ap.tile([P, 2], F32, tag="lnmv")
            nc.vector.bn_aggr(mv[:], stats[:])
            rstd = ap.tile([P, 1], F32, tag="lnrstd")
            nc.scalar.activation(out=rstd[:], in_=mv[:, 1:2],
                                 func=mybir.ActivationFunctionType.Sqrt,
                                 bias=epst[:, 0:1])
            nc.vector.reciprocal(rstd[:], rstd[:])
            nc.vector.tensor_scalar(out=out[:], in0=src[:], scalar1=mv[:, 0:1],
                                    scalar2=rstd[:, 0:1],
                                    op0=mybir.AluOpType.subtract,
                                    op1=mybir.AluOpType.mult)
            nc.vector.tensor_tensor(out=out[:], in0=out[:],
                                    in1=bcast[:, gcol:gcol + HD],
                                    op=mybir.AluOpType.mult)
            nc.vector.tensor_tensor(out=out[:], in0=out[:],
                                    in1=bcast[:, bcol:bcol + HD],
                                    op=mybir.AluOpType.add)

        def transpose_to(src, dst):
            for kt in range(4):
                tp_ = pqa.tile([P, P], F32, tag="pa")
                nc.tensor.transpose(tp_[:], src[:, kt * P:(kt + 1) * P], ident[:])
                nc.scalar.activation(out=dst[:, kt, :], in_=tp_[:],
                                     func=mybir.ActivationFunctionType.Copy)

        def gather0(gall):
            """l=0: combined K+V rows from KV0D into [P, 16, 2, HD]."""
            for j in range(16):
                n, s = j // 8, j % 8
                if n == 1 and s < i_val:
                    sec = sec_starts[s]
                    nc.sync.dma_start(gall[:, j, :, :], KV0D.ap()[sec:sec + P, :, :])
                else:
                    # indirect DMA requires 2D src/dst APs (3D reads garbage)
                    nc.gpsimd.indirect_dma_start(
                        out=gall[:, j, :, :].rearrange("p a hd -> p (a hd)"),
                        out_offset=None,
                        in_=KV0D.ap().rearrange("r a hd -> r (a hd)"),
                        in_offset=bass.IndirectOffsetOnAxis(ap=idx[:, j:j + 1], axis=0))

        def gather1(src, gall):
            """l=1: one of KD1/VD1 into [P, 16, HD]."""
            for j in range(16):
                n, s = j // 8, j % 8
                if n == 1 and s < i_val:
                    sec = sec_starts[s]
                    nc.sync.dma_start(gall[:, j, :], src.ap()[sec:sec + P, :])
                else:
                    nc.gpsimd.indirect_dma_start(
                        out=gall[:, j, :], out_offset=None, in_=src.ap(),
                        in_offset=bass.IndirectOffsetOnAxis(ap=idx[:, j:j + 1], axis=0))

        def attn_phase1(l, av_in, wall, kg):
            """kg: callable js_slice -> [P, 4, HD] bf16 K rows."""
            avbf = ap.tile([P, HD], BF16, tag="avbf")
            nc.vector.tensor_copy(out=avbf[:], in_=av_in[:])
            scrb = ap.tile([P, 4, HD], BF16, tag="scrb")
            logits = ap.tile([P, 16, 8], BF16, tag="logits")
            avb = avbf[:, None, :].to_broadcast([P, 4, HD])
            for js in range(4):
                jsl = slice(js * 4, (js + 1) * 4)
                nc.vector.tensor_tensor(out=scrb[:], in0=kg(jsl),
                                        in1=avb, op=mybir.AluOpType.mult)
                with nc.allow_low_precision(reason="logits bf16; sigmoid smooths"):
                    nc.vector.tensor_reduce(
                        out=logits[:, jsl, :],
                        in_=scrb[:].rearrange("p a (h d) -> p (a h) d", d=DK),
                        axis=mybir.AxisListType.X, op=mybir.AluOpType.add)
            delta = ap.tile([P, 8, 8], BF16, tag="delta")
            nc.vector.tensor_tensor(out=delta[:], in0=logits[:, 0:8, :],
                                    in1=logits[:, 8:16, :],
                                    op=mybir.AluOpType.subtract)
            scale = DK ** (-0.5)
            dflat = delta[:].rearrange("p a b -> p (a b)")
            nc.scalar.activation(out=wall[:, 0:64], in_=dflat,
                                 func=mybir.ActivationFunctionType.Sigmoid,
                                 scale=scale)
            nc.scalar.activation(out=wall[:, 64:128], in_=dflat,
                                 func=mybir.ActivationFunctionType.Sigmoid,
                                 scale=-scale)

        def phase2_step(l, wall, vg, js, att, part, scr, half=None):
            jsl = slice(js * 4, (js + 1) * 4)
            wv = wall[:].rearrange("p (j h) -> p j h", h=8)
            if half is None:
                hsl, nh = slice(0, HD), 8
            else:
                hsl, nh = slice(half * (HD // 2), (half + 1) * (HD // 2)), 4
            wvh = wv[:, jsl, half * 4:half * 4 + 4] if half is not None \
                else wv[:, jsl, :]
            nc.vector.tensor_tensor(
                out=scr[:, :, hsl].rearrange("p a (h d) -> p a h d", d=DK),
                in0=vg(jsl).rearrange("p a (h d) -> p a h d", d=DK),
                in1=wvh[:, :, :, None].to_broadcast([P, 4, nh, DK]),
                op=mybir.AluOpType.mult)
            dst = att if js == 0 else part
            nc.vector.tensor_reduce(
                out=dst[:, hsl],
                in_=scr[:, :, hsl].rearrange("p a hd -> p hd a"),
                axis=mybir.AxisListType.X, op=mybir.AluOpType.add)
            if js > 0:
                nc.vector.tensor_tensor(out=att[:, hsl], in0=att[:, hsl],
                                        in1=part[:, hsl],
                                        op=mybir.AluOpType.add)

        def attn_phase2_tail(l, avp, att):
            nc.vector.tensor_tensor(out=att[:], in0=att[:], in1=avp[:],
                                    op=mybir.AluOpType.add)
            xn = ap.tile([P, HD], F32, tag="xn")
            layer_norm(att, BC_LN1G + l * HD, BC_LN1B + l * HD, xn)
            xT = ap.tile([P, 4, P], BF16, tag="xT")
            transpose_to(xn, xT)
            ff1 = ap.tile([P, 4, P], BF16, tag="ff1")
            for ft in range(4):
                psf = pqa.tile([P, P], F32, tag="pa")
                for kt in range(4):
                    nc.tensor.matmul(psf[:], ffw1[:, l, kt, ft * P:(ft + 1) * P],
                                     xT[:, kt, :], start=(kt == 0), stop=(kt == 3))
                nc.scalar.activation(
                    out=ff1[:, ft, :], in_=psf[:],
                    func=mybir.ActivationFunctionType.Relu,
                    bias=pbias[:, PB_FFB1 + 4 * l + ft:PB_FFB1 + 4 * l + ft + 1])
            ps2 = pqa.tile([P, HD], F32, tag="pa")
            for kt in range(4):
                nc.tensor.matmul(ps2[:], ff1[:, kt, :], ffw2[:, l, kt, :],
                                 start=(kt == 0), stop=(kt == 3))
            ffx = ap.tile([P, HD], F32, tag="ffx")
            nc.vector.tensor_tensor(out=ffx[:], in0=ps2[:],
                                    in1=bcast[:, BC_FFB2 + l * HD:BC_FFB2 + (l + 1) * HD],
                                    op=mybir.AluOpType.add)
            nc.vector.tensor_tensor(out=ffx[:], in0=ffx[:], in1=xn[:],
                                    op=mybir.AluOpType.add)
            av_out = ap.tile([P, HD], F32, tag="av")
            layer_norm(ffx, BC_LN2G + l * HD, BC_LN2B + l * HD, av_out)
            return av_out

        def make_avp(l, av_in):
            avp = ap.tile([P, HD], F32, tag="avp")
            nc.vector.tensor_tensor(
                out=avp[:], in0=av_in[:],
                in1=bcast[:, BC_B3V8 + l * HD:BC_B3V8 + (l + 1) * HD],
                op=mybir.AluOpType.add)
            return avp

        # ================= schedule =================
        stag = sp.tile([P, NRT, HD], BF16, tag="stag")
        av = None
        for h in range(H):
            k_chain(0 * 8 + h, h, stag)
            if h == 0:
                av = ds_project()
        nc.sync.dma_start(
            KV0D.ap()[:, 0, :].rearrange("(rt p) hd -> p rt hd", p=P), stag[:])

        stag = sp.tile([P, NRT, HD], BF16, tag="stag")
        for h in range(H):
            v_chain(0 * 8 + h, h, stag)
        nc.sync.dma_start(
            KV0D.ap()[:, 1, :].rearrange("(rt p) hd -> p rt hd", p=P), stag[:])
        kv0gall = gp0.tile([P, 16, 2, HD], BF16, tag="gall0")
        gather0(kv0gall)

        # l=1 K chains; l=0 attention interleaved (phase2 spread over chains)
        stag = sp.tile([P, NRT, HD], BF16, tag="stag")
        wall0 = ap.tile([P, 128], F32, tag="wall")
        att0 = ap.tile([P, HD], F32, tag="att")
        part0 = ap.tile([P, HD], F32, tag="part")
        scr0 = ap.tile([P, 4, HD], F32, tag="scr")
        vg0 = lambda jsl: kv0gall[:, jsl, 1, :]
        avp0 = None
        for h in range(H):
            k_chain(1 * 8 + h, h, stag)
            if h == 1:
                attn_phase1(0, av, wall0, lambda jsl: kv0gall[:, jsl, 0, :])
                avp0 = make_avp(0, av)
            elif 2 <= h <= 5:
                phase2_step(0, wall0, vg0, h - 2, att0, part0, scr0)
        nc.sync.dma_start(
            KD1.ap().rearrange("(rt p) hd -> p rt hd", p=P), stag[:])
        k1gall = gp1.tile([P, 16, HD], BF16, tag="gall1")
        gather1(KD1, k1gall)
        av = attn_phase2_tail(0, avp0, att0)

        # l=1 V chains; l=1 phase1 interleaved. The stag is written to DRAM
        # in two head-halves so V gathers + weighted sums start while the
        # last four chains still compute.
        stag = sp.tile([P, NRT, HD], BF16, tag="stag")
        wall1 = ap.tile([P, 128], F32, tag="wall")
        att1 = ap.tile([P, HD], F32, tag="att")
        part1 = ap.tile([P, HD], F32, tag="part")
        scr1 = ap.tile([P, 4, HD], F32, tag="scr")
        v1gall = gp1.tile([P, 16, HD], BF16, tag="gall1")
        avp1 = None

        def v1_half(half, src_t):
            hsl = slice(half * (HD // 2), (half + 1) * (HD // 2))
            nc.sync.dma_start(
                src_t.ap().rearrange("(rt p) hd -> p rt hd", p=P),
                stag[:, :, hsl])
            for js in range(4):
                for j in range(js * 4, js * 4 + 4):
                    n, s = j // 8, j % 8
                    if n == 1 and s < i_val:
                        nc.sync.dma_start(
                            v1gall[:, j, hsl],
                            src_t.ap()[sec_starts[s]:sec_starts[s] + P, :])
                    else:
                        nc.gpsimd.indirect_dma_start(
                            out=v1gall[:, j, hsl], out_offset=None,
                            in_=src_t.ap(),
                            in_offset=bass.IndirectOffsetOnAxis(
                                ap=idx[:, j:j + 1], axis=0))
                phase2_step(1, wall1, lambda jsl: v1gall[:, jsl, hsl], js,
                            att1, part1, scr1, half=half)

        for h in range(H):
            v_chain(1 * 8 + h, h, stag)
            if h == 1:
                attn_phase1(1, av, wall1, lambda jsl: k1gall[:, jsl, :])
                avp1 = make_avp(1, av)
            elif h == 4:
                v1_half(0, VD1A)
        v1_half(1, VD1B)
        av = attn_phase2_tail(1, avp1, att1)

        # ---- dist extractor ----
        avT = ap.tile([P, 4, P], BF16, tag="avT")
        transpose_to(av, avT)
        h1 = ap.tile([P, 2, P], BF16, tag="deh1")
        for ft in range(2):
            psd = pqa.tile([P, P], F32, tag="pa")
            for kt in range(4):
                nc.tensor.matmul(psd[:], dew1[:, kt, ft * P:(ft + 1) * P],
                                 avT[:, kt, :], start=(kt == 0), stop=(kt == 3))
            nc.scalar.activation(out=h1[:, ft, :], in_=psd[:],
                                 func=mybir.ActivationFunctionType.Relu,
                                 bias=pbias[:, PB_DEB1 + ft:PB_DEB1 + ft + 1])
        h2 = ap.tile([P, 2, P], BF16, tag="deh2")
        for ft in range(2):
            psd = pqa.tile([P, P], F32, tag="pa")
            for kt in range(2):
                nc.tensor.matmul(psd[:], dew2[:, kt, ft * P:(ft + 1) * P],
                                 h1[:, kt, :], start=(kt == 0), stop=(kt == 1))
            nc.scalar.activation(out=h2[:, ft, :], in_=psd[:],
                                 func=mybir.ActivationFunctionType.Relu,
                                 bias=pbias[:, PB_DEB2 + ft:PB_DEB2 + ft + 1])
        pso = pqa.tile([P, R], F32, tag="pa")
        for kt in range(2):
            nc.tensor.matmul(pso[:], h2[:, kt, :], dew3[:, kt, :],
                             start=(kt == 0), stop=(kt == 1))
        o = ap.tile([P, R], F32, tag="out")
        nc.vector.tensor_tensor(out=o[:], in0=pso[:],
                                in1=bcast[:, BC_DEB3:BC_DEB3 + R],
                                op=mybir.AluOpType.add)
        nc.sync.dma_start(OUT.ap()[:, 0:R // 2], o[:, 0:R // 2])
        nc.scalar.dma_start(OUT.ap()[:, R // 2:], o[:, R // 2:])


_BUILD_CACHE = {}


def _build(i_val, nrowc, sec_starts, has_b2=False):
    key = (i_val, nrowc, tuple(sec_starts), has_b2)
    if key in _BUILD_CACHE:
        return _BUILD_CACHE[key]
    nc = bacc.Bacc("TRN2", target_bir_lowering=False, debug=False)
    t = []
    t.append(nc.dram_tensor("XT8", [P, 2, nrowc], FP8, kind="ExternalInput"))
    t.append(nc.dram_tensor("XTB", [P, 2, nrowc], BF16, kind="ExternalInput"))
    t.append(nc.dram_tensor("XTUB", [4, nrowc], BF16, kind="ExternalInput"))
    t.append(nc.dram_tensor("PREDT", [P, 2, M], F32R, kind="ExternalInput"))
    t.append(nc.dram_tensor("DSW", [P, 2, HD], F32R, kind="ExternalInput"))
    t.append(nc.dram_tensor("W1K8", [NCH, P, 2, F], FP8, kind="ExternalInput"))
    t.append(nc.dram_tensor("W2K8", [NCH, P, 2, F], FP8, kind="ExternalInput"))
    t.append(nc.dram_tensor("W3K8", [NCH, P, 2, DK], FP8, kind="ExternalInput"))
    t.append(nc.dram_tensor("W1LB", [2 * NCH, 4, F], BF16, kind="ExternalInput"))
    t.append(nc.dram_tensor("W1V", [NCH, P, 2, F], BF16, kind="ExternalInput"))
    t.append(nc.dram_tensor("W2V", [NCH, P, 2, F], BF16, kind="ExternalInput"))
    t.append(nc.dram_tensor("W3V", [NCH, P, 2, DK], BF16, kind="ExternalInput"))
    t.append(nc.dram_tensor("FFW1", [L, P, 4, HD], BF16, kind="ExternalInput"))
    t.append(nc.dram_tensor("FFW2", [L, P, 4, HD], BF16, kind="ExternalInput"))
    t.append(nc.dram_tensor("DEW1", [P, 4, F], BF16, kind="ExternalInput"))
    t.append(nc.dram_tensor("DEW2", [P, 2, F], BF16, kind="ExternalInput"))
    t.append(nc.dram_tensor("DEW3", [P, 2, R], BF16, kind="ExternalInput"))
    t.append(nc.dram_tensor("PBIAS", [P, PB_COLS], F32, kind="ExternalInput"))
    t.append(nc.dram_tensor("BCAST", [P, BC_COLS], BF16, kind="ExternalInput"))
    t.append(nc.dram_tensor("IDX", [P, 16], I32, kind="ExternalInput"))
    t.append(nc.dram_tensor("OUT", [M, R], F32, kind="ExternalOutput"))
    t.append(nc.dram_tensor("KV0D", [nrowc, 2, HD], BF16))
    t.append(nc.dram_tensor("KD1", [nrowc, HD], BF16))
    t.append(nc.dram_tensor("VD1A", [nrowc, HD // 2], BF16))
    t.append(nc.dram_tensor("VD1B", [nrowc, HD // 2], BF16))
    with tile.TileContext(nc) as tc:
        _emit(nc, tc, t, i_val, nrowc, sec_starts, has_b2)
    nc.compile()
    _BUILD_CACHE[key] = nc
    return nc


def _fp8(x, scale):
    return np.clip(np.asarray(x, np.float32) * scale, -240.0, 240.0).astype(E4NP)


def plan_compaction(ins):
    i = int(ins["i"])
    left = np.asarray(ins["left_idx"], np.int64)
    right = np.asarray(ins["right_idx"], np.int64)
    m_ar = np.arange(M, dtype=np.int64)
    tsets = []
    for s in range(S):
        tset = np.union1d(left, m_ar) if s < i else np.union1d(left, right)
        tsets.append(np.sort(tset))
    sec_starts = []
    rows = []
    rowmap = {}
    for s in range(S):
        sec_starts.append(len(rows))
        for tt in tsets[s]:
            rowmap[(s, int(tt))] = len(rows)
            rows.append((s, int(tt)))
    nraw = len(rows)
    nrowc = ((nraw + P - 1) // P) * P
    idxm = np.zeros((P, 16), np.int32)
    for s in range(S):
        for mm in range(M):
            idxm[mm, 0 * 8 + s] = rowmap[(s, int(left[mm]))]
            t1 = mm if s < i else int(right[mm])
            idxm[mm, 1 * 8 + s] = rowmap[(s, t1)]
    return {
        "i": i, "rows": rows, "nrowc": nrowc,
        "sec_starts": tuple(sec_starts[:i]), "idx": idxm,
    }


def _prep_shared(ins, plan):
    f32 = np.float32
    kW1, kW2, kW3 = ins["kW1"], ins["kW2"], ins["kW3"]
    vW1, vW2, vW3 = ins["vW1"], ins["vW2"], ins["vW3"]
    kb1, kb2 = ins["kb1"], ins["kb2"]
    vb1, vb2, vb3 = ins["vb1"], ins["vb2"], ins["vb3"]
    if np.any(np.asarray(kb1, np.float32)) or np.any(np.asarray(vb1, np.float32)):
        raise NotImplementedError("nonzero kb1/vb1 not folded (u-row rank-1 "
                                  "carries no bias term)")

    W1K8 = np.empty((NCH, P, 2, F), E4NP)
    W2K8 = np.empty((NCH, P, 2, F), E4NP)
    W3K8 = np.empty((NCH, P, 2, DK), E4NP)
    W1V = np.empty((NCH, P, 2, F), BFNP)
    W2V = np.empty((NCH, P, 2, F), BFNP)
    W3V = np.empty((NCH, P, 2, DK), BFNP)
    W1LB = np.empty((2 * NCH, 4, F), BFNP)
    PB = np.zeros((P, PB_COLS), f32)
    BC = np.zeros((BC_COLS,), f32)

    def pack2(w, ncols):
        return np.asarray(w, f32).reshape(2, P, ncols).transpose(1, 0, 2)

    for l in range(L):
        for h in range(H):
            c = l * 8 + h
            W1K8[c] = _fp8(pack2(kW1[l, h][:256], F), W1S)
            W2K8[c] = _fp8(pack2(kW2[l, h], F), W2S)
            W3K8[c] = _fp8(pack2(kW3[l, h], DK), W3S)
            W1LB[c] = np.broadcast_to(
                (np.asarray(kW1[l, h][256], f32) * W1S).astype(BFNP), (4, F))
            W1V[c] = (pack2(vW1[l, h][:256], F) * 64.0).astype(BFNP)
            W2V[c] = pack2(vW2[l, h], F).astype(BFNP)
            W3V[c] = pack2(vW3[l, h], DK).astype(BFNP)
            W1LB[NCH + c] = np.broadcast_to(
                (np.asarray(vW1[l, h][256], f32) * W1S).astype(BFNP), (4, F))
            for gt in range(2):
                gsl = slice(gt * P, (gt + 1) * P)
                PB[:, PB_B2KP + 2 * c + gt] = B2KS * np.asarray(kb2[l, h][gsl], f32)
                PB[:, PB_B2KN + 2 * c + gt] = -B2KS * np.asarray(kb2[l, h][gsl], f32)
                PB[:, PB_B2VP + 2 * c + gt] = np.asarray(vb2[l, h][gsl], f32)
                PB[:, PB_B2VN + 2 * c + gt] = -np.asarray(vb2[l, h][gsl], f32)

    DSW = np.asarray(ins["ds_W"], f32).reshape(2, P, HD).transpose(1, 0, 2).copy()
    BC[BC_DSB:BC_DSB + HD] = np.asarray(ins["ds_b"], f32)
    for l in range(L):
        BC[BC_B3V8 + l * HD:BC_B3V8 + (l + 1) * HD] = \
            S * np.asarray(vb3[l], f32).reshape(HD)

    FFW1 = np.empty((L, P, 4, HD), BFNP)
    FFW2 = np.empty((L, P, 4, HD), BFNP)
    for l in range(L):
        FFW1[l] = np.asarray(ins["ffW1"][l], f32).reshape(4, P, HD).transpose(1, 0, 2).astype(BFNP)
        FFW2[l] = np.asarray(ins["ffW2"][l], f32).reshape(4, P, HD).transpose(1, 0, 2).astype(BFNP)
        for ft in range(4):
            PB[:, PB_FFB1 + 4 * l + ft] = np.asarray(ins["ffb1"][l][ft * P:(ft + 1) * P], f32)
        BC[BC_FFB2 + l * HD:BC_FFB2 + (l + 1) * HD] = np.asarray(ins["ffb2"][l], f32)
        BC[BC_LN1G + l * HD:BC_LN1G + (l + 1) * HD] = np.asarray(ins["ln1_g"][l], f32)
        BC[BC_LN1B + l * HD:BC_LN1B + (l + 1) * HD] = np.asarray(ins["ln1_b"][l], f32)
        BC[BC_LN2G + l * HD:BC_LN2G + (l + 1) * HD] = np.asarray(ins["ln2_g"][l], f32)
        BC[BC_LN2B + l * HD:BC_LN2B + (l + 1) * HD] = np.asarray(ins["ln2_b"][l], f32)

    DEW1 = np.asarray(ins["deW1"], f32).reshape(4, P, F).transpose(1, 0, 2).astype(BFNP)
    DEW2 = np.asarray(ins["deW2"], f32).reshape(2, P, F).transpose(1, 0, 2).astype(BFNP)
    DEW3 = np.asarray(ins["deW3"], f32).reshape(2, P, R).transpose(1, 0, 2).astype(BFNP)
    for ft in range(2):
        PB[:, PB_DEB1 + ft] = np.asarray(ins["deb1"][ft * P:(ft + 1) * P], f32)
        PB[:, PB_DEB2 + ft] = np.asarray(ins["deb2"][ft * P:(ft + 1) * P], f32)
    BC[BC_DEB3:BC_DEB3 + R] = np.asarray(ins["deb3"], f32)

    BCAST = np.broadcast_to(BC.astype(BFNP), (P, BC_COLS)).copy()

    return {
        "W1K8": W1K8, "W2K8": W2K8, "W3K8": W3K8, "W1LB": W1LB,
        "W1V": W1V, "W2V": W2V, "W3V": W3V,
        "DSW": DSW, "FFW1": FFW1, "FFW2": FFW2,
        "DEW1": DEW1, "DEW2": DEW2, "DEW3": DEW3,
        "PBIAS": PB, "BCAST": BCAST, "IDX": plan["idx"],
    }


def make_in_maps(ins, plan=None):
    if plan is None:
        plan = plan_compaction(ins)
    shared = _prep_shared(ins, plan)
    enc = np.asarray(ins["encoded"], np.float32)
    tu = np.asarray(ins["true_u"], np.float32)
    mid = np.asarray(ins["mid_idx"], np.int64)
    i = plan["i"]
    nrowc = plan["nrowc"]
    rows = plan["rows"]
    s_idx = np.array([r[0] for r in rows], np.int64)
    t_idx = np.array([r[1] for r in rows], np.int64)

    in_maps = []
    for b in range(B):
        xg = np.zeros((nrowc, D), np.float32)
        xg[:len(rows)] = enc[b][s_idx, t_idx]
        ug = np.zeros((nrowc,), np.float32)
        ug[:len(rows)] = tu[b][s_idx, t_idx]
        xs16 = xg.T.reshape(2, P, nrowc).transpose(1, 0, 2) * XS
        xub = np.broadcast_to((ug * XS).astype(BFNP), (4, nrowc))
        pred = enc[b, i][mid]
        predt = pred.T.reshape(2, P, M).transpose(1, 0, 2).copy()
        m = dict(shared)
        m["XT8"] = _fp8(xs16, 1.0)
        m["XTB"] = xs16.astype(BFNP)
        m["XTUB"] = np.ascontiguousarray(xub)
        m["PREDT"] = predt
        in_maps.append(m)
    return in_maps


def kernel(**inputs):
    ins = {k: np.asarray(v) for k, v in inputs.items()}
    plan = plan_compaction(ins)
    in_maps = make_in_maps(ins, plan)
    has_b2 = bool(np.any(np.asarray(ins["kb2"], np.float32))
                  or np.any(np.asarray(ins["vb2"], np.float32)))
    nc = _build(plan["i"], plan["nrowc"], plan["sec_starts"], has_b2)
    res = run_bass_kernel_spmd(nc, in_maps, core_ids=list(range(NCORES)))
    out = np.stack([res.results[c]["OUT"] for c in range(NCORES)])
    return out.astype(np.float32)


# revision 21
# speedup vs baseline: 1.0368x; 1.0059x over previous
"""Trainium2 Bass kernel for nn_AttentionalCopula (sparse_attention).

Sharding: data-parallel over batch (B=8 -> 8 cores); per-head K/V MLP stacks
computed locally per core. Weights replicated. No collectives.

Key optimizations over the v1 baseline:
  * Row compaction: the attention only gathers K/V rows from
    union_s({left} u ({arange} if s<i else {right})) -- ~1350 of 2048 rows.
    The MLP chains run only on those rows.
  * K chains in fp8 (float8e4) with MatmulPerfMode.DoubleRow: K_eff=256 per
    matmul at 0.5 cyc/col (4x bf16). V chains stay bf16 (V values feed the
    output directly; fp8 there fails the accuracy gate -- measured).
  * The u-row (feature 257) enters L1 psum via a K=1 bf16 rank-1 matmul
    (tile_position-paired across the two f-tiles).
  * Relu/copy work is round-robined across ACT/DVE/Pool engines.
  * l=0 K/V staged interleaved in DRAM so one indirect gather per slot
    fetches both; contiguous slots (n=1, s<i) use direct DMA.
"""

from contextlib import ExitStack

import ml_dtypes
import numpy as np

import concourse.bass as bass
import concourse.mybir as mybir
import concourse.tile as tile
from concourse import bacc
from concourse.bass_utils import run_bass_kernel_spmd
from concourse.masks import make_identity

B, S, T, D = 8, 8, 256, 256
H, DK = 8, 64
HD = H * DK            # 512
L = 2
F = 256
R = 512
M = 128
EPS = 1e-5
NCORES = 8
P = 128

F32 = mybir.dt.float32
F32R = mybir.dt.float32r
BF16 = mybir.dt.bfloat16
FP8 = mybir.dt.float8e4
I32 = mybir.dt.int32
E4NP = ml_dtypes.float8_e4m3
BFNP = ml_dtypes.bfloat16
DRMODE = mybir.MatmulPerfMode.DoubleRow

NCH = L * H            # 16 chains per kv side

# fp8 scale plan (K side):  X*16, W1*64 -> psum 1024*pre1 ; h1'=psum/64 (16*h1)
# W2*2 -> psum2 32*pre2 ; h2' = max(psum2 + 32*b2, 0) (32*h2)
# W3*64 -> psum3 2048*kv ; stag = psum3/2048
XS = 16.0
W1S = 64.0
H1DIV = 64.0
W2S = 2.0
B2KS = 32.0
W3S = 64.0
L3KDIV = 2048.0
# V side: X*16 (bf16), W1*64 (bf16) -> psum 1024*pre1 ; h1 = psum/1024
V1DIV = 1024.0

# PBIAS layout (per-partition f32 scalars)
PB_FFB1 = 0                      # 4 per l
PB_DEB1 = PB_FFB1 + 4 * L
PB_DEB2 = PB_DEB1 + 2
PB_B2KP = PB_DEB2 + 2            # +32*b2k per (c, gt)
PB_B2KN = PB_B2KP + 2 * NCH      # -32*b2k
PB_B2VP = PB_B2KN + 2 * NCH      # +b2v
PB_B2VN = PB_B2VP + 2 * NCH      # -b2v
PB_COLS = PB_B2VN + 2 * NCH

# BCAST layout (free-dim vectors, bf16, replicated across partitions)
BC_DSB = 0
BC_B3V8 = BC_DSB + HD            # S * vb3 per l
BC_FFB2 = BC_B3V8 + L * HD
BC_DEB3 = BC_FFB2 + L * HD
BC_LN1G = BC_DEB3 + R
BC_LN1B = BC_LN1G + L * HD
BC_LN2G = BC_LN1B + L * HD
BC_LN2B = BC_LN2G + L * HD
BC_COLS = BC_LN2B + L * HD


def _chunks(nrowc):
    out = []
    col = 0
    while col < nrowc:
        w = min(512, nrowc - col)
        out.append((col, w))
        col += w
    return out


class _RR:
    def __init__(self, pattern):
        self.pattern = pattern
        self.i = 0

    def next(self):
        e = self.pattern[self.i % len(self.pattern)]
        self.i += 1
        return e


def _emit(nc, tc, t, i_val, nrowc, sec_starts, has_b2):
    (XT8, XTB, XTUB, PREDT, DSW, W1K8, W2K8, W3K8, W1LB,
     W1V, W2V, W3V, FFW1, FFW2, DEW1, DEW2, DEW3,
     PBIAS, BCAST, IDX, OUT, KV0D, KD1, VD1A, VD1B) = t

    NRT = nrowc // P
    chunks = _chunks(nrowc)
    relu_rr = _RR("AADAD")      # Pool cannot read PSUM on TRN2
    copy_rr = _RR("AD")

    with ExitStack() as ctx:
        cp = ctx.enter_context(tc.tile_pool(name="const", bufs=1))
        wp = ctx.enter_context(tc.tile_pool(name="w", bufs=3))
        sp = ctx.enter_context(tc.tile_pool(name="stag", bufs=2))
        hp = ctx.enter_context(tc.tile_pool(name="h", bufs=2))
        gp0 = ctx.enter_context(tc.tile_pool(name="gath0", bufs=1))
        gp1 = ctx.enter_context(tc.tile_pool(name="gath1", bufs=2))
        ap = ctx.enter_context(tc.tile_pool(name="attn", bufs=1))
        pp = ctx.enter_context(tc.tile_pool(name="ps", bufs=5, space="PSUM"))
        pq3 = ctx.enter_context(tc.tile_pool(name="ps3", bufs=2, space="PSUM"))
        pqa = ctx.enter_context(tc.tile_pool(name="pa", bufs=1, space="PSUM"))

        # ---- resident loads; chain-0 critical path first ----
        xt8 = cp.tile([P, 2, nrowc], FP8)
        nc.sync.dma_start(xt8[:, :, 0:256], XT8.ap()[:, :, 0:256])
        nc.scalar.dma_start(xt8[:, :, 256:512], XT8.ap()[:, :, 256:512])
        xtub = cp.tile([97, nrowc], BF16)
        nc.scalar.dma_start(xtub[0:97:32, :], XTUB.ap())
        pbias = cp.tile([P, PB_COLS], F32)
        nc.gpsimd.dma_start(pbias[:], PBIAS.ap())
        for col, w in chunks:
            if col == 0:
                continue
            nc.scalar.dma_start(xt8[:, :, col:col + w], XT8.ap()[:, :, col:col + w])
        idx = cp.tile([P, 16], I32)
        nc.gpsimd.dma_start(idx[:], IDX.ap())
        xtb = cp.tile([P, 2, nrowc], BF16)
        for col, w in chunks:
            nc.scalar.dma_start(xtb[:, :, col:col + w], XTB.ap()[:, :, col:col + w])
        predt = cp.tile([P, 2, M], F32R)
        nc.gpsimd.dma_start(predt[:], PREDT.ap())
        dsw = cp.tile([P, 2, HD], F32R)
        nc.gpsimd.dma_start(dsw[:], DSW.ap())
        bcast = cp.tile([P, BC_COLS], BF16)
        nc.gpsimd.dma_start(bcast[:], BCAST.ap())
        ffw1 = cp.tile([P, L, 4, HD], BF16)
        nc.gpsimd.dma_start(ffw1[:], FFW1.ap().rearrange("l p a h -> p l a h"))
        ffw2 = cp.tile([P, L, 4, HD], BF16)
        nc.gpsimd.dma_start(ffw2[:], FFW2.ap().rearrange("l p a h -> p l a h"))
        dew1 = cp.tile([P, 4, F], BF16)
        nc.gpsimd.dma_start(dew1[:], DEW1.ap())
        dew2 = cp.tile([P, 2, F], BF16)
        nc.gpsimd.dma_start(dew2[:], DEW2.ap())
        dew3 = cp.tile([P, 2, R], BF16)
        nc.gpsimd.dma_start(dew3[:], DEW3.ap())
        ident = cp.tile([P, P], F32)
        make_identity(nc, ident[:])
        epst = cp.tile([P, 1], F32)
        nc.vector.memset(epst[:], EPS)

        def relu_out(ps_ap, out_ap, scale=None, bias=None):
            e = relu_rr.next()
            if e == "A":
                if bias is None:
                    nc.scalar.activation(out=out_ap, in_=ps_ap,
                                         func=mybir.ActivationFunctionType.Relu,
                                         scale=scale)
                else:
                    nc.scalar.activation(out=out_ap, in_=ps_ap,
                                         func=mybir.ActivationFunctionType.Relu,
                                         bias=pbias[:, bias[0]:bias[0] + 1])
            else:
                eng = nc.vector if e == "D" else nc.gpsimd
                if bias is None:
                    eng.tensor_scalar(out=out_ap, in0=ps_ap,
                                      scalar1=float(scale), scalar2=0.0,
                                      op0=mybir.AluOpType.mult,
                                      op1=mybir.AluOpType.max)
                else:
                    eng.tensor_scalar(out=out_ap, in0=ps_ap,
                                      scalar1=pbias[:, bias[1]:bias[1] + 1],
                                      scalar2=pbias[:, bias[0]:bias[0] + 1],
                                      op0=mybir.AluOpType.max,
                                      op1=mybir.AluOpType.add)

        def copy_out(ps_ap, out_ap, scale):
            e = copy_rr.next()
            if e == "A":
                nc.scalar.activation(out=out_ap, in_=ps_ap,
                                     func=mybir.ActivationFunctionType.Copy,
                                     scale=scale)
            else:
                eng = nc.vector if e == "D" else nc.gpsimd
                eng.tensor_scalar(out=out_ap, in0=ps_ap,
                                  scalar1=float(scale), scalar2=0.0,
                                  op0=mybir.AluOpType.mult,
                                  op1=mybir.AluOpType.add)

        def k_chain(c, h, stag):
            """fp8 DoubleRow chain, software-pipelined:
            L1(ci) -> L2(ci-1) -> L3(ci-2) so relus hide under matmuls."""
            w1 = wp.tile([P, 2, F], FP8, tag="w1k")
            nc.sync.dma_start(w1[:], W1K8.ap()[c])
            w2 = wp.tile([P, 2, F], FP8, tag="w2k")
            nc.sync.dma_start(w2[:], W2K8.ap()[c])
            w3 = wp.tile([P, 2, DK], FP8, tag="w3k")
            nc.sync.dma_start(w3[:], W3K8.ap()[c])
            w1l = wp.tile([97, F], BF16, tag="w1lk")
            nc.sync.dma_start(w1l[0:97:32, :], W1LB.ap()[c])

            st = {}

            def l1(ci):
                col, cw = chunks[ci]
                h1t = hp.tile([P, 2, 512], FP8, tag="h1k")
                # rank-1 u-terms OPEN the full psum region (a start=False
                # accumulate spanning two DR-opened regions corrupts psum, so
                # open wide first, then let the DR mains accumulate densely)
                ps = []
                for ft in range(2):
                    fsl = slice(ft * P, (ft + 1) * P)
                    rg = 32 * ft
                    ps1 = pp.tile([P, 512], F32, tag="ps")
                    ps.append(ps1)
                    nc.tensor.matmul(ps1[:, 0:cw], w1l[rg:rg + 1, fsl],
                                     xtub[rg:rg + 1, col:col + cw],
                                     start=True, stop=False, tile_position=(rg, 0))
                for ft in range(2):
                    fsl = slice(ft * P, (ft + 1) * P)
                    for cc in range(0, cw, 256):
                        ccw = min(256, cw - cc)
                        csl = slice(col + cc, col + cc + ccw)
                        nc.tensor.matmul(ps[ft][:, cc:cc + ccw], w1[:, :, fsl],
                                         xt8[:, :, csl], start=False,
                                         stop=(cc + 256 >= cw),
                                         perf_mode=DRMODE, skip_group_check=True)
                for ft in range(2):
                    relu_out(ps[ft][:, 0:cw], h1t[:, ft, 0:cw], scale=1.0 / H1DIV)
                st[ci] = [h1t, None]

            def l2(ci):
                col, cw = chunks[ci]
                h1t = st[ci][0]
                h2t = hp.tile([P, 2, 512], FP8, tag="h2k")
                ps = []
                for gt in range(2):
                    gsl = slice(gt * P, (gt + 1) * P)
                    ps2 = pp.tile([P, 512], F32, tag="ps")
                    ps.append(ps2)
                    for cc in range(0, cw, 256):
                        ccw = min(256, cw - cc)
                        nc.tensor.matmul(ps2[:, cc:cc + ccw], w2[:, :, gsl],
                                         h1t[:, :, cc:cc + ccw], start=True,
                                         stop=True, perf_mode=DRMODE)
                for gt in range(2):
                    relu_out(ps[gt][:, 0:cw], h2t[:, gt, 0:cw], scale=1.0,
                             bias=((PB_B2KP + 2 * c + gt, PB_B2KN + 2 * c + gt)
                                   if has_b2 else None))
                st[ci][1] = h2t

            def l3(ci):
                col, cw = chunks[ci]
                nrti = cw // P
                h2t = st[ci][1]
                ps3 = pq3.tile([P, 256], F32, tag="ps3")
                for rti in range(nrti):
                    rsl = slice(rti * P, (rti + 1) * P)
                    nc.tensor.matmul(ps3[:, rti * DK:(rti + 1) * DK],
                                     h2t[:, :, rsl], w3[:, :, :],
                                     start=True, stop=True, perf_mode=DRMODE)
                copy_out(ps3[:, 0:nrti * DK].rearrange("p (r d) -> p r d", d=DK),
                         stag[:, col // P:col // P + nrti, h * DK:(h + 1) * DK],
                         1.0 / L3KDIV)

            _pipeline(l1, l2, l3)

        def _pipeline(l1, l2, l3):
            n = len(chunks)
            for ci in range(n):
                l1(ci)
                if ci >= 1:
                    l2(ci - 1)
                if ci >= 2:
                    l3(ci - 2)
            l2(n - 1)
            if n >= 2:
                l3(n - 2)
            l3(n - 1)

        def v_chain(c, h, stag):
            w1 = wp.tile([P, 2, F], BF16, tag="w1v")
            nc.scalar.dma_start(w1[:], W1V.ap()[c])
            w2 = wp.tile([P, 2, F], BF16, tag="w2v")
            nc.scalar.dma_start(w2[:], W2V.ap()[c])
            w3 = wp.tile([P, 2, DK], BF16, tag="w3v")
            nc.scalar.dma_start(w3[:], W3V.ap()[c])
            w1l = wp.tile([97, F], BF16, tag="w1lv")
            nc.scalar.dma_start(w1l[0:97:32, :], W1LB.ap()[NCH + c])

            st = {}

            def l1(ci):
                col, cw = chunks[ci]
                h1t = hp.tile([P, 2, 512], BF16, tag="h1v")
                ps = []
                for ft in range(2):
                    fsl = slice(ft * P, (ft + 1) * P)
                    rg = 32 * ft
                    ps1 = pp.tile([P, 512], F32, tag="ps")
                    ps.append(ps1)
                    nc.tensor.matmul(ps1[:, 0:cw], w1l[rg:rg + 1, fsl],
                                     xtub[rg:rg + 1, col:col + cw],
                                     start=True, stop=False, tile_position=(rg, 0))
                for ft in range(2):
                    fsl = slice(ft * P, (ft + 1) * P)
                    nc.tensor.matmul(ps[ft][:, 0:cw], w1[:, 0, fsl],
                                     xtb[:, 0, col:col + cw], start=False, stop=False,
                                     skip_group_check=True)
                    nc.tensor.matmul(ps[ft][:, 0:cw], w1[:, 1, fsl],
                                     xtb[:, 1, col:col + cw], start=False, stop=True,
                                     skip_group_check=True)
                for ft in range(2):
                    relu_out(ps[ft][:, 0:cw], h1t[:, ft, 0:cw], scale=1.0 / V1DIV)
                st[ci] = [h1t, None]

            def l2(ci):
                col, cw = chunks[ci]
                h1t = st[ci][0]
                h2t = hp.tile([P, 2, 512], BF16, tag="h2v")
                ps = []
                for gt in range(2):
                    gsl = slice(gt * P, (gt + 1) * P)
                    ps2 = pp.tile([P, 512], F32, tag="ps")
                    ps.append(ps2)
                    nc.tensor.matmul(ps2[:, 0:cw], w2[:, 0, gsl],
                                     h1t[:, 0, 0:cw], start=True, stop=False)
                    nc.tensor.matmul(ps2[:, 0:cw], w2[:, 1, gsl],
                                     h1t[:, 1, 0:cw], start=False, stop=True)
                for gt in range(2):
                    relu_out(ps[gt][:, 0:cw], h2t[:, gt, 0:cw], scale=1.0,
                             bias=((PB_B2VP + 2 * c + gt, PB_B2VN + 2 * c + gt)
                                   if has_b2 else None))
                st[ci][1] = h2t

            def l3(ci):
                col, cw = chunks[ci]
                nrti = cw // P
                h2t = st[ci][1]
                ps3 = pq3.tile([P, 256], F32, tag="ps3")
                for rti in range(nrti):
                    rsl = slice(rti * P, (rti + 1) * P)
                    nc.tensor.matmul(ps3[:, rti * DK:(rti + 1) * DK],
                                     h2t[:, 0, rsl], w3[:, 0, :],
                                     start=True, stop=False)
                    nc.tensor.matmul(ps3[:, rti * DK:(rti + 1) * DK],
                                     h2t[:, 1, rsl], w3[:, 1, :],
                                     start=False, stop=True)
                copy_out(ps3[:, 0:nrti * DK].rearrange("p (r d) -> p r d", d=DK),
                         stag[:, col // P:col // P + nrti, h * DK:(h + 1) * DK],
                         1.0)

            _pipeline(l1, l2, l3)

        def ds_project():
            ps = pqa.tile([P, HD], F32, tag="pa")
            nc.tensor.matmul(ps[:], predt[:, 0, :], dsw[:, 0, :], start=True, stop=False)
            nc.tensor.matmul(ps[:], predt[:, 1, :], dsw[:, 1, :], start=False, stop=True)
            av0 = ap.tile([P, HD], F32, tag="av")
            nc.vector.tensor_tensor(out=av0[:], in0=ps[:],
                                    in1=bcast[:, BC_DSB:BC_DSB + HD],
                                    op=mybir.AluOpType.add)
            return av0

        def layer_norm(src, gcol, bcol, out):
            stats = ap.tile([P, 6], F32, tag="lnstat")
            nc.vector.bn_stats(stats[:], src[:])
            mv = ap.tile([P, 2], F32, tag="lnmv")
            nc.vector.bn_aggr(mv[:], stats[:])
            rstd = ap.tile([P, 1], F32, tag="lnrstd")
            nc.scalar.activation(out=rstd[:], in_=mv[:, 1:2],
                                 func=mybir.ActivationFunctionType.Sqrt,
                                 bias=epst[:, 0:1])
            nc.vector.reciprocal(rstd[:], rstd[:])
            nc.vector.tensor_scalar(out=out[:], in0=src[:], scalar1=mv[:, 0:1],
                                    scalar2=rstd[:, 0:1],
                                    op0=mybir.AluOpType.subtract,
                                    op1=mybir.AluOpType.mult)
            nc.vector.tensor_tensor(out=out[:], in0=out[:],
                                    in1=bcast[:, gcol:gcol + HD],
                                    op=mybir.AluOpType.mult)
            nc.vector.tensor_tensor(out=out[:], in0=out[:],
                                    in1=bcast[:, bcol:bcol + HD],
                                    op=mybir.AluOpType.add)

        def transpose_to(src, dst):
            for kt in range(4):
                tp_ = pqa.tile([P, P], F32, tag="pa")
                nc.tensor.transpose(tp_[:], src[:, kt * P:(kt + 1) * P], ident[:])
                nc.scalar.activation(out=dst[:, kt, :], in_=tp_[:],
                                     func=mybir.ActivationFunctionType.Copy)

        def gather0(gall):
            """l=0: combined K+V rows from KV0D into [P, 16, 2, HD]."""
            for j in range(16):
                n, s = j // 8, j % 8
                if n == 1 and s < i_val:
                    sec = sec_starts[s]
                    nc.sync.dma_start(gall[:, j, :, :], KV0D.ap()[sec:sec + P, :, :])
                else:
                    # indirect DMA requires 2D src/dst APs (3D reads garbage)
                    nc.gpsimd.indirect_dma_start(
                        out=gall[:, j, :, :].rearrange("p a hd -> p (a hd)"),
                        out_offset=None,
                        in_=KV0D.ap().rearrange("r a hd -> r (a hd)"),
                        in_offset=bass.IndirectOffsetOnAxis(ap=idx[:, j:j + 1], axis=0))

        def gather1(src, gall):
            """l=1: one of KD1/VD1 into [P, 16, HD]."""
            for j in range(16):
                n, s = j // 8, j % 8
                if n == 1 and s < i_val:
                    sec = sec_starts[s]
                    nc.sync.dma_start(gall[:, j, :], src.ap()[sec:sec + P, :])
                else:
                    nc.gpsimd.indirect_dma_start(
                        out=gall[:, j, :], out_offset=None, in_=src.ap(),
                        in_offset=bass.IndirectOffsetOnAxis(ap=idx[:, j:j + 1], axis=0))

        def attn_phase1(l, av_in, wall, kg):
            """kg: callable js_slice -> [P, 4, HD] bf16 K rows."""
            avbf = ap.tile([P, HD], BF16, tag="avbf")
            nc.vector.tensor_copy(out=avbf[:], in_=av_in[:])
            scrb = ap.tile([P, 4, HD], BF16, tag="scrb")
            logits = ap.tile([P, 16, 8], BF16, tag="logits")
            avb = avbf[:, None, :].to_broadcast([P, 4, HD])
            for js in range(4):
                jsl = slice(js * 4, (js + 1) * 4)
                nc.vector.tensor_tensor(out=scrb[:], in0=kg(jsl),
                                        in1=avb, op=mybir.AluOpType.mult)
                with nc.allow_low_precision(reason="logits bf16; sigmoid smooths"):
                    nc.vector.tensor_reduce(
                        out=logits[:, jsl, :],
                        in_=scrb[:].rearrange("p a (h d) -> p (a h) d", d=DK),
                        axis=mybir.AxisListType.X, op=mybir.AluOpType.add)
            delta = ap.tile([P, 8, 8], BF16, tag="delta")
            nc.vector.tensor_tensor(out=delta[:], in0=logits[:, 0:8, :],
                                    in1=logits[:, 8:16, :],
                                    op=mybir.AluOpType.subtract)
            scale = DK ** (-0.5)
            dflat = delta[:].rearrange("p a b -> p (a b)")
            nc.scalar.activation(out=wall[:, 0:64], in_=dflat,
                                 func=mybir.ActivationFunctionType.Sigmoid,
                                 scale=scale)
            nc.scalar.activation(out=wall[:, 64:128], in_=dflat,
                                 func=mybir.ActivationFunctionType.Sigmoid,
                                 scale=-scale)

        def phase2_step(l, wall, vg, js, att, part, scr, half=None):
            jsl = slice(js * 4, (js + 1) * 4)
            wv = wall[:].rearrange("p (j h) -> p j h", h=8)
            if half is None:
                hsl, nh = slice(0, HD), 8
            else:
                hsl, nh = slice(half * (HD // 2), (half + 1) * (HD // 2)), 4
            wvh = wv[:, jsl, half * 4:half * 4 + 4] if half is not None \
                else wv[:, jsl, :]
            nc.vector.tensor_tensor(
                out=scr[:, :, hsl].rearrange("p a (h d) -> p a h d", d=DK),
                in0=vg(jsl).rearrange("p a (h d) -> p a h d", d=DK),
                in1=wvh[:, :, :, None].to_broadcast([P, 4, nh, DK]),
                op=mybir.AluOpType.mult)
            dst = att if js == 0 else part
            nc.vector.tensor_reduce(
                out=dst[:, hsl],
                in_=scr[:, :, hsl].rearrange("p a hd -> p hd a"),
                axis=mybir.AxisListType.X, op=mybir.AluOpType.add)
            if js > 0:
                nc.vector.tensor_tensor(out=att[:, hsl], in0=att[:, hsl],
                                        in1=part[:, hsl],
                                        op=mybir.AluOpType.add)

        def attn_phase2_tail(l, avp, att):
            nc.vector.tensor_tensor(out=att[:], in0=att[:], in1=avp[:],
                                    op=mybir.AluOpType.add)
            xn = ap.tile([P, HD], F32, tag="xn")
            layer_norm(att, BC_LN1G + l * HD, BC_LN1B + l * HD, xn)
            xT = ap.tile([P, 4, P], BF16, tag="xT")
            transpose_to(xn, xT)
            ff1 = ap.tile([P, 4, P], BF16, tag="ff1")
            for ft in range(4):
                psf = pqa.tile([P, P], F32, tag="pa")
                for kt in range(4):
                    nc.tensor.matmul(psf[:], ffw1[:, l, kt, ft * P:(ft + 1) * P],
                                     xT[:, kt, :], start=(kt == 0), stop=(kt == 3))
                nc.scalar.activation(
                    out=ff1[:, ft, :], in_=psf[:],
                    func=mybir.ActivationFunctionType.Relu,
                    bias=pbias[:, PB_FFB1 + 4 * l + ft:PB_FFB1 + 4 * l + ft + 1])
            ps2 = pqa.tile([P, HD], F32, tag="pa")
            for kt in range(4):
                nc.tensor.matmul(ps2[:], ff1[:, kt, :], ffw2[:, l, kt, :],
                                 start=(kt == 0), stop=(kt == 3))
            ffx = ap.tile([P, HD], F32, tag="ffx")
            nc.vector.tensor_tensor(out=ffx[:], in0=ps2[:],
                                    in1=bcast[:, BC_FFB2 + l * HD:BC_FFB2 + (l + 1) * HD],
                                    op=mybir.AluOpType.add)
            nc.vector.tensor_tensor(out=ffx[:], in0=ffx[:], in1=xn[:],
                                    op=mybir.AluOpType.add)
            av_out = ap.tile([P, HD], F32, tag="av")
            layer_norm(ffx, BC_LN2G + l * HD, BC_LN2B + l * HD, av_out)
            return av_out

        def make_avp(l, av_in):
            avp = ap.tile([P, HD], F32, tag="avp")
            nc.vector.tensor_tensor(
                out=avp[:], in0=av_in[:],
                in1=bcast[:, BC_B3V8 + l * HD:BC_B3V8 + (l + 1) * HD],
                op=mybir.AluOpType.add)
            return avp

        # ================= schedule =================
        stag = sp.tile([P, NRT, HD], BF16, tag="stag")
        av = None
        for h in range(H):
            k_chain(0 * 8 + h, h, stag)
            if h == 0:
                av = ds_project()
        nc.sync.dma_start(
            KV0D.ap()[:, 0, :].rearrange("(rt p) hd -> p rt hd", p=P), stag[:])

        stag = sp.tile([P, NRT, HD], BF16, tag="stag")
        for h in range(H):
            v_chain(0 * 8 + h, h, stag)
        nc.sync.dma_start(
            KV0D.ap()[:, 1, :].rearrange("(rt p) hd -> p rt hd", p=P), stag[:])
        kv0gall = gp0.tile([P, 16, 2, HD], BF16, tag="gall0")
        gather0(kv0gall)

        # l=1 K chains; l=0 attention interleaved (phase2 spread over chains)
        stag = sp.tile([P, NRT, HD], BF16, tag="stag")
        wall0 = ap.tile([P, 128], F32, tag="wall")
        att0 = ap.tile([P, HD], F32, tag="att")
        part0 = ap.tile([P, HD], F32, tag="part")
        scr0 = ap.tile([P, 4, HD], F32, tag="scr")
        vg0 = lambda jsl: kv0gall[:, jsl, 1, :]
        avp0 = None
        for h in range(H):
            k_chain(1 * 8 + h, h, stag)
            if h == 1:
                attn_phase1(0, av, wall0, lambda jsl: kv0gall[:, jsl, 0, :])
                avp0 = make_avp(0, av)
            elif 2 <= h <= 5:
                phase2_step(0, wall0, vg0, h - 2, att0, part0, scr0)
        nc.sync.dma_start(
            KD1.ap().rearrange("(rt p) hd -> p rt hd", p=P), stag[:])
        k1gall = gp1.tile([P, 16, HD], BF16, tag="gall1")
        gather1(KD1, k1gall)
        av = attn_phase2_tail(0, avp0, att0)

        # l=1 V chains; l=1 phase1 interleaved. The stag is written to DRAM
        # in two head-halves so V gathers + weighted sums start while the
        # last four chains still compute.
        stag = sp.tile([P, NRT, HD], BF16, tag="stag")
        wall1 = ap.tile([P, 128], F32, tag="wall")
        att1 = ap.tile([P, HD], F32, tag="att")
        part1 = ap.tile([P, HD], F32, tag="part")
        scr1 = ap.tile([P, 4, HD], F32, tag="scr")
        v1gall = gp1.tile([P, 16, HD], BF16, tag="gall1")
        avp1 = None

        def v1_half(half, src_t):
            hsl = slice(half * (HD // 2), (half + 1) * (HD // 2))
            nc.sync.dma_start(
                src_t.ap().rearrange("(rt p) hd -> p rt hd", p=P),
                stag[:, :, hsl])
            for js in range(4):
                for j in range(js * 4, js * 4 + 4):
                    n, s = j // 8, j % 8
                    if n == 1 and s < i_val:
                        nc.sync.dma_start(
                            v1gall[:, j, hsl],
                            src_t.ap()[sec_starts[s]:sec_starts[s] + P, :])
                    else:
                        nc.gpsimd.indirect_dma_start(
                            out=v1gall[:, j, hsl], out_offset=None,
                            in_=src_t.ap(),
                            in_offset=bass.IndirectOffsetOnAxis(
                                ap=idx[:, j:j + 1], axis=0))
                phase2_step(1, wall1, lambda jsl: v1gall[:, jsl, hsl], js,
                            att1, part1, scr1, half=half)

        for h in range(H):
            v_chain(1 * 8 + h, h, stag)
            if h == 1:
                attn_phase1(1, av, wall1, lambda jsl: k1gall[:, jsl, :])
                avp1 = make_avp(1, av)
            elif h == 4:
                v1_half(0, VD1A)
        v1_half(1, VD1B)
        av = attn_phase2_tail(1, avp1, att1)

        # ---- dist extractor ----
        avT = ap.tile([P, 4, P], BF16, tag="avT")
        transpose_to(av, avT)
        h1 = ap.tile([P, 2, P], BF16, tag="deh1")
        for ft in range(2):
            psd = pqa.tile([P, P], F32, tag="pa")
            for kt in range(4):
                nc.tensor.matmul(psd[:], dew1[:, kt, ft * P:(ft + 1) * P],
                                 avT[:, kt, :], start=(kt == 0), stop=(kt == 3))
            nc.scalar.activation(out=h1[:, ft, :], in_=psd[:],
                                 func=mybir.ActivationFunctionType.Relu,
                                 bias=pbias[:, PB_DEB1 + ft:PB_DEB1 + ft + 1])
        h2 = ap.tile([P, 2, P], BF16, tag="deh2")
        for ft in range(2):
            psd = pqa.tile([P, P], F32, tag="pa")
            for kt in range(2):
                nc.tensor.matmul(psd[:], dew2[:, kt, ft * P:(ft + 1) * P],
                                 h1[:, kt, :], start=(kt == 0), stop=(kt == 1))
            nc.scalar.activation(out=h2[:, ft, :], in_=psd[:],
                                 func=mybir.ActivationFunctionType.Relu,
                                 bias=pbias[:, PB_DEB2 + ft:PB_DEB2 + ft + 1])
        pso = pqa.tile([P, R], F32, tag="pa")
        for kt in range(2):
            nc.tensor.matmul(pso[:], h2[:, kt, :], dew3[:, kt, :],
                             start=(kt == 0), stop=(kt == 1))
        o = ap.tile([P, R], F32, tag="out")
        nc.vector.tensor_tensor(out=o[:], in0=pso[:],
                                in1=bcast[:, BC_DEB3:BC_DEB3 + R],
                                op=mybir.AluOpType.add)
        nc.sync.dma_start(OUT.ap()[:, 0:R // 2], o[:, 0:R // 2])
        nc.scalar.dma_start(OUT.ap()[:, R // 2:], o[:, R // 2:])


_BUILD_CACHE = {}


def _build(i_val, nrowc, sec_starts, has_b2=False):
    key = (i_val, nrowc, tuple(sec_starts), has_b2)
    if key in _BUILD_CACHE:
        return _BUILD_CACHE[key]
    nc = bacc.Bacc("TRN2", target_bir_lowering=False, debug=False)
    t = []
    t.append(nc.dram_tensor("XT8", [P, 2, nrowc], FP8, kind="ExternalInput"))
    t.append(nc.dram_tensor("XTB", [P, 2, nrowc], BF16, kind="ExternalInput"))
    t.append(nc.dram_tensor("XTUB", [4, nrowc], BF16, kind="ExternalInput"))
    t.append(nc.dram_tensor("PREDT", [P, 2, M], F32R, kind="ExternalInput"))
    t.append(nc.dram_tensor("DSW", [P, 2, HD], F32R, kind="ExternalInput"))
    t.append(nc.dram_tensor("W1K8", [NCH, P, 2, F], FP8, kind="ExternalInput"))
    t.append(nc.dram_tensor("W2K8", [NCH, P, 2, F], FP8, kind="ExternalInput"))
    t.append(nc.dram_tensor("W3K8", [NCH, P, 2, DK], FP8, kind="ExternalInput"))
    t.append(nc.dram_tensor("W1LB", [2 * NCH, 4, F], BF16, kind="ExternalInput"))
    t.append(nc.dram_tensor("W1V", [NCH, P, 2, F], BF16, kind="ExternalInput"))
    t.append(nc.dram_tensor("W2V", [NCH, P, 2, F], BF16, kind="ExternalInput"))
    t.append(nc.dram_tensor("W3V", [NCH, P, 2, DK], BF16, kind="ExternalInput"))
    t.append(nc.dram_tensor("FFW1", [L, P, 4, HD], BF16, kind="ExternalInput"))
    t.append(nc.dram_tensor("FFW2", [L, P, 4, HD], BF16, kind="ExternalInput"))
    t.append(nc.dram_tensor("DEW1", [P, 4, F], BF16, kind="ExternalInput"))
    t.append(nc.dram_tensor("DEW2", [P, 2, F], BF16, kind="ExternalInput"))
    t.append(nc.dram_tensor("DEW3", [P, 2, R], BF16, kind="ExternalInput"))
    t.append(nc.dram_tensor("PBIAS", [P, PB_COLS], F32, kind="ExternalInput"))
    t.append(nc.dram_tensor("BCAST", [P, BC_COLS], BF16, kind="ExternalInput"))
    t.append(nc.dram_tensor("IDX", [P, 16], I32, kind="ExternalInput"))
    t.append(nc.dram_tensor("OUT", [M, R], F32, kind="ExternalOutput"))
    t.append(nc.dram_tensor("KV0D", [nrowc, 2, HD], BF16))
    t.append(nc.dram_tensor("KD1", [nrowc, HD], BF16))
    t.append(nc.dram_tensor("VD1A", [nrowc, HD // 2], BF16))
    t.append(nc.dram_tensor("VD1B", [nrowc, HD // 2], BF16))
    with tile.TileContext(nc) as tc:
        _emit(nc, tc, t, i_val, nrowc, sec_starts, has_b2)
    nc.compile()
    _BUILD_CACHE[key] = nc
    return nc


def _fp8(x, scale):
    return np.clip(np.asarray(x, np.float32) * scale, -240.0, 240.0).astype(E4NP)


def plan_compaction(ins):
    i = int(ins["i"])
    left = np.asarray(ins["left_idx"], np.int64)
    right = np.asarray(ins["right_idx"], np.int64)
    m_ar = np.arange(M, dtype=np.int64)
    tsets = []
    for s in range(S):
        tset = np.union1d(left, m_ar) if s < i else np.union1d(left, right)
        tsets.append(np.sort(tset))
    sec_starts = []
    rows = []
    rowmap = {}
    for s in range(S):
        sec_starts.append(len(rows))
        for tt in tsets[s]:
            rowmap[(s, int(tt))] = len(rows)
            rows.append((s, int(tt)))
    nraw = len(rows)
    nrowc = ((nraw + P - 1) // P) * P
    idxm = np.zeros((P, 16), np.int32)
    for s in range(S):
        for mm in range(M):
            idxm[mm, 0 * 8 + s] = rowmap[(s, int(left[mm]))]
            t1 = mm if s < i else int(right[mm])
            idxm[mm, 1 * 8 + s] = rowmap[(s, t1)]
    return {
        "i": i, "rows": rows, "nrowc": nrowc,
        "sec_starts": tuple(sec_starts[:i]), "idx": idxm,
    }


def _prep_shared(ins, plan):
    f32 = np.float32
    kW1, kW2, kW3 = ins["kW1"], ins["kW2"], ins["kW3"]
    vW1, vW2, vW3 = ins["vW1"], ins["vW2"], ins["vW3"]
    kb1, kb2 = ins["kb1"], ins["kb2"]
    vb1, vb2, vb3 = ins["vb1"], ins["vb2"], ins["vb3"]
    if np.any(np.asarray(kb1, np.float32)) or np.any(np.asarray(vb1, np.float32)):
        raise NotImplementedError("nonzero kb1/vb1 not folded (u-row rank-1 "
                                  "carries no bias term)")

    W1K8 = np.empty((NCH, P, 2, F), E4NP)
    W2K8 = np.empty((NCH, P, 2, F), E4NP)
    W3K8 = np.empty((NCH, P, 2, DK), E4NP)
    W1V = np.empty((NCH, P, 2, F), BFNP)
    W2V = np.empty((NCH, P, 2, F), BFNP)
    W3V = np.empty((NCH, P, 2, DK), BFNP)
    W1LB = np.empty((2 * NCH, 4, F), BFNP)
    PB = np.zeros((P, PB_COLS), f32)
    BC = np.zeros((BC_COLS,), f32)

    def pack2(w, ncols):
        return np.asarray(w, f32).reshape(2, P, ncols).transpose(1, 0, 2)

    for l in range(L):
        for h in range(H):
            c = l * 8 + h
            W1K8[c] = _fp8(pack2(kW1[l, h][:256], F), W1S)
            W2K8[c] = _fp8(pack2(kW2[l, h], F), W2S)
            W3K8[c] = _fp8(pack2(kW3[l, h], DK), W3S)
            W1LB[c] = np.broadcast_to(
                (np.asarray(kW1[l, h][256], f32) * W1S).astype(BFNP), (4, F))
            W1V[c] = (pack2(vW1[l, h][:256], F) * 64.0).astype(BFNP)
            W2V[c] = pack2(vW2[l, h], F).astype(BFNP)
            W3V[c] = pack2(vW3[l, h], DK).astype(BFNP)
            W1LB[NCH + c] = np.broadcast_to(
                (np.asarray(vW1[l, h][256], f32) * W1S).astype(BFNP), (4, F))
            for gt in range(2):
                gsl = slice(gt * P, (gt + 1) * P)
                PB[:, PB_B2KP + 2 * c + gt] = B2KS * np.asarray(kb2[l, h][gsl], f32)
                PB[:, PB_B2KN + 2 * c + gt] = -B2KS * np.asarray(kb2[l, h][gsl], f32)
                PB[:, PB_B2VP + 2 * c + gt] = np.asarray(vb2[l, h][gsl], f32)
                PB[:, PB_B2VN + 2 * c + gt] = -np.asarray(vb2[l, h][gsl], f32)

    DSW = np.asarray(ins["ds_W"], f32).reshape(2, P, HD).transpose(1, 0, 2).copy()
    BC[BC_DSB:BC_DSB + HD] = np.asarray(ins["ds_b"], f32)
    for l in range(L):
        BC[BC_B3V8 + l * HD:BC_B3V8 + (l + 1) * HD] = \
            S * np.asarray(vb3[l], f32).reshape(HD)

    FFW1 = np.empty((L, P, 4, HD), BFNP)
    FFW2 = np.empty((L, P, 4, HD), BFNP)
    for l in range(L):
        FFW1[l] = np.asarray(ins["ffW1"][l], f32).reshape(4, P, HD).transpose(1, 0, 2).astype(BFNP)
        FFW2[l] = np.asarray(ins["ffW2"][l], f32).reshape(4, P, HD).transpose(1, 0, 2).astype(BFNP)
        for ft in range(4):
            PB[:, PB_FFB1 + 4 * l + ft] = np.asarray(ins["ffb1"][l][ft * P:(ft + 1) * P], f32)
        BC[BC_FFB2 + l * HD:BC_FFB2 + (l + 1) * HD] = np.asarray(ins["ffb2"][l], f32)
        BC[BC_LN1G + l * HD:BC_LN1G + (l + 1) * HD] = np.asarray(ins["ln1_g"][l], f32)
        BC[BC_LN1B + l * HD:BC_LN1B + (l + 1) * HD] = np.asarray(ins["ln1_b"][l], f32)
        BC[BC_LN2G + l * HD:BC_LN2G + (l + 1) * HD] = np.asarray(ins["ln2_g"][l], f32)
        BC[BC_LN2B + l * HD:BC_LN2B + (l + 1) * HD] = np.asarray(ins["ln2_b"][l], f32)

    DEW1 = np.asarray(ins["deW1"], f32).reshape(4, P, F).transpose(1, 0, 2).astype(BFNP)
    DEW2 = np.asarray(ins["deW2"], f32).reshape(2, P, F).transpose(1, 0, 2).astype(BFNP)
    DEW3 = np.asarray(ins["deW3"], f32).reshape(2, P, R).transpose(1, 0, 2).astype(BFNP)
    for ft in range(2):
        PB[:, PB_DEB1 + ft] = np.asarray(ins["deb1"][ft * P:(ft + 1) * P], f32)
        PB[:, PB_DEB2 + ft] = np.asarray(ins["deb2"][ft * P:(ft + 1) * P], f32)
    BC[BC_DEB3:BC_DEB3 + R] = np.asarray(ins["deb3"], f32)

    BCAST = np.broadcast_to(BC.astype(BFNP), (P, BC_COLS)).copy()

    return {
        "W1K8": W1K8, "W2K8": W2K8, "W3K8": W3K8, "W1LB": W1LB,
        "W1V": W1V, "W2V": W2V, "W3V": W3V,
        "DSW": DSW, "FFW1": FFW1, "FFW2": FFW2,
        "DEW1": DEW1, "DEW2": DEW2, "DEW3": DEW3,
        "PBIAS": PB, "BCAST": BCAST, "IDX": plan["idx"],
    }


def make_in_maps(ins, plan=None):
    if plan is None:
        plan = plan_compaction(ins)
    shared = _prep_shared(ins, plan)
    enc = np.asarray(ins["encoded"], np.float32)
    tu = np.asarray(ins["true_u"], np.float32)
    mid = np.asarray(ins["mid_idx"], np.int64)
    i = plan["i"]
    nrowc = plan["nrowc"]
    rows = plan["rows"]
    s_idx = np.array([r[0] for r in rows], np.int64)
    t_idx = np.array([r[1] for r in rows], np.int64)

    in_maps = []
    for b in range(B):
        xg = np.zeros((nrowc, D), np.float32)
        xg[:len(rows)] = enc[b][s_idx, t_idx]
        ug = np.zeros((nrowc,), np.float32)
        ug[:len(rows)] = tu[b][s_idx, t_idx]
        xs16 = xg.T.reshape(2, P, nrowc).transpose(1, 0, 2) * XS
        xub = np.broadcast_to((ug * XS).astype(BFNP), (4, nrowc))
        pred = enc[b, i][mid]
        predt = pred.T.reshape(2, P, M).transpose(1, 0, 2).copy()
        m = dict(shared)
        m["XT8"] = _fp8(xs16, 1.0)
        m["XTB"] = xs16.astype(BFNP)
        m["XTUB"] = np.ascontiguousarray(xub)
        m["PREDT"] = predt
        in_maps.append(m)
    return in_maps


def kernel(**inputs):
    ins = {k: np.asarray(v) for k, v in inputs.items()}
    plan = plan_compaction(ins)
    in_maps = make_in_maps(ins, plan)
    has_b2 = bool(np.any(np.asarray(ins["kb2"], np.float32))
                  or np.any(np.asarray(ins["vb2"], np.float32)))
    nc = _build(plan["i"], plan["nrowc"], plan["sec_starts"], has_b2)
    res = run_bass_kernel_spmd(nc, in_maps, core_ids=list(range(NCORES)))
    out = np.stack([res.results[c]["OUT"] for c in range(NCORES)])
    return out.astype(np.float32)


# revision 22
# speedup vs baseline: 1.0436x; 1.0066x over previous
"""Trainium2 Bass kernel for nn_AttentionalCopula (sparse_attention).

Sharding: data-parallel over batch (B=8 -> 8 cores); per-head K/V MLP stacks
computed locally per core. Weights replicated. No collectives.

Key optimizations over the v1 baseline:
  * Row compaction: the attention only gathers K/V rows from
    union_s({left} u ({arange} if s<i else {right})) -- ~1350 of 2048 rows.
    The MLP chains run only on those rows.
  * K chains in fp8 (float8e4) with MatmulPerfMode.DoubleRow: K_eff=256 per
    matmul at 0.5 cyc/col (4x bf16). V chains stay bf16 (V values feed the
    output directly; fp8 there fails the accuracy gate -- measured).
  * The u-row (feature 257) enters L1 psum via a K=1 bf16 rank-1 matmul
    (tile_position-paired across the two f-tiles).
  * Relu/copy work is round-robined across ACT/DVE/Pool engines.
  * l=0 K/V staged interleaved in DRAM so one indirect gather per slot
    fetches both; contiguous slots (n=1, s<i) use direct DMA.
"""

from contextlib import ExitStack

import ml_dtypes
import numpy as np

import concourse.bass as bass
import concourse.mybir as mybir
import concourse.tile as tile
from concourse import bacc
from concourse.bass_utils import run_bass_kernel_spmd
from concourse.masks import make_identity

B, S, T, D = 8, 8, 256, 256
H, DK = 8, 64
HD = H * DK            # 512
L = 2
F = 256
R = 512
M = 128
EPS = 1e-5
NCORES = 8
P = 128

F32 = mybir.dt.float32
F32R = mybir.dt.float32r
BF16 = mybir.dt.bfloat16
FP8 = mybir.dt.float8e4
I32 = mybir.dt.int32
E4NP = ml_dtypes.float8_e4m3
BFNP = ml_dtypes.bfloat16
DRMODE = mybir.MatmulPerfMode.DoubleRow

NCH = L * H            # 16 chains per kv side

# fp8 scale plan (K side):  X*16, W1*64 -> psum 1024*pre1 ; h1'=psum/64 (16*h1)
# W2*2 -> psum2 32*pre2 ; h2' = max(psum2 + 32*b2, 0) (32*h2)
# W3*64 -> psum3 2048*kv ; stag = psum3/2048
XS = 16.0
W1S = 64.0
H1DIV = 64.0
W2S = 2.0
B2KS = 32.0
W3S = 64.0
L3KDIV = 2048.0
# V side: X*16 (bf16), W1*64 (bf16) -> psum 1024*pre1 ; h1 = psum/1024
V1DIV = 1024.0

# PBIAS layout (per-partition f32 scalars)
PB_FFB1 = 0                      # 4 per l
PB_DEB1 = PB_FFB1 + 4 * L
PB_DEB2 = PB_DEB1 + 2
PB_B2KP = PB_DEB2 + 2            # +32*b2k per (c, gt)
PB_B2KN = PB_B2KP + 2 * NCH      # -32*b2k
PB_B2VP = PB_B2KN + 2 * NCH      # +b2v
PB_B2VN = PB_B2VP + 2 * NCH      # -b2v
PB_COLS = PB_B2VN + 2 * NCH

# BCAST layout (free-dim vectors, bf16, replicated across partitions)
BC_DSB = 0
BC_B3V8 = BC_DSB + HD            # S * vb3 per l
BC_FFB2 = BC_B3V8 + L * HD
BC_DEB3 = BC_FFB2 + L * HD
BC_LN1G = BC_DEB3 + R
BC_LN1B = BC_LN1G + L * HD
BC_LN2G = BC_LN1B + L * HD
BC_LN2B = BC_LN2G + L * HD
BC_COLS = BC_LN2B + L * HD


def _chunks(nrowc):
    out = []
    col = 0
    while col < nrowc:
        w = min(512, nrowc - col)
        out.append((col, w))
        col += w
    return out


class _RR:
    def __init__(self, pattern):
        self.pattern = pattern
        self.i = 0

    def next(self):
        e = self.pattern[self.i % len(self.pattern)]
        self.i += 1
        return e


def _emit(nc, tc, t, i_val, nrowc, sec_starts, has_b2):
    (XT8, XTB, XTUB, PREDT, DSW, W1K8, W2K8, W3K8, W1LB,
     W1V, W2V, W3V, FFW1, FFW2, DEW1, DEW2, DEW3,
     PBIAS, BCAST, IDX, OUT, KV0D, KD1, VD1A, VD1B) = t

    NRT = nrowc // P
    chunks = _chunks(nrowc)
    relu_rr = _RR("AAD")      # Pool cannot read PSUM on TRN2
    copy_rr = _RR("AD")

    with ExitStack() as ctx:
        cp = ctx.enter_context(tc.tile_pool(name="const", bufs=1))
        wp = ctx.enter_context(tc.tile_pool(name="w", bufs=3))
        sp = ctx.enter_context(tc.tile_pool(name="stag", bufs=2))
        hp = ctx.enter_context(tc.tile_pool(name="h", bufs=2))
        gp0 = ctx.enter_context(tc.tile_pool(name="gath0", bufs=1))
        gp1 = ctx.enter_context(tc.tile_pool(name="gath1", bufs=2))
        ap = ctx.enter_context(tc.tile_pool(name="attn", bufs=1))
        pp = ctx.enter_context(tc.tile_pool(name="ps", bufs=5, space="PSUM"))
        pq3 = ctx.enter_context(tc.tile_pool(name="ps3", bufs=2, space="PSUM"))
        pqa = ctx.enter_context(tc.tile_pool(name="pa", bufs=1, space="PSUM"))

        # ---- resident loads; chain-0 critical path first ----
        xt8 = cp.tile([P, 2, nrowc], FP8)
        nc.sync.dma_start(xt8[:, :, 0:256], XT8.ap()[:, :, 0:256])
        nc.scalar.dma_start(xt8[:, :, 256:512], XT8.ap()[:, :, 256:512])
        xtub = cp.tile([97, nrowc], BF16)
        nc.scalar.dma_start(xtub[0:97:32, :], XTUB.ap())
        pbias = cp.tile([P, PB_COLS], F32)
        nc.gpsimd.dma_start(pbias[:], PBIAS.ap())
        for col, w in chunks:
            if col == 0:
                continue
            nc.scalar.dma_start(xt8[:, :, col:col + w], XT8.ap()[:, :, col:col + w])
        idx = cp.tile([P, 16], I32)
        nc.gpsimd.dma_start(idx[:], IDX.ap())
        xtb = cp.tile([P, 2, nrowc], BF16)
        for col, w in chunks:
            nc.scalar.dma_start(xtb[:, :, col:col + w], XTB.ap()[:, :, col:col + w])
        predt = cp.tile([P, 2, M], F32R)
        nc.gpsimd.dma_start(predt[:], PREDT.ap())
        dsw = cp.tile([P, 2, HD], F32R)
        nc.gpsimd.dma_start(dsw[:], DSW.ap())
        bcast = cp.tile([P, BC_COLS], BF16)
        nc.gpsimd.dma_start(bcast[:], BCAST.ap())
        ffw1 = cp.tile([P, L, 4, HD], BF16)
        nc.gpsimd.dma_start(ffw1[:], FFW1.ap().rearrange("l p a h -> p l a h"))
        ffw2 = cp.tile([P, L, 4, HD], BF16)
        nc.gpsimd.dma_start(ffw2[:], FFW2.ap().rearrange("l p a h -> p l a h"))
        dew1 = cp.tile([P, 4, F], BF16)
        nc.gpsimd.dma_start(dew1[:], DEW1.ap())
        dew2 = cp.tile([P, 2, F], BF16)
        nc.gpsimd.dma_start(dew2[:], DEW2.ap())
        dew3 = cp.tile([P, 2, R], BF16)
        nc.gpsimd.dma_start(dew3[:], DEW3.ap())
        ident = cp.tile([P, P], F32)
        make_identity(nc, ident[:])
        epst = cp.tile([P, 1], F32)
        nc.vector.memset(epst[:], EPS)

        def relu_out(ps_ap, out_ap, scale=None, bias=None):
            e = relu_rr.next()
            if e == "A":
                if bias is None:
                    nc.scalar.activation(out=out_ap, in_=ps_ap,
                                         func=mybir.ActivationFunctionType.Relu,
                                         scale=scale)
                else:
                    nc.scalar.activation(out=out_ap, in_=ps_ap,
                                         func=mybir.ActivationFunctionType.Relu,
                                         bias=pbias[:, bias[0]:bias[0] + 1])
            else:
                eng = nc.vector if e == "D" else nc.gpsimd
                if bias is None:
                    eng.tensor_scalar(out=out_ap, in0=ps_ap,
                                      scalar1=float(scale), scalar2=0.0,
                                      op0=mybir.AluOpType.mult,
                                      op1=mybir.AluOpType.max)
                else:
                    eng.tensor_scalar(out=out_ap, in0=ps_ap,
                                      scalar1=pbias[:, bias[1]:bias[1] + 1],
                                      scalar2=pbias[:, bias[0]:bias[0] + 1],
                                      op0=mybir.AluOpType.max,
                                      op1=mybir.AluOpType.add)

        def copy_out(ps_ap, out_ap, scale):
            e = copy_rr.next()
            if e == "A":
                nc.scalar.activation(out=out_ap, in_=ps_ap,
                                     func=mybir.ActivationFunctionType.Copy,
                                     scale=scale)
            else:
                eng = nc.vector if e == "D" else nc.gpsimd
                eng.tensor_scalar(out=out_ap, in0=ps_ap,
                                  scalar1=float(scale), scalar2=0.0,
                                  op0=mybir.AluOpType.mult,
                                  op1=mybir.AluOpType.add)

        def k_chain(c, h, stag):
            """fp8 DoubleRow chain, software-pipelined:
            L1(ci) -> L2(ci-1) -> L3(ci-2) so relus hide under matmuls."""
            w1 = wp.tile([P, 2, F], FP8, tag="w1k")
            nc.sync.dma_start(w1[:], W1K8.ap()[c])
            w2 = wp.tile([P, 2, F], FP8, tag="w2k")
            nc.sync.dma_start(w2[:], W2K8.ap()[c])
            w3 = wp.tile([P, 2, DK], FP8, tag="w3k")
            nc.sync.dma_start(w3[:], W3K8.ap()[c])
            w1l = wp.tile([97, F], BF16, tag="w1lk")
            nc.sync.dma_start(w1l[0:97:32, :], W1LB.ap()[c])

            st = {}

            def l1(ci):
                col, cw = chunks[ci]
                h1t = hp.tile([P, 2, 512], FP8, tag="h1k")
                # rank-1 u-terms OPEN the full psum region (a start=False
                # accumulate spanning two DR-opened regions corrupts psum, so
                # open wide first, then let the DR mains accumulate densely)
                ps = []
                for ft in range(2):
                    fsl = slice(ft * P, (ft + 1) * P)
                    rg = 32 * ft
                    ps1 = pp.tile([P, 512], F32, tag="ps")
                    ps.append(ps1)
                    nc.tensor.matmul(ps1[:, 0:cw], w1l[rg:rg + 1, fsl],
                                     xtub[rg:rg + 1, col:col + cw],
                                     start=True, stop=False, tile_position=(rg, 0))
                for ft in range(2):
                    fsl = slice(ft * P, (ft + 1) * P)
                    for cc in range(0, cw, 256):
                        ccw = min(256, cw - cc)
                        csl = slice(col + cc, col + cc + ccw)
                        nc.tensor.matmul(ps[ft][:, cc:cc + ccw], w1[:, :, fsl],
                                         xt8[:, :, csl], start=False,
                                         stop=(cc + 256 >= cw),
                                         perf_mode=DRMODE, skip_group_check=True)
                for ft in range(2):
                    relu_out(ps[ft][:, 0:cw], h1t[:, ft, 0:cw], scale=1.0 / H1DIV)
                st[ci] = [h1t, None]

            def l2(ci):
                col, cw = chunks[ci]
                h1t = st[ci][0]
                h2t = hp.tile([P, 2, 512], FP8, tag="h2k")
                ps = []
                for gt in range(2):
                    gsl = slice(gt * P, (gt + 1) * P)
                    ps2 = pp.tile([P, 512], F32, tag="ps")
                    ps.append(ps2)
                    for cc in range(0, cw, 256):
                        ccw = min(256, cw - cc)
                        nc.tensor.matmul(ps2[:, cc:cc + ccw], w2[:, :, gsl],
                                         h1t[:, :, cc:cc + ccw], start=True,
                                         stop=True, perf_mode=DRMODE)
                for gt in range(2):
                    relu_out(ps[gt][:, 0:cw], h2t[:, gt, 0:cw], scale=1.0,
                             bias=((PB_B2KP + 2 * c + gt, PB_B2KN + 2 * c + gt)
                                   if has_b2 else None))
                st[ci][1] = h2t

            def l3(ci):
                col, cw = chunks[ci]
                nrti = cw // P
                h2t = st[ci][1]
                ps3 = pq3.tile([P, 256], F32, tag="ps3")
                for rti in range(nrti):
                    rsl = slice(rti * P, (rti + 1) * P)
                    nc.tensor.matmul(ps3[:, rti * DK:(rti + 1) * DK],
                                     h2t[:, :, rsl], w3[:, :, :],
                                     start=True, stop=True, perf_mode=DRMODE)
                copy_out(ps3[:, 0:nrti * DK].rearrange("p (r d) -> p r d", d=DK),
                         stag[:, col // P:col // P + nrti, h * DK:(h + 1) * DK],
                         1.0 / L3KDIV)

            _pipeline(l1, l2, l3)

        def _pipeline(l1, l2, l3):
            n = len(chunks)
            for ci in range(n):
                l1(ci)
                if ci >= 1:
                    l2(ci - 1)
                if ci >= 2:
                    l3(ci - 2)
            l2(n - 1)
            if n >= 2:
                l3(n - 2)
            l3(n - 1)

        def v_chain(c, h, stag):
            w1 = wp.tile([P, 2, F], BF16, tag="w1v")
            nc.scalar.dma_start(w1[:], W1V.ap()[c])
            w2 = wp.tile([P, 2, F], BF16, tag="w2v")
            nc.scalar.dma_start(w2[:], W2V.ap()[c])
            w3 = wp.tile([P, 2, DK], BF16, tag="w3v")
            nc.scalar.dma_start(w3[:], W3V.ap()[c])
            w1l = wp.tile([97, F], BF16, tag="w1lv")
            nc.scalar.dma_start(w1l[0:97:32, :], W1LB.ap()[NCH + c])

            st = {}

            def l1(ci):
                col, cw = chunks[ci]
                h1t = hp.tile([P, 2, 512], BF16, tag="h1v")
                ps = []
                for ft in range(2):
                    fsl = slice(ft * P, (ft + 1) * P)
                    rg = 32 * ft
                    ps1 = pp.tile([P, 512], F32, tag="ps")
                    ps.append(ps1)
                    nc.tensor.matmul(ps1[:, 0:cw], w1l[rg:rg + 1, fsl],
                                     xtub[rg:rg + 1, col:col + cw],
                                     start=True, stop=False, tile_position=(rg, 0))
                for ft in range(2):
                    fsl = slice(ft * P, (ft + 1) * P)
                    nc.tensor.matmul(ps[ft][:, 0:cw], w1[:, 0, fsl],
                                     xtb[:, 0, col:col + cw], start=False, stop=False,
                                     skip_group_check=True)
                    nc.tensor.matmul(ps[ft][:, 0:cw], w1[:, 1, fsl],
                                     xtb[:, 1, col:col + cw], start=False, stop=True,
                                     skip_group_check=True)
                for ft in range(2):
                    relu_out(ps[ft][:, 0:cw], h1t[:, ft, 0:cw], scale=1.0 / V1DIV)
                st[ci] = [h1t, None]

            def l2(ci):
                col, cw = chunks[ci]
                h1t = st[ci][0]
                h2t = hp.tile([P, 2, 512], BF16, tag="h2v")
                ps = []
                for gt in range(2):
                    gsl = slice(gt * P, (gt + 1) * P)
                    ps2 = pp.tile([P, 512], F32, tag="ps")
                    ps.append(ps2)
                    nc.tensor.matmul(ps2[:, 0:cw], w2[:, 0, gsl],
                                     h1t[:, 0, 0:cw], start=True, stop=False)
                    nc.tensor.matmul(ps2[:, 0:cw], w2[:, 1, gsl],
                                     h1t[:, 1, 0:cw], start=False, stop=True)
                for gt in range(2):
                    relu_out(ps[gt][:, 0:cw], h2t[:, gt, 0:cw], scale=1.0,
                             bias=((PB_B2VP + 2 * c + gt, PB_B2VN + 2 * c + gt)
                                   if has_b2 else None))
                st[ci][1] = h2t

            def l3(ci):
                col, cw = chunks[ci]
                nrti = cw // P
                h2t = st[ci][1]
                ps3 = pq3.tile([P, 256], F32, tag="ps3")
                for rti in range(nrti):
                    rsl = slice(rti * P, (rti + 1) * P)
                    nc.tensor.matmul(ps3[:, rti * DK:(rti + 1) * DK],
                                     h2t[:, 0, rsl], w3[:, 0, :],
                                     start=True, stop=False)
                    nc.tensor.matmul(ps3[:, rti * DK:(rti + 1) * DK],
                                     h2t[:, 1, rsl], w3[:, 1, :],
                                     start=False, stop=True)
                copy_out(ps3[:, 0:nrti * DK].rearrange("p (r d) -> p r d", d=DK),
                         stag[:, col // P:col // P + nrti, h * DK:(h + 1) * DK],
                         1.0)

            _pipeline(l1, l2, l3)

        def ds_project():
            ps = pqa.tile([P, HD], F32, tag="pa")
            nc.tensor.matmul(ps[:], predt[:, 0, :], dsw[:, 0, :], start=True, stop=False)
            nc.tensor.matmul(ps[:], predt[:, 1, :], dsw[:, 1, :], start=False, stop=True)
            av0 = ap.tile([P, HD], F32, tag="av")
            nc.vector.tensor_tensor(out=av0[:], in0=ps[:],
                                    in1=bcast[:, BC_DSB:BC_DSB + HD],
                                    op=mybir.AluOpType.add)
            return av0

        def layer_norm(src, gcol, bcol, out):
            stats = ap.tile([P, 6], F32, tag="lnstat")
            nc.vector.bn_stats(stats[:], src[:])
            mv = ap.tile([P, 2], F32, tag="lnmv")
            nc.vector.bn_aggr(mv[:], stats[:])
            rstd = ap.tile([P, 1], F32, tag="lnrstd")
            nc.scalar.activation(out=rstd[:], in_=mv[:, 1:2],
                                 func=mybir.ActivationFunctionType.Sqrt,
                                 bias=epst[:, 0:1])
            nc.vector.reciprocal(rstd[:], rstd[:])
            nc.vector.tensor_scalar(out=out[:], in0=src[:], scalar1=mv[:, 0:1],
                                    scalar2=rstd[:, 0:1],
                                    op0=mybir.AluOpType.subtract,
                                    op1=mybir.AluOpType.mult)
            nc.vector.tensor_tensor(out=out[:], in0=out[:],
                                    in1=bcast[:, gcol:gcol + HD],
                                    op=mybir.AluOpType.mult)
            nc.vector.tensor_tensor(out=out[:], in0=out[:],
                                    in1=bcast[:, bcol:bcol + HD],
                                    op=mybir.AluOpType.add)

        def transpose_to(src, dst):
            for kt in range(4):
                tp_ = pqa.tile([P, P], F32, tag="pa")
                nc.tensor.transpose(tp_[:], src[:, kt * P:(kt + 1) * P], ident[:])
                nc.scalar.activation(out=dst[:, kt, :], in_=tp_[:],
                                     func=mybir.ActivationFunctionType.Copy)

        def gather0(gall):
            """l=0: combined K+V rows from KV0D into [P, 16, 2, HD]."""
            for j in range(16):
                n, s = j // 8, j % 8
                if n == 1 and s < i_val:
                    sec = sec_starts[s]
                    nc.sync.dma_start(gall[:, j, :, :], KV0D.ap()[sec:sec + P, :, :])
                else:
                    # indirect DMA requires 2D src/dst APs (3D reads garbage)
                    nc.gpsimd.indirect_dma_start(
                        out=gall[:, j, :, :].rearrange("p a hd -> p (a hd)"),
                        out_offset=None,
                        in_=KV0D.ap().rearrange("r a hd -> r (a hd)"),
                        in_offset=bass.IndirectOffsetOnAxis(ap=idx[:, j:j + 1], axis=0))

        def gather1(src, gall):
            """l=1: one of KD1/VD1 into [P, 16, HD]."""
            for j in range(16):
                n, s = j // 8, j % 8
                if n == 1 and s < i_val:
                    sec = sec_starts[s]
                    nc.sync.dma_start(gall[:, j, :], src.ap()[sec:sec + P, :])
                else:
                    nc.gpsimd.indirect_dma_start(
                        out=gall[:, j, :], out_offset=None, in_=src.ap(),
                        in_offset=bass.IndirectOffsetOnAxis(ap=idx[:, j:j + 1], axis=0))

        def attn_phase1(l, av_in, wall, kg):
            """kg: callable js_slice -> [P, 4, HD] bf16 K rows."""
            avbf = ap.tile([P, HD], BF16, tag="avbf")
            nc.vector.tensor_copy(out=avbf[:], in_=av_in[:])
            scrb = ap.tile([P, 4, HD], BF16, tag="scrb")
            logits = ap.tile([P, 16, 8], BF16, tag="logits")
            avb = avbf[:, None, :].to_broadcast([P, 4, HD])
            for js in range(4):
                jsl = slice(js * 4, (js + 1) * 4)
                nc.vector.tensor_tensor(out=scrb[:], in0=kg(jsl),
                                        in1=avb, op=mybir.AluOpType.mult)
                with nc.allow_low_precision(reason="logits bf16; sigmoid smooths"):
                    nc.vector.tensor_reduce(
                        out=logits[:, jsl, :],
                        in_=scrb[:].rearrange("p a (h d) -> p (a h) d", d=DK),
                        axis=mybir.AxisListType.X, op=mybir.AluOpType.add)
            delta = ap.tile([P, 8, 8], BF16, tag="delta")
            nc.vector.tensor_tensor(out=delta[:], in0=logits[:, 0:8, :],
                                    in1=logits[:, 8:16, :],
                                    op=mybir.AluOpType.subtract)
            scale = DK ** (-0.5)
            dflat = delta[:].rearrange("p a b -> p (a b)")
            nc.scalar.activation(out=wall[:, 0:64], in_=dflat,
                                 func=mybir.ActivationFunctionType.Sigmoid,
                                 scale=scale)
            nc.scalar.activation(out=wall[:, 64:128], in_=dflat,
                                 func=mybir.ActivationFunctionType.Sigmoid,
                                 scale=-scale)

        def phase2_step(l, wall, vg, js, att, part, scr, half=None):
            # scr is [P, HD, 4] bf16: slot dim packed last so the reduce and
            # the accumulation run in the DVE fast (2x/4x) modes
            jsl = slice(js * 4, (js + 1) * 4)
            wv = wall[:].rearrange("p (j h) -> p j h", h=8)
            if half is None:
                hsl, nh = slice(0, HD), 8
            else:
                hsl, nh = slice(half * (HD // 2), (half + 1) * (HD // 2)), 4
            wvh = wv[:, jsl, half * 4:half * 4 + 4] if half is not None \
                else wv[:, jsl, :]
            nc.vector.tensor_tensor(
                out=scr[:, hsl, :].rearrange("p (h d) a -> p a h d", d=DK),
                in0=vg(jsl).rearrange("p a (h d) -> p a h d", d=DK),
                in1=wvh[:, :, :, None].to_broadcast([P, 4, nh, DK]),
                op=mybir.AluOpType.mult)
            dst = att if js == 0 else part
            with nc.allow_low_precision(reason="slot-sum in bf16"):
                nc.vector.tensor_reduce(
                    out=dst[:, hsl],
                    in_=scr[:, hsl, :],
                    axis=mybir.AxisListType.X, op=mybir.AluOpType.add)
            if js > 0:
                nc.vector.tensor_tensor(out=att[:, hsl], in0=att[:, hsl],
                                        in1=part[:, hsl],
                                        op=mybir.AluOpType.add)

        def attn_phase2_tail(l, avp, att):
            attf = ap.tile([P, HD], F32, tag="attf")
            nc.vector.tensor_tensor(out=attf[:], in0=att[:], in1=avp[:],
                                    op=mybir.AluOpType.add)
            xn = ap.tile([P, HD], F32, tag="xn")
            layer_norm(attf, BC_LN1G + l * HD, BC_LN1B + l * HD, xn)
            xT = ap.tile([P, 4, P], BF16, tag="xT")
            transpose_to(xn, xT)
            ff1 = ap.tile([P, 4, P], BF16, tag="ff1")
            for ft in range(4):
                psf = pqa.tile([P, P], F32, tag="pa")
                for kt in range(4):
                    nc.tensor.matmul(psf[:], ffw1[:, l, kt, ft * P:(ft + 1) * P],
                                     xT[:, kt, :], start=(kt == 0), stop=(kt == 3))
                nc.scalar.activation(
                    out=ff1[:, ft, :], in_=psf[:],
                    func=mybir.ActivationFunctionType.Relu,
                    bias=pbias[:, PB_FFB1 + 4 * l + ft:PB_FFB1 + 4 * l + ft + 1])
            ps2 = pqa.tile([P, HD], F32, tag="pa")
            for kt in range(4):
                nc.tensor.matmul(ps2[:], ff1[:, kt, :], ffw2[:, l, kt, :],
                                 start=(kt == 0), stop=(kt == 3))
            ffx = ap.tile([P, HD], F32, tag="ffx")
            nc.vector.tensor_tensor(out=ffx[:], in0=ps2[:],
                                    in1=bcast[:, BC_FFB2 + l * HD:BC_FFB2 + (l + 1) * HD],
                                    op=mybir.AluOpType.add)
            nc.vector.tensor_tensor(out=ffx[:], in0=ffx[:], in1=xn[:],
                                    op=mybir.AluOpType.add)
            av_out = ap.tile([P, HD], F32, tag="av")
            layer_norm(ffx, BC_LN2G + l * HD, BC_LN2B + l * HD, av_out)
            return av_out

        def make_avp(l, av_in):
            avp = ap.tile([P, HD], F32, tag="avp")
            nc.vector.tensor_tensor(
                out=avp[:], in0=av_in[:],
                in1=bcast[:, BC_B3V8 + l * HD:BC_B3V8 + (l + 1) * HD],
                op=mybir.AluOpType.add)
            return avp

        # ================= schedule =================
        stag = sp.tile([P, NRT, HD], BF16, tag="stag")
        av = None
        for h in range(H):
            k_chain(0 * 8 + h, h, stag)
            if h == 0:
                av = ds_project()
        nc.sync.dma_start(
            KV0D.ap()[:, 0, :].rearrange("(rt p) hd -> p rt hd", p=P), stag[:])

        stag = sp.tile([P, NRT, HD], BF16, tag="stag")
        for h in range(H):
            v_chain(0 * 8 + h, h, stag)
        nc.sync.dma_start(
            KV0D.ap()[:, 1, :].rearrange("(rt p) hd -> p rt hd", p=P), stag[:])
        kv0gall = gp0.tile([P, 16, 2, HD], BF16, tag="gall0")
        gather0(kv0gall)

        # l=1 K chains; l=0 attention interleaved (phase2 spread over chains)
        stag = sp.tile([P, NRT, HD], BF16, tag="stag")
        wall0 = ap.tile([P, 128], F32, tag="wall")
        att0 = ap.tile([P, HD], BF16, tag="att")
        part0 = ap.tile([P, HD], BF16, tag="part")
        scr0 = ap.tile([P, HD, 4], BF16, tag="scr")
        vg0 = lambda jsl: kv0gall[:, jsl, 1, :]
        avp0 = None
        for h in range(H):
            k_chain(1 * 8 + h, h, stag)
            if h == 1:
                attn_phase1(0, av, wall0, lambda jsl: kv0gall[:, jsl, 0, :])
                avp0 = make_avp(0, av)
            elif 2 <= h <= 5:
                phase2_step(0, wall0, vg0, h - 2, att0, part0, scr0)
        nc.sync.dma_start(
            KD1.ap().rearrange("(rt p) hd -> p rt hd", p=P), stag[:])
        k1gall = gp1.tile([P, 16, HD], BF16, tag="gall1")
        gather1(KD1, k1gall)
        av = attn_phase2_tail(0, avp0, att0)

        # l=1 V chains; l=1 phase1 interleaved. The stag is written to DRAM
        # in two head-halves so V gathers + weighted sums start while the
        # last four chains still compute.
        stag = sp.tile([P, NRT, HD], BF16, tag="stag")
        wall1 = ap.tile([P, 128], F32, tag="wall")
        att1 = ap.tile([P, HD], BF16, tag="att")
        part1 = ap.tile([P, HD], BF16, tag="part")
        scr1 = ap.tile([P, HD, 4], BF16, tag="scr")
        v1gall = gp1.tile([P, 16, HD], BF16, tag="gall1")
        avp1 = None

        def v1_half(half, src_t):
            hsl = slice(half * (HD // 2), (half + 1) * (HD // 2))
            nc.sync.dma_start(
                src_t.ap().rearrange("(rt p) hd -> p rt hd", p=P),
                stag[:, :, hsl])
            for js in range(4):
                for j in range(js * 4, js * 4 + 4):
                    n, s = j // 8, j % 8
                    if n == 1 and s < i_val:
                        nc.sync.dma_start(
                            v1gall[:, j, hsl],
                            src_t.ap()[sec_starts[s]:sec_starts[s] + P, :])
                    else:
                        nc.gpsimd.indirect_dma_start(
                            out=v1gall[:, j, hsl], out_offset=None,
                            in_=src_t.ap(),
                            in_offset=bass.IndirectOffsetOnAxis(
                                ap=idx[:, j:j + 1], axis=0))
                phase2_step(1, wall1, lambda jsl: v1gall[:, jsl, hsl], js,
                            att1, part1, scr1, half=half)

        for h in range(H):
            v_chain(1 * 8 + h, h, stag)
            if h == 1:
                attn_phase1(1, av, wall1, lambda jsl: k1gall[:, jsl, :])
                avp1 = make_avp(1, av)
            elif h == 4:
                v1_half(0, VD1A)
        v1_half(1, VD1B)
        av = attn_phase2_tail(1, avp1, att1)

        # ---- dist extractor ----
        avT = ap.tile([P, 4, P], BF16, tag="avT")
        transpose_to(av, avT)
        h1 = ap.tile([P, 2, P], BF16, tag="deh1")
        for ft in range(2):
            psd = pqa.tile([P, P], F32, tag="pa")
            for kt in range(4):
                nc.tensor.matmul(psd[:], dew1[:, kt, ft * P:(ft + 1) * P],
                                 avT[:, kt, :], start=(kt == 0), stop=(kt == 3))
            nc.scalar.activation(out=h1[:, ft, :], in_=psd[:],
                                 func=mybir.ActivationFunctionType.Relu,
                                 bias=pbias[:, PB_DEB1 + ft:PB_DEB1 + ft + 1])
        h2 = ap.tile([P, 2, P], BF16, tag="deh2")
        for ft in range(2):
            psd = pqa.tile([P, P], F32, tag="pa")
            for kt in range(2):
                nc.tensor.matmul(psd[:], dew2[:, kt, ft * P:(ft + 1) * P],
                                 h1[:, kt, :], start=(kt == 0), stop=(kt == 1))
            nc.scalar.activation(out=h2[:, ft, :], in_=psd[:],
                                 func=mybir.ActivationFunctionType.Relu,
                                 bias=pbias[:, PB_DEB2 + ft:PB_DEB2 + ft + 1])
        pso = pqa.tile([P, R], F32, tag="pa")
        for kt in range(2):
            nc.tensor.matmul(pso[:], h2[:, kt, :], dew3[:, kt, :],
                             start=(kt == 0), stop=(kt == 1))
        o = ap.tile([P, R], F32, tag="out")
        nc.vector.tensor_tensor(out=o[:], in0=pso[:],
                                in1=bcast[:, BC_DEB3:BC_DEB3 + R],
                                op=mybir.AluOpType.add)
        nc.sync.dma_start(OUT.ap()[:, 0:R // 2], o[:, 0:R // 2])
        nc.scalar.dma_start(OUT.ap()[:, R // 2:], o[:, R // 2:])


_BUILD_CACHE = {}


def _build(i_val, nrowc, sec_starts, has_b2=False):
    key = (i_val, nrowc, tuple(sec_starts), has_b2)
    if key in _BUILD_CACHE:
        return _BUILD_CACHE[key]
    nc = bacc.Bacc("TRN2", target_bir_lowering=False, debug=False)
    t = []
    t.append(nc.dram_tensor("XT8", [P, 2, nrowc], FP8, kind="ExternalInput"))
    t.append(nc.dram_tensor("XTB", [P, 2, nrowc], BF16, kind="ExternalInput"))
    t.append(nc.dram_tensor("XTUB", [4, nrowc], BF16, kind="ExternalInput"))
    t.append(nc.dram_tensor("PREDT", [P, 2, M], F32R, kind="ExternalInput"))
    t.append(nc.dram_tensor("DSW", [P, 2, HD], F32R, kind="ExternalInput"))
    t.append(nc.dram_tensor("W1K8", [NCH, P, 2, F], FP8, kind="ExternalInput"))
    t.append(nc.dram_tensor("W2K8", [NCH, P, 2, F], FP8, kind="ExternalInput"))
    t.append(nc.dram_tensor("W3K8", [NCH, P, 2, DK], FP8, kind="ExternalInput"))
    t.append(nc.dram_tensor("W1LB", [2 * NCH, 4, F], BF16, kind="ExternalInput"))
    t.append(nc.dram_tensor("W1V", [NCH, P, 2, F], BF16, kind="ExternalInput"))
    t.append(nc.dram_tensor("W2V", [NCH, P, 2, F], BF16, kind="ExternalInput"))
    t.append(nc.dram_tensor("W3V", [NCH, P, 2, DK], BF16, kind="ExternalInput"))
    t.append(nc.dram_tensor("FFW1", [L, P, 4, HD], BF16, kind="ExternalInput"))
    t.append(nc.dram_tensor("FFW2", [L, P, 4, HD], BF16, kind="ExternalInput"))
    t.append(nc.dram_tensor("DEW1", [P, 4, F], BF16, kind="ExternalInput"))
    t.append(nc.dram_tensor("DEW2", [P, 2, F], BF16, kind="ExternalInput"))
    t.append(nc.dram_tensor("DEW3", [P, 2, R], BF16, kind="ExternalInput"))
    t.append(nc.dram_tensor("PBIAS", [P, PB_COLS], F32, kind="ExternalInput"))
    t.append(nc.dram_tensor("BCAST", [P, BC_COLS], BF16, kind="ExternalInput"))
    t.append(nc.dram_tensor("IDX", [P, 16], I32, kind="ExternalInput"))
    t.append(nc.dram_tensor("OUT", [M, R], F32, kind="ExternalOutput"))
    t.append(nc.dram_tensor("KV0D", [nrowc, 2, HD], BF16))
    t.append(nc.dram_tensor("KD1", [nrowc, HD], BF16))
    t.append(nc.dram_tensor("VD1A", [nrowc, HD // 2], BF16))
    t.append(nc.dram_tensor("VD1B", [nrowc, HD // 2], BF16))
    with tile.TileContext(nc) as tc:
        _emit(nc, tc, t, i_val, nrowc, sec_starts, has_b2)
    nc.compile()
    _BUILD_CACHE[key] = nc
    return nc


def _fp8(x, scale):
    return np.clip(np.asarray(x, np.float32) * scale, -240.0, 240.0).astype(E4NP)


def plan_compaction(ins):
    i = int(ins["i"])
    left = np.asarray(ins["left_idx"], np.int64)
    right = np.asarray(ins["right_idx"], np.int64)
    m_ar = np.arange(M, dtype=np.int64)
    tsets = []
    for s in range(S):
        tset = np.union1d(left, m_ar) if s < i else np.union1d(left, right)
        tsets.append(np.sort(tset))
    sec_starts = []
    rows = []
    rowmap = {}
    for s in range(S):
        sec_starts.append(len(rows))
        for tt in tsets[s]:
            rowmap[(s, int(tt))] = len(rows)
            rows.append((s, int(tt)))
    nraw = len(rows)
    nrowc = ((nraw + P - 1) // P) * P
    idxm = np.zeros((P, 16), np.int32)
    for s in range(S):
        for mm in range(M):
            idxm[mm, 0 * 8 + s] = rowmap[(s, int(left[mm]))]
            t1 = mm if s < i else int(right[mm])
            idxm[mm, 1 * 8 + s] = rowmap[(s, t1)]
    return {
        "i": i, "rows": rows, "nrowc": nrowc,
        "sec_starts": tuple(sec_starts[:i]), "idx": idxm,
    }


def _prep_shared(ins, plan):
    f32 = np.float32
    kW1, kW2, kW3 = ins["kW1"], ins["kW2"], ins["kW3"]
    vW1, vW2, vW3 = ins["vW1"], ins["vW2"], ins["vW3"]
    kb1, kb2 = ins["kb1"], ins["kb2"]
    vb1, vb2, vb3 = ins["vb1"], ins["vb2"], ins["vb3"]
    if np.any(np.asarray(kb1, np.float32)) or np.any(np.asarray(vb1, np.float32)):
        raise NotImplementedError("nonzero kb1/vb1 not folded (u-row rank-1 "
                                  "carries no bias term)")

    W1K8 = np.empty((NCH, P, 2, F), E4NP)
    W2K8 = np.empty((NCH, P, 2, F), E4NP)
    W3K8 = np.empty((NCH, P, 2, DK), E4NP)
    W1V = np.empty((NCH, P, 2, F), BFNP)
    W2V = np.empty((NCH, P, 2, F), BFNP)
    W3V = np.empty((NCH, P, 2, DK), BFNP)
    W1LB = np.empty((2 * NCH, 4, F), BFNP)
    PB = np.zeros((P, PB_COLS), f32)
    BC = np.zeros((BC_COLS,), f32)

    def pack2(w, ncols):
        return np.asarray(w, f32).reshape(2, P, ncols).transpose(1, 0, 2)

    for l in range(L):
        for h in range(H):
            c = l * 8 + h
            W1K8[c] = _fp8(pack2(kW1[l, h][:256], F), W1S)
            W2K8[c] = _fp8(pack2(kW2[l, h], F), W2S)
            W3K8[c] = _fp8(pack2(kW3[l, h], DK), W3S)
            W1LB[c] = np.broadcast_to(
                (np.asarray(kW1[l, h][256], f32) * W1S).astype(BFNP), (4, F))
            W1V[c] = (pack2(vW1[l, h][:256], F) * 64.0).astype(BFNP)
            W2V[c] = pack2(vW2[l, h], F).astype(BFNP)
            W3V[c] = pack2(vW3[l, h], DK).astype(BFNP)
            W1LB[NCH + c] = np.broadcast_to(
                (np.asarray(vW1[l, h][256], f32) * W1S).astype(BFNP), (4, F))
            for gt in range(2):
                gsl = slice(gt * P, (gt + 1) * P)
                PB[:, PB_B2KP + 2 * c + gt] = B2KS * np.asarray(kb2[l, h][gsl], f32)
                PB[:, PB_B2KN + 2 * c + gt] = -B2KS * np.asarray(kb2[l, h][gsl], f32)
                PB[:, PB_B2VP + 2 * c + gt] = np.asarray(vb2[l, h][gsl], f32)
                PB[:, PB_B2VN + 2 * c + gt] = -np.asarray(vb2[l, h][gsl], f32)

    DSW = np.asarray(ins["ds_W"], f32).reshape(2, P, HD).transpose(1, 0, 2).copy()
    BC[BC_DSB:BC_DSB + HD] = np.asarray(ins["ds_b"], f32)
    for l in range(L):
        BC[BC_B3V8 + l * HD:BC_B3V8 + (l + 1) * HD] = \
            S * np.asarray(vb3[l], f32).reshape(HD)

    FFW1 = np.empty((L, P, 4, HD), BFNP)
    FFW2 = np.empty((L, P, 4, HD), BFNP)
    for l in range(L):
        FFW1[l] = np.asarray(ins["ffW1"][l], f32).reshape(4, P, HD).transpose(1, 0, 2).astype(BFNP)
        FFW2[l] = np.asarray(ins["ffW2"][l], f32).reshape(4, P, HD).transpose(1, 0, 2).astype(BFNP)
        for ft in range(4):
            PB[:, PB_FFB1 + 4 * l + ft] = np.asarray(ins["ffb1"][l][ft * P:(ft + 1) * P], f32)
        BC[BC_FFB2 + l * HD:BC_FFB2 + (l + 1) * HD] = np.asarray(ins["ffb2"][l], f32)
        BC[BC_LN1G + l * HD:BC_LN1G + (l + 1) * HD] = np.asarray(ins["ln1_g"][l], f32)
        BC[BC_LN1B + l * HD:BC_LN1B + (l + 1) * HD] = np.asarray(ins["ln1_b"][l], f32)
        BC[BC_LN2G + l * HD:BC_LN2G + (l + 1) * HD] = np.asarray(ins["ln2_g"][l], f32)
        BC[BC_LN2B + l * HD:BC_LN2B + (l + 1) * HD] = np.asarray(ins["ln2_b"][l], f32)

    DEW1 = np.asarray(ins["deW1"], f32).reshape(4, P, F).transpose(1, 0, 2).astype(BFNP)
    DEW2 = np.asarray(ins["deW2"], f32).reshape(2, P, F).transpose(1, 0, 2).astype(BFNP)
    DEW3 = np.asarray(ins["deW3"], f32).reshape(2, P, R).transpose(1, 0, 2).astype(BFNP)
    for ft in range(2):
        PB[:, PB_DEB1 + ft] = np.asarray(ins["deb1"][ft * P:(ft + 1) * P], f32)
        PB[:, PB_DEB2 + ft] = np.asarray(ins["deb2"][ft * P:(ft + 1) * P], f32)
    BC[BC_DEB3:BC_DEB3 + R] = np.asarray(ins["deb3"], f32)

    BCAST = np.broadcast_to(BC.astype(BFNP), (P, BC_COLS)).copy()

    return {
        "W1K8": W1K8, "W2K8": W2K8, "W3K8": W3K8, "W1LB": W1LB,
        "W1V": W1V, "W2V": W2V, "W3V": W3V,
        "DSW": DSW, "FFW1": FFW1, "FFW2": FFW2,
        "DEW1": DEW1, "DEW2": DEW2, "DEW3": DEW3,
        "PBIAS": PB, "BCAST": BCAST, "IDX": plan["idx"],
    }


def make_in_maps(ins, plan=None):
    if plan is None:
        plan = plan_compaction(ins)
    shared = _prep_shared(ins, plan)
    enc = np.asarray(ins["encoded"], np.float32)
    tu = np.asarray(ins["true_u"], np.float32)
    mid = np.asarray(ins["mid_idx"], np.int64)
    i = plan["i"]
    nrowc = plan["nrowc"]
    rows = plan["rows"]
    s_idx = np.array([r[0] for r in rows], np.int64)
    t_idx = np.array([r[1] for r in rows], np.int64)

    in_maps = []
    for b in range(B):
        xg = np.zeros((nrowc, D), np.float32)
        xg[:len(rows)] = enc[b][s_idx, t_idx]
        ug = np.zeros((nrowc,), np.float32)
        ug[:len(rows)] = tu[b][s_idx, t_idx]
        xs16 = xg.T.reshape(2, P, nrowc).transpose(1, 0, 2) * XS
        xub = np.broadcast_to((ug * XS).astype(BFNP), (4, nrowc))
        pred = enc[b, i][mid]
        predt = pred.T.reshape(2, P, M).transpose(1, 0, 2).copy()
        m = dict(shared)
        m["XT8"] = _fp8(xs16, 1.0)
        m["XTB"] = xs16.astype(BFNP)
        m["XTUB"] = np.ascontiguousarray(xub)
        m["PREDT"] = predt
        in_maps.append(m)
    return in_maps


def kernel(**inputs):
    ins = {k: np.asarray(v) for k, v in inputs.items()}
    plan = plan_compaction(ins)
    in_maps = make_in_maps(ins, plan)
    has_b2 = bool(np.any(np.asarray(ins["kb2"], np.float32))
                  or np.any(np.asarray(ins["vb2"], np.float32)))
    nc = _build(plan["i"], plan["nrowc"], plan["sec_starts"], has_b2)
    res = run_bass_kernel_spmd(nc, in_maps, core_ids=list(range(NCORES)))
    out = np.stack([res.results[c]["OUT"] for c in range(NCORES)])
    return out.astype(np.float32)


# revision 24
# speedup vs baseline: 1.0541x; 1.0100x over previous
"""Trainium2 Bass kernel for nn_AttentionalCopula (sparse_attention).

Sharding: data-parallel over batch (B=8 -> 8 cores); per-head K/V MLP stacks
computed locally per core. Weights replicated. No collectives.

Key optimizations over the v1 baseline:
  * Row compaction: the attention only gathers K/V rows from
    union_s({left} u ({arange} if s<i else {right})) -- ~1350 of 2048 rows.
    The MLP chains run only on those rows.
  * K chains in fp8 (float8e4) with MatmulPerfMode.DoubleRow: K_eff=256 per
    matmul at 0.5 cyc/col (4x bf16). V chains stay bf16 (V values feed the
    output directly; fp8 there fails the accuracy gate -- measured).
  * The u-row (feature 257) enters L1 psum via a K=1 bf16 rank-1 matmul
    (tile_position-paired across the two f-tiles).
  * Relu/copy work is round-robined across ACT/DVE/Pool engines.
  * l=0 K/V staged interleaved in DRAM so one indirect gather per slot
    fetches both; contiguous slots (n=1, s<i) use direct DMA.
"""

from contextlib import ExitStack

import ml_dtypes
import numpy as np

import concourse.bass as bass
import concourse.mybir as mybir
import concourse.tile as tile
from concourse import bacc
from concourse.bass_utils import run_bass_kernel_spmd
from concourse.masks import make_identity

B, S, T, D = 8, 8, 256, 256
H, DK = 8, 64
HD = H * DK            # 512
L = 2
F = 256
R = 512
M = 128
EPS = 1e-5
NCORES = 8
P = 128

F32 = mybir.dt.float32
F32R = mybir.dt.float32r
BF16 = mybir.dt.bfloat16
FP8 = mybir.dt.float8e4
I32 = mybir.dt.int32
E4NP = ml_dtypes.float8_e4m3
BFNP = ml_dtypes.bfloat16
DRMODE = mybir.MatmulPerfMode.DoubleRow

NCH = L * H            # 16 chains per kv side

# fp8 scale plan (K side):  X*16, W1*64 -> psum 1024*pre1 ; h1'=psum/64 (16*h1)
# W2*2 -> psum2 32*pre2 ; h2' = max(psum2 + 32*b2, 0) (32*h2)
# W3*64 -> psum3 2048*kv ; stag = psum3/2048
XS = 16.0
W1S = 64.0
H1DIV = 64.0
W2S = 2.0
B2KS = 32.0
W3S = 64.0
L3KDIV = 2048.0
# V side: X*16 (bf16), W1*64 (bf16) -> psum 1024*pre1 ; h1 = psum/1024
V1DIV = 1024.0

# PBIAS layout (per-partition f32 scalars)
PB_FFB1 = 0                      # 4 per l
PB_DEB1 = PB_FFB1 + 4 * L
PB_DEB2 = PB_DEB1 + 2
PB_B2KP = PB_DEB2 + 2            # +32*b2k per (c, gt)
PB_B2KN = PB_B2KP + 2 * NCH      # -32*b2k
PB_B2VP = PB_B2KN + 2 * NCH      # +b2v
PB_B2VN = PB_B2VP + 2 * NCH      # -b2v
PB_COLS = PB_B2VN + 2 * NCH

# BCAST layout (free-dim vectors, bf16, replicated across partitions)
BC_DSB = 0
BC_B3V8 = BC_DSB + HD            # S * vb3 per l
BC_FFB2 = BC_B3V8 + L * HD
BC_DEB3 = BC_FFB2 + L * HD
BC_LN1G = BC_DEB3 + R
BC_LN1B = BC_LN1G + L * HD
BC_LN2G = BC_LN1B + L * HD
BC_LN2B = BC_LN2G + L * HD
BC_COLS = BC_LN2B + L * HD


def _chunks(nrowc):
    out = []
    col = 0
    while col < nrowc:
        w = min(512, nrowc - col)
        out.append((col, w))
        col += w
    return out


class _RR:
    def __init__(self, pattern):
        self.pattern = pattern
        self.i = 0

    def next(self):
        e = self.pattern[self.i % len(self.pattern)]
        self.i += 1
        return e


def _emit(nc, tc, t, i_val, nrowc, sec_starts, has_b2):
    (XT8, XTB, XTUB, PREDT, DSW, W1K8, W2K8, W3K8, W1LB,
     W1V, W2V, W3V, FFW1, FFW2, DEW1, DEW2, DEW3,
     PBIAS, BCAST, IDX, OUT, KD0, VD0, KD1, VD1A, VD1B) = t

    NRT = nrowc // P
    chunks = _chunks(nrowc)
    relu_rr = _RR("AAD")      # Pool cannot read PSUM on TRN2
    copy_rr = _RR("AD")

    with ExitStack() as ctx:
        cp = ctx.enter_context(tc.tile_pool(name="const", bufs=1))
        wp = ctx.enter_context(tc.tile_pool(name="w", bufs=3))
        sp = ctx.enter_context(tc.tile_pool(name="stag", bufs=2))
        hp = ctx.enter_context(tc.tile_pool(name="h", bufs=2))
        gp0 = ctx.enter_context(tc.tile_pool(name="gath0", bufs=2))
        gp1 = ctx.enter_context(tc.tile_pool(name="gath1", bufs=2))
        ap = ctx.enter_context(tc.tile_pool(name="attn", bufs=1))
        pp = ctx.enter_context(tc.tile_pool(name="ps", bufs=5, space="PSUM"))
        pq3 = ctx.enter_context(tc.tile_pool(name="ps3", bufs=2, space="PSUM"))
        pqa = ctx.enter_context(tc.tile_pool(name="pa", bufs=1, space="PSUM"))

        # ---- resident loads; chain-0 critical path first ----
        xtub = cp.tile([97, nrowc], BF16)
        nc.scalar.dma_start(xtub[0:97:32, 0:512], XTUB.ap()[:, 0:512])
        nc.scalar.dma_start(xtub[0:97:32, 512:], XTUB.ap()[:, 512:])
        xt8 = cp.tile([P, 2, nrowc], FP8)
        nc.sync.dma_start(xt8[:, :, 0:256], XT8.ap()[:, :, 0:256])
        nc.scalar.dma_start(xt8[:, :, 256:512], XT8.ap()[:, :, 256:512])
        pbias = cp.tile([P, PB_COLS], F32)
        nc.gpsimd.dma_start(pbias[:], PBIAS.ap())
        for col, w in chunks:
            if col == 0:
                continue
            nc.scalar.dma_start(xt8[:, :, col:col + w], XT8.ap()[:, :, col:col + w])
        idx = cp.tile([P, 16], I32)
        nc.gpsimd.dma_start(idx[:], IDX.ap())
        xtb = cp.tile([P, 2, nrowc], BF16)
        for col, w in chunks:
            nc.scalar.dma_start(xtb[:, :, col:col + w], XTB.ap()[:, :, col:col + w])
        predt = cp.tile([P, 2, M], F32R)
        nc.gpsimd.dma_start(predt[:], PREDT.ap())
        dsw = cp.tile([P, 2, HD], F32R)
        nc.gpsimd.dma_start(dsw[:], DSW.ap())
        bcast = cp.tile([P, BC_COLS], BF16)
        nc.gpsimd.dma_start(bcast[:], BCAST.ap())
        ffw1 = cp.tile([P, L, 4, HD], BF16)
        nc.gpsimd.dma_start(ffw1[:], FFW1.ap().rearrange("l p a h -> p l a h"))
        ffw2 = cp.tile([P, L, 4, HD], BF16)
        nc.gpsimd.dma_start(ffw2[:], FFW2.ap().rearrange("l p a h -> p l a h"))
        dew1 = cp.tile([P, 4, F], BF16)
        nc.gpsimd.dma_start(dew1[:], DEW1.ap())
        dew2 = cp.tile([P, 2, F], BF16)
        nc.gpsimd.dma_start(dew2[:], DEW2.ap())
        dew3 = cp.tile([P, 2, R], BF16)
        nc.gpsimd.dma_start(dew3[:], DEW3.ap())
        ident = cp.tile([P, P], F32)
        make_identity(nc, ident[:])
        epst = cp.tile([P, 1], F32)
        nc.vector.memset(epst[:], EPS)

        def relu_out(ps_ap, out_ap, scale=None, bias=None):
            e = relu_rr.next()
            if e == "A":
                if bias is None:
                    nc.scalar.activation(out=out_ap, in_=ps_ap,
                                         func=mybir.ActivationFunctionType.Relu,
                                         scale=scale)
                else:
                    nc.scalar.activation(out=out_ap, in_=ps_ap,
                                         func=mybir.ActivationFunctionType.Relu,
                                         bias=pbias[:, bias[0]:bias[0] + 1])
            else:
                eng = nc.vector if e == "D" else nc.gpsimd
                if bias is None:
                    eng.tensor_scalar(out=out_ap, in0=ps_ap,
                                      scalar1=float(scale), scalar2=0.0,
                                      op0=mybir.AluOpType.mult,
                                      op1=mybir.AluOpType.max)
                else:
                    eng.tensor_scalar(out=out_ap, in0=ps_ap,
                                      scalar1=pbias[:, bias[1]:bias[1] + 1],
                                      scalar2=pbias[:, bias[0]:bias[0] + 1],
                                      op0=mybir.AluOpType.max,
                                      op1=mybir.AluOpType.add)

        def copy_out(ps_ap, out_ap, scale):
            e = copy_rr.next()
            if e == "A":
                nc.scalar.activation(out=out_ap, in_=ps_ap,
                                     func=mybir.ActivationFunctionType.Copy,
                                     scale=scale)
            else:
                eng = nc.vector if e == "D" else nc.gpsimd
                eng.tensor_scalar(out=out_ap, in0=ps_ap,
                                  scalar1=float(scale), scalar2=0.0,
                                  op0=mybir.AluOpType.mult,
                                  op1=mybir.AluOpType.add)

        def k_chain(c, h, stag):
            """fp8 DoubleRow chain, software-pipelined:
            L1(ci) -> L2(ci-1) -> L3(ci-2) so relus hide under matmuls."""
            w1l = wp.tile([97, F], BF16, tag="w1lk")
            nc.sync.dma_start(w1l[0:97:32, :], W1LB.ap()[c])
            w1 = wp.tile([P, 2, F], FP8, tag="w1k")
            nc.sync.dma_start(w1[:], W1K8.ap()[c])
            w2 = wp.tile([P, 2, F], FP8, tag="w2k")
            nc.sync.dma_start(w2[:], W2K8.ap()[c])
            w3 = wp.tile([P, 2, DK], FP8, tag="w3k")
            nc.sync.dma_start(w3[:], W3K8.ap()[c])

            st = {}

            def l1(ci):
                col, cw = chunks[ci]
                h1t = hp.tile([P, 2, 512], FP8, tag="h1k")
                # rank-1 u-terms OPEN the full psum region (a start=False
                # accumulate spanning two DR-opened regions corrupts psum, so
                # open wide first, then let the DR mains accumulate densely)
                ps = []
                for ft in range(2):
                    fsl = slice(ft * P, (ft + 1) * P)
                    rg = 32 * ft
                    ps1 = pp.tile([P, 512], F32, tag="ps")
                    ps.append(ps1)
                    nc.tensor.matmul(ps1[:, 0:cw], w1l[rg:rg + 1, fsl],
                                     xtub[rg:rg + 1, col:col + cw],
                                     start=True, stop=False, tile_position=(rg, 0))
                for ft in range(2):
                    fsl = slice(ft * P, (ft + 1) * P)
                    for cc in range(0, cw, 256):
                        ccw = min(256, cw - cc)
                        csl = slice(col + cc, col + cc + ccw)
                        nc.tensor.matmul(ps[ft][:, cc:cc + ccw], w1[:, :, fsl],
                                         xt8[:, :, csl], start=False,
                                         stop=(cc + 256 >= cw),
                                         perf_mode=DRMODE, skip_group_check=True)
                for ft in range(2):
                    relu_out(ps[ft][:, 0:cw], h1t[:, ft, 0:cw], scale=1.0 / H1DIV)
                st[ci] = [h1t, None]

            def l2(ci):
                col, cw = chunks[ci]
                h1t = st[ci][0]
                h2t = hp.tile([P, 2, 512], FP8, tag="h2k")
                ps = []
                for gt in range(2):
                    gsl = slice(gt * P, (gt + 1) * P)
                    ps2 = pp.tile([P, 512], F32, tag="ps")
                    ps.append(ps2)
                    for cc in range(0, cw, 256):
                        ccw = min(256, cw - cc)
                        nc.tensor.matmul(ps2[:, cc:cc + ccw], w2[:, :, gsl],
                                         h1t[:, :, cc:cc + ccw], start=True,
                                         stop=True, perf_mode=DRMODE)
                for gt in range(2):
                    relu_out(ps[gt][:, 0:cw], h2t[:, gt, 0:cw], scale=1.0,
                             bias=((PB_B2KP + 2 * c + gt, PB_B2KN + 2 * c + gt)
                                   if has_b2 else None))
                st[ci][1] = h2t

            def l3(ci):
                col, cw = chunks[ci]
                nrti = cw // P
                h2t = st[ci][1]
                ps3 = pq3.tile([P, 256], F32, tag="ps3")
                for rti in range(nrti):
                    rsl = slice(rti * P, (rti + 1) * P)
                    nc.tensor.matmul(ps3[:, rti * DK:(rti + 1) * DK],
                                     h2t[:, :, rsl], w3[:, :, :],
                                     start=True, stop=True, perf_mode=DRMODE)
                copy_out(ps3[:, 0:nrti * DK].rearrange("p (r d) -> p r d", d=DK),
                         stag[:, col // P:col // P + nrti, h * DK:(h + 1) * DK],
                         1.0 / L3KDIV)

            _pipeline(l1, l2, l3)

        def _pipeline(l1, l2, l3):
            n = len(chunks)
            for ci in range(n):
                l1(ci)
                if ci >= 1:
                    l2(ci - 1)
                if ci >= 2:
                    l3(ci - 2)
            l2(n - 1)
            if n >= 2:
                l3(n - 2)
            l3(n - 1)

        def v_chain(c, h, stag):
            w1 = wp.tile([P, 2, F], BF16, tag="w1v")
            nc.scalar.dma_start(w1[:], W1V.ap()[c])
            w2 = wp.tile([P, 2, F], BF16, tag="w2v")
            nc.scalar.dma_start(w2[:], W2V.ap()[c])
            w3 = wp.tile([P, 2, DK], BF16, tag="w3v")
            nc.scalar.dma_start(w3[:], W3V.ap()[c])
            w1l = wp.tile([97, F], BF16, tag="w1lv")
            nc.scalar.dma_start(w1l[0:97:32, :], W1LB.ap()[NCH + c])

            st = {}

            def l1(ci):
                col, cw = chunks[ci]
                h1t = hp.tile([P, 2, 512], BF16, tag="h1v")
                ps = []
                for ft in range(2):
                    fsl = slice(ft * P, (ft + 1) * P)
                    rg = 32 * ft
                    ps1 = pp.tile([P, 512], F32, tag="ps")
                    ps.append(ps1)
                    nc.tensor.matmul(ps1[:, 0:cw], w1l[rg:rg + 1, fsl],
                                     xtub[rg:rg + 1, col:col + cw],
                                     start=True, stop=False, tile_position=(rg, 0))
                for ft in range(2):
                    fsl = slice(ft * P, (ft + 1) * P)
                    nc.tensor.matmul(ps[ft][:, 0:cw], w1[:, 0, fsl],
                                     xtb[:, 0, col:col + cw], start=False, stop=False,
                                     skip_group_check=True)
                    nc.tensor.matmul(ps[ft][:, 0:cw], w1[:, 1, fsl],
                                     xtb[:, 1, col:col + cw], start=False, stop=True,
                                     skip_group_check=True)
                for ft in range(2):
                    relu_out(ps[ft][:, 0:cw], h1t[:, ft, 0:cw], scale=1.0 / V1DIV)
                st[ci] = [h1t, None]

            def l2(ci):
                col, cw = chunks[ci]
                h1t = st[ci][0]
                h2t = hp.tile([P, 2, 512], BF16, tag="h2v")
                ps = []
                for gt in range(2):
                    gsl = slice(gt * P, (gt + 1) * P)
                    ps2 = pp.tile([P, 512], F32, tag="ps")
                    ps.append(ps2)
                    nc.tensor.matmul(ps2[:, 0:cw], w2[:, 0, gsl],
                                     h1t[:, 0, 0:cw], start=True, stop=False)
                    nc.tensor.matmul(ps2[:, 0:cw], w2[:, 1, gsl],
                                     h1t[:, 1, 0:cw], start=False, stop=True)
                for gt in range(2):
                    relu_out(ps[gt][:, 0:cw], h2t[:, gt, 0:cw], scale=1.0,
                             bias=((PB_B2VP + 2 * c + gt, PB_B2VN + 2 * c + gt)
                                   if has_b2 else None))
                st[ci][1] = h2t

            def l3(ci):
                col, cw = chunks[ci]
                nrti = cw // P
                h2t = st[ci][1]
                ps3 = pq3.tile([P, 256], F32, tag="ps3")
                for rti in range(nrti):
                    rsl = slice(rti * P, (rti + 1) * P)
                    nc.tensor.matmul(ps3[:, rti * DK:(rti + 1) * DK],
                                     h2t[:, 0, rsl], w3[:, 0, :],
                                     start=True, stop=False)
                    nc.tensor.matmul(ps3[:, rti * DK:(rti + 1) * DK],
                                     h2t[:, 1, rsl], w3[:, 1, :],
                                     start=False, stop=True)
                copy_out(ps3[:, 0:nrti * DK].rearrange("p (r d) -> p r d", d=DK),
                         stag[:, col // P:col // P + nrti, h * DK:(h + 1) * DK],
                         1.0)

            _pipeline(l1, l2, l3)

        def ds_project():
            ps = pqa.tile([P, HD], F32, tag="pa")
            nc.tensor.matmul(ps[:], predt[:, 0, :], dsw[:, 0, :], start=True, stop=False)
            nc.tensor.matmul(ps[:], predt[:, 1, :], dsw[:, 1, :], start=False, stop=True)
            av0 = ap.tile([P, HD], F32, tag="av")
            nc.vector.tensor_tensor(out=av0[:], in0=ps[:],
                                    in1=bcast[:, BC_DSB:BC_DSB + HD],
                                    op=mybir.AluOpType.add)
            return av0

        def layer_norm(src, gcol, bcol, out):
            stats = ap.tile([P, 6], F32, tag="lnstat")
            nc.vector.bn_stats(stats[:], src[:])
            mv = ap.tile([P, 2], F32, tag="lnmv")
            nc.vector.bn_aggr(mv[:], stats[:])
            rstd = ap.tile([P, 1], F32, tag="lnrstd")
            nc.scalar.activation(out=rstd[:], in_=mv[:, 1:2],
                                 func=mybir.ActivationFunctionType.Sqrt,
                                 bias=epst[:, 0:1])
            nc.vector.reciprocal(rstd[:], rstd[:])
            nc.vector.tensor_scalar(out=out[:], in0=src[:], scalar1=mv[:, 0:1],
                                    scalar2=rstd[:, 0:1],
                                    op0=mybir.AluOpType.subtract,
                                    op1=mybir.AluOpType.mult)
            nc.vector.tensor_tensor(out=out[:], in0=out[:],
                                    in1=bcast[:, gcol:gcol + HD],
                                    op=mybir.AluOpType.mult)
            nc.vector.tensor_tensor(out=out[:], in0=out[:],
                                    in1=bcast[:, bcol:bcol + HD],
                                    op=mybir.AluOpType.add)

        def transpose_to(src, dst):
            for kt in range(4):
                tp_ = pqa.tile([P, P], F32, tag="pa")
                nc.tensor.transpose(tp_[:], src[:, kt * P:(kt + 1) * P], ident[:])
                nc.scalar.activation(out=dst[:, kt, :], in_=tp_[:],
                                     func=mybir.ActivationFunctionType.Copy)

        def gather1(src, gall):
            """l=1: one of KD1/VD1 into [P, 16, HD]."""
            for j in range(16):
                n, s = j // 8, j % 8
                if n == 1 and s < i_val:
                    sec = sec_starts[s]
                    nc.sync.dma_start(gall[:, j, :], src.ap()[sec:sec + P, :])
                else:
                    nc.gpsimd.indirect_dma_start(
                        out=gall[:, j, :], out_offset=None, in_=src.ap(),
                        in_offset=bass.IndirectOffsetOnAxis(ap=idx[:, j:j + 1], axis=0))

        def attn_phase1(l, av_in, wall, kg):
            """kg: callable js_slice -> [P, 4, HD] bf16 K rows."""
            avbf = ap.tile([P, HD], BF16, tag="avbf")
            nc.vector.tensor_copy(out=avbf[:], in_=av_in[:])
            scrb = ap.tile([P, 4, HD], BF16, tag="scrb")
            logits = ap.tile([P, 16, 8], BF16, tag="logits")
            avb = avbf[:, None, :].to_broadcast([P, 4, HD])
            for js in range(4):
                jsl = slice(js * 4, (js + 1) * 4)
                nc.vector.tensor_tensor(out=scrb[:], in0=kg(jsl),
                                        in1=avb, op=mybir.AluOpType.mult)
                with nc.allow_low_precision(reason="logits bf16; sigmoid smooths"):
                    nc.vector.tensor_reduce(
                        out=logits[:, jsl, :],
                        in_=scrb[:].rearrange("p a (h d) -> p (a h) d", d=DK),
                        axis=mybir.AxisListType.X, op=mybir.AluOpType.add)
            delta = ap.tile([P, 8, 8], BF16, tag="delta")
            nc.vector.tensor_tensor(out=delta[:], in0=logits[:, 0:8, :],
                                    in1=logits[:, 8:16, :],
                                    op=mybir.AluOpType.subtract)
            scale = DK ** (-0.5)
            dflat = delta[:].rearrange("p a b -> p (a b)")
            nc.scalar.activation(out=wall[:, 0:64], in_=dflat,
                                 func=mybir.ActivationFunctionType.Sigmoid,
                                 scale=scale)
            nc.scalar.activation(out=wall[:, 64:128], in_=dflat,
                                 func=mybir.ActivationFunctionType.Sigmoid,
                                 scale=-scale)

        def phase2_step(l, wall, vg, js, att, part, scr, half=None):
            # scr is [P, HD, 4] bf16: slot dim packed last so the reduce and
            # the accumulation run in the DVE fast (2x/4x) modes
            jsl = slice(js * 4, (js + 1) * 4)
            wv = wall[:].rearrange("p (j h) -> p j h", h=8)
            if half is None:
                hsl, nh = slice(0, HD), 8
            else:
                hsl, nh = slice(half * (HD // 2), (half + 1) * (HD // 2)), 4
            wvh = wv[:, jsl, half * 4:half * 4 + 4] if half is not None \
                else wv[:, jsl, :]
            nc.vector.tensor_tensor(
                out=scr[:, hsl, :].rearrange("p (h d) a -> p a h d", d=DK),
                in0=vg(jsl).rearrange("p a (h d) -> p a h d", d=DK),
                in1=wvh[:, :, :, None].to_broadcast([P, 4, nh, DK]),
                op=mybir.AluOpType.mult)
            dst = att if js == 0 else part
            with nc.allow_low_precision(reason="slot-sum in bf16"):
                nc.vector.tensor_reduce(
                    out=dst[:, hsl],
                    in_=scr[:, hsl, :],
                    axis=mybir.AxisListType.X, op=mybir.AluOpType.add)
            if js > 0:
                nc.vector.tensor_tensor(out=att[:, hsl], in0=att[:, hsl],
                                        in1=part[:, hsl],
                                        op=mybir.AluOpType.add)

        def attn_phase2_tail(l, avp, att):
            attf = ap.tile([P, HD], F32, tag="attf")
            nc.vector.tensor_tensor(out=attf[:], in0=att[:], in1=avp[:],
                                    op=mybir.AluOpType.add)
            xn = ap.tile([P, HD], F32, tag="xn")
            layer_norm(attf, BC_LN1G + l * HD, BC_LN1B + l * HD, xn)
            xT = ap.tile([P, 4, P], BF16, tag="xT")
            transpose_to(xn, xT)
            ff1 = ap.tile([P, 4, P], BF16, tag="ff1")
            for ft in range(4):
                psf = pqa.tile([P, P], F32, tag="pa")
                for kt in range(4):
                    nc.tensor.matmul(psf[:], ffw1[:, l, kt, ft * P:(ft + 1) * P],
                                     xT[:, kt, :], start=(kt == 0), stop=(kt == 3))
                nc.scalar.activation(
                    out=ff1[:, ft, :], in_=psf[:],
                    func=mybir.ActivationFunctionType.Relu,
                    bias=pbias[:, PB_FFB1 + 4 * l + ft:PB_FFB1 + 4 * l + ft + 1])
            ps2 = pqa.tile([P, HD], F32, tag="pa")
            for kt in range(4):
                nc.tensor.matmul(ps2[:], ff1[:, kt, :], ffw2[:, l, kt, :],
                                 start=(kt == 0), stop=(kt == 3))
            ffx = ap.tile([P, HD], F32, tag="ffx")
            nc.vector.tensor_tensor(out=ffx[:], in0=ps2[:],
                                    in1=bcast[:, BC_FFB2 + l * HD:BC_FFB2 + (l + 1) * HD],
                                    op=mybir.AluOpType.add)
            nc.vector.tensor_tensor(out=ffx[:], in0=ffx[:], in1=xn[:],
                                    op=mybir.AluOpType.add)
            av_out = ap.tile([P, HD], F32, tag="av")
            layer_norm(ffx, BC_LN2G + l * HD, BC_LN2B + l * HD, av_out)
            return av_out

        def make_avp(l, av_in):
            avp = ap.tile([P, HD], F32, tag="avp")
            nc.vector.tensor_tensor(
                out=avp[:], in0=av_in[:],
                in1=bcast[:, BC_B3V8 + l * HD:BC_B3V8 + (l + 1) * HD],
                op=mybir.AluOpType.add)
            return avp

        # ================= schedule =================
        stag = sp.tile([P, NRT, HD], BF16, tag="stag")
        av = None
        for h in range(H):
            k_chain(0 * 8 + h, h, stag)
            if h == 0:
                av = ds_project()
        nc.sync.dma_start(
            KD0.ap().rearrange("(rt p) hd -> p rt hd", p=P), stag[:])
        k0gall = gp0.tile([P, 16, HD], BF16, tag="gall0")
        gather1(KD0, k0gall)

        stag = sp.tile([P, NRT, HD], BF16, tag="stag")
        for h in range(H):
            v_chain(0 * 8 + h, h, stag)
        nc.sync.dma_start(
            VD0.ap().rearrange("(rt p) hd -> p rt hd", p=P), stag[:])
        v0gall = gp0.tile([P, 16, HD], BF16, tag="gall0")
        gather1(VD0, v0gall)

        # l=1 K chains; l=0 attention interleaved (phase2 spread over chains)
        stag = sp.tile([P, NRT, HD], BF16, tag="stag")
        wall0 = ap.tile([P, 128], F32, tag="wall")
        att0 = ap.tile([P, HD], BF16, tag="att")
        part0 = ap.tile([P, HD], BF16, tag="part")
        scr0 = ap.tile([P, HD, 4], BF16, tag="scr")
        vg0 = lambda jsl: v0gall[:, jsl, :]
        avp0 = None
        for h in range(H):
            k_chain(1 * 8 + h, h, stag)
            if h == 1:
                attn_phase1(0, av, wall0, lambda jsl: k0gall[:, jsl, :])
                avp0 = make_avp(0, av)
            elif 2 <= h <= 5:
                phase2_step(0, wall0, vg0, h - 2, att0, part0, scr0)
        nc.sync.dma_start(
            KD1.ap().rearrange("(rt p) hd -> p rt hd", p=P), stag[:])
        k1gall = gp1.tile([P, 16, HD], BF16, tag="gall1")
        gather1(KD1, k1gall)
        av = attn_phase2_tail(0, avp0, att0)

        # l=1 V chains; l=1 phase1 interleaved. The stag is written to DRAM
        # in two head-halves so V gathers + weighted sums start while the
        # last four chains still compute.
        stag = sp.tile([P, NRT, HD], BF16, tag="stag")
        wall1 = ap.tile([P, 128], F32, tag="wall")
        att1 = ap.tile([P, HD], BF16, tag="att")
        part1 = ap.tile([P, HD], BF16, tag="part")
        scr1 = ap.tile([P, HD, 4], BF16, tag="scr")
        v1gall = gp1.tile([P, 16, HD], BF16, tag="gall1")
        avp1 = None

        def v1_half(half, src_t):
            hsl = slice(half * (HD // 2), (half + 1) * (HD // 2))
            nc.sync.dma_start(
                src_t.ap().rearrange("(rt p) hd -> p rt hd", p=P),
                stag[:, :, hsl])
            for js in range(4):
                for j in range(js * 4, js * 4 + 4):
                    n, s = j // 8, j % 8
                    if n == 1 and s < i_val:
                        nc.sync.dma_start(
                            v1gall[:, j, hsl],
                            src_t.ap()[sec_starts[s]:sec_starts[s] + P, :])
                    else:
                        nc.gpsimd.indirect_dma_start(
                            out=v1gall[:, j, hsl], out_offset=None,
                            in_=src_t.ap(),
                            in_offset=bass.IndirectOffsetOnAxis(
                                ap=idx[:, j:j + 1], axis=0))
                phase2_step(1, wall1, lambda jsl: v1gall[:, jsl, hsl], js,
                            att1, part1, scr1, half=half)

        for h in range(H):
            v_chain(1 * 8 + h, h, stag)
            if h == 1:
                attn_phase1(1, av, wall1, lambda jsl: k1gall[:, jsl, :])
                avp1 = make_avp(1, av)
            elif h == 4:
                v1_half(0, VD1A)
        v1_half(1, VD1B)
        av = attn_phase2_tail(1, avp1, att1)

        # ---- dist extractor ----
        avT = ap.tile([P, 4, P], BF16, tag="avT")
        transpose_to(av, avT)
        h1 = ap.tile([P, 2, P], BF16, tag="deh1")
        for ft in range(2):
            psd = pqa.tile([P, P], F32, tag="pa")
            for kt in range(4):
                nc.tensor.matmul(psd[:], dew1[:, kt, ft * P:(ft + 1) * P],
                                 avT[:, kt, :], start=(kt == 0), stop=(kt == 3))
            nc.scalar.activation(out=h1[:, ft, :], in_=psd[:],
                                 func=mybir.ActivationFunctionType.Relu,
                                 bias=pbias[:, PB_DEB1 + ft:PB_DEB1 + ft + 1])
        h2 = ap.tile([P, 2, P], BF16, tag="deh2")
        for ft in range(2):
            psd = pqa.tile([P, P], F32, tag="pa")
            for kt in range(2):
                nc.tensor.matmul(psd[:], dew2[:, kt, ft * P:(ft + 1) * P],
                                 h1[:, kt, :], start=(kt == 0), stop=(kt == 1))
            nc.scalar.activation(out=h2[:, ft, :], in_=psd[:],
                                 func=mybir.ActivationFunctionType.Relu,
                                 bias=pbias[:, PB_DEB2 + ft:PB_DEB2 + ft + 1])
        pso = pqa.tile([P, R], F32, tag="pa")
        for kt in range(2):
            nc.tensor.matmul(pso[:], h2[:, kt, :], dew3[:, kt, :],
                             start=(kt == 0), stop=(kt == 1))
        o = ap.tile([P, R], F32, tag="out")
        nc.vector.tensor_tensor(out=o[:], in0=pso[:],
                                in1=bcast[:, BC_DEB3:BC_DEB3 + R],
                                op=mybir.AluOpType.add)
        nc.sync.dma_start(OUT.ap()[:, 0:R // 2], o[:, 0:R // 2])
        nc.scalar.dma_start(OUT.ap()[:, R // 2:], o[:, R // 2:])


_BUILD_CACHE = {}


def _build(i_val, nrowc, sec_starts, has_b2=False):
    key = (i_val, nrowc, tuple(sec_starts), has_b2)
    if key in _BUILD_CACHE:
        return _BUILD_CACHE[key]
    nc = bacc.Bacc("TRN2", target_bir_lowering=False, debug=False)
    t = []
    t.append(nc.dram_tensor("XT8", [P, 2, nrowc], FP8, kind="ExternalInput"))
    t.append(nc.dram_tensor("XTB", [P, 2, nrowc], BF16, kind="ExternalInput"))
    t.append(nc.dram_tensor("XTUB", [4, nrowc], BF16, kind="ExternalInput"))
    t.append(nc.dram_tensor("PREDT", [P, 2, M], F32R, kind="ExternalInput"))
    t.append(nc.dram_tensor("DSW", [P, 2, HD], F32R, kind="ExternalInput"))
    t.append(nc.dram_tensor("W1K8", [NCH, P, 2, F], FP8, kind="ExternalInput"))
    t.append(nc.dram_tensor("W2K8", [NCH, P, 2, F], FP8, kind="ExternalInput"))
    t.append(nc.dram_tensor("W3K8", [NCH, P, 2, DK], FP8, kind="ExternalInput"))
    t.append(nc.dram_tensor("W1LB", [2 * NCH, 4, F], BF16, kind="ExternalInput"))
    t.append(nc.dram_tensor("W1V", [NCH, P, 2, F], BF16, kind="ExternalInput"))
    t.append(nc.dram_tensor("W2V", [NCH, P, 2, F], BF16, kind="ExternalInput"))
    t.append(nc.dram_tensor("W3V", [NCH, P, 2, DK], BF16, kind="ExternalInput"))
    t.append(nc.dram_tensor("FFW1", [L, P, 4, HD], BF16, kind="ExternalInput"))
    t.append(nc.dram_tensor("FFW2", [L, P, 4, HD], BF16, kind="ExternalInput"))
    t.append(nc.dram_tensor("DEW1", [P, 4, F], BF16, kind="ExternalInput"))
    t.append(nc.dram_tensor("DEW2", [P, 2, F], BF16, kind="ExternalInput"))
    t.append(nc.dram_tensor("DEW3", [P, 2, R], BF16, kind="ExternalInput"))
    t.append(nc.dram_tensor("PBIAS", [P, PB_COLS], F32, kind="ExternalInput"))
    t.append(nc.dram_tensor("BCAST", [P, BC_COLS], BF16, kind="ExternalInput"))
    t.append(nc.dram_tensor("IDX", [P, 16], I32, kind="ExternalInput"))
    t.append(nc.dram_tensor("OUT", [M, R], F32, kind="ExternalOutput"))
    t.append(nc.dram_tensor("KD0", [nrowc, HD], BF16))
    t.append(nc.dram_tensor("VD0", [nrowc, HD], BF16))
    t.append(nc.dram_tensor("KD1", [nrowc, HD], BF16))
    t.append(nc.dram_tensor("VD1A", [nrowc, HD // 2], BF16))
    t.append(nc.dram_tensor("VD1B", [nrowc, HD // 2], BF16))
    with tile.TileContext(nc) as tc:
        _emit(nc, tc, t, i_val, nrowc, sec_starts, has_b2)
    nc.compile()
    _BUILD_CACHE[key] = nc
    return nc


def _fp8(x, scale):
    return np.clip(np.asarray(x, np.float32) * scale, -240.0, 240.0).astype(E4NP)


def plan_compaction(ins):
    i = int(ins["i"])
    left = np.asarray(ins["left_idx"], np.int64)
    right = np.asarray(ins["right_idx"], np.int64)
    m_ar = np.arange(M, dtype=np.int64)
    tsets = []
    for s in range(S):
        tset = np.union1d(left, m_ar) if s < i else np.union1d(left, right)
        tsets.append(np.sort(tset))
    sec_starts = []
    rows = []
    rowmap = {}
    for s in range(S):
        sec_starts.append(len(rows))
        for tt in tsets[s]:
            rowmap[(s, int(tt))] = len(rows)
            rows.append((s, int(tt)))
    nraw = len(rows)
    nrowc = ((nraw + P - 1) // P) * P
    idxm = np.zeros((P, 16), np.int32)
    for s in range(S):
        for mm in range(M):
            idxm[mm, 0 * 8 + s] = rowmap[(s, int(left[mm]))]
            t1 = mm if s < i else int(right[mm])
            idxm[mm, 1 * 8 + s] = rowmap[(s, t1)]
    return {
        "i": i, "rows": rows, "nrowc": nrowc,
        "sec_starts": tuple(sec_starts[:i]), "idx": idxm,
    }


def _prep_shared(ins, plan):
    f32 = np.float32
    kW1, kW2, kW3 = ins["kW1"], ins["kW2"], ins["kW3"]
    vW1, vW2, vW3 = ins["vW1"], ins["vW2"], ins["vW3"]
    kb1, kb2 = ins["kb1"], ins["kb2"]
    vb1, vb2, vb3 = ins["vb1"], ins["vb2"], ins["vb3"]
    if np.any(np.asarray(kb1, np.float32)) or np.any(np.asarray(vb1, np.float32)):
        raise NotImplementedError("nonzero kb1/vb1 not folded (u-row rank-1 "
                                  "carries no bias term)")

    W1K8 = np.empty((NCH, P, 2, F), E4NP)
    W2K8 = np.empty((NCH, P, 2, F), E4NP)
    W3K8 = np.empty((NCH, P, 2, DK), E4NP)
    W1V = np.empty((NCH, P, 2, F), BFNP)
    W2V = np.empty((NCH, P, 2, F), BFNP)
    W3V = np.empty((NCH, P, 2, DK), BFNP)
    W1LB = np.empty((2 * NCH, 4, F), BFNP)
    PB = np.zeros((P, PB_COLS), f32)
    BC = np.zeros((BC_COLS,), f32)

    def pack2(w, ncols):
        return np.asarray(w, f32).reshape(2, P, ncols).transpose(1, 0, 2)

    for l in range(L):
        for h in range(H):
            c = l * 8 + h
            W1K8[c] = _fp8(pack2(kW1[l, h][:256], F), W1S)
            W2K8[c] = _fp8(pack2(kW2[l, h], F), W2S)
            W3K8[c] = _fp8(pack2(kW3[l, h], DK), W3S)
            W1LB[c] = np.broadcast_to(
                (np.asarray(kW1[l, h][256], f32) * W1S).astype(BFNP), (4, F))
            W1V[c] = (pack2(vW1[l, h][:256], F) * 64.0).astype(BFNP)
            W2V[c] = pack2(vW2[l, h], F).astype(BFNP)
            W3V[c] = pack2(vW3[l, h], DK).astype(BFNP)
            W1LB[NCH + c] = np.broadcast_to(
                (np.asarray(vW1[l, h][256], f32) * W1S).astype(BFNP), (4, F))
            for gt in range(2):
                gsl = slice(gt * P, (gt + 1) * P)
                PB[:, PB_B2KP + 2 * c + gt] = B2KS * np.asarray(kb2[l, h][gsl], f32)
                PB[:, PB_B2KN + 2 * c + gt] = -B2KS * np.asarray(kb2[l, h][gsl], f32)
                PB[:, PB_B2VP + 2 * c + gt] = np.asarray(vb2[l, h][gsl], f32)
                PB[:, PB_B2VN + 2 * c + gt] = -np.asarray(vb2[l, h][gsl], f32)

    DSW = np.asarray(ins["ds_W"], f32).reshape(2, P, HD).transpose(1, 0, 2).copy()
    BC[BC_DSB:BC_DSB + HD] = np.asarray(ins["ds_b"], f32)
    for l in range(L):
        BC[BC_B3V8 + l * HD:BC_B3V8 + (l + 1) * HD] = \
            S * np.asarray(vb3[l], f32).reshape(HD)

    FFW1 = np.empty((L, P, 4, HD), BFNP)
    FFW2 = np.empty((L, P, 4, HD), BFNP)
    for l in range(L):
        FFW1[l] = np.asarray(ins["ffW1"][l], f32).reshape(4, P, HD).transpose(1, 0, 2).astype(BFNP)
        FFW2[l] = np.asarray(ins["ffW2"][l], f32).reshape(4, P, HD).transpose(1, 0, 2).astype(BFNP)
        for ft in range(4):
            PB[:, PB_FFB1 + 4 * l + ft] = np.asarray(ins["ffb1"][l][ft * P:(ft + 1) * P], f32)
        BC[BC_FFB2 + l * HD:BC_FFB2 + (l + 1) * HD] = np.asarray(ins["ffb2"][l], f32)
        BC[BC_LN1G + l * HD:BC_LN1G + (l + 1) * HD] = np.asarray(ins["ln1_g"][l], f32)
        BC[BC_LN1B + l * HD:BC_LN1B + (l + 1) * HD] = np.asarray(ins["ln1_b"][l], f32)
        BC[BC_LN2G + l * HD:BC_LN2G + (l + 1) * HD] = np.asarray(ins["ln2_g"][l], f32)
        BC[BC_LN2B + l * HD:BC_LN2B + (l + 1) * HD] = np.asarray(ins["ln2_b"][l], f32)

    DEW1 = np.asarray(ins["deW1"], f32).reshape(4, P, F).transpose(1, 0, 2).astype(BFNP)
    DEW2 = np.asarray(ins["deW2"], f32).reshape(2, P, F).transpose(1, 0, 2).astype(BFNP)
    DEW3 = np.asarray(ins["deW3"], f32).reshape(2, P, R).transpose(1, 0, 2).astype(BFNP)
    for ft in range(2):
        PB[:, PB_DEB1 + ft] = np.asarray(ins["deb1"][ft * P:(ft + 1) * P], f32)
        PB[:, PB_DEB2 + ft] = np.asarray(ins["deb2"][ft * P:(ft + 1) * P], f32)
    BC[BC_DEB3:BC_DEB3 + R] = np.asarray(ins["deb3"], f32)

    BCAST = np.broadcast_to(BC.astype(BFNP), (P, BC_COLS)).copy()

    return {
        "W1K8": W1K8, "W2K8": W2K8, "W3K8": W3K8, "W1LB": W1LB,
        "W1V": W1V, "W2V": W2V, "W3V": W3V,
        "DSW": DSW, "FFW1": FFW1, "FFW2": FFW2,
        "DEW1": DEW1, "DEW2": DEW2, "DEW3": DEW3,
        "PBIAS": PB, "BCAST": BCAST, "IDX": plan["idx"],
    }


def make_in_maps(ins, plan=None):
    if plan is None:
        plan = plan_compaction(ins)
    shared = _prep_shared(ins, plan)
    enc = np.asarray(ins["encoded"], np.float32)
    tu = np.asarray(ins["true_u"], np.float32)
    mid = np.asarray(ins["mid_idx"], np.int64)
    i = plan["i"]
    nrowc = plan["nrowc"]
    rows = plan["rows"]
    s_idx = np.array([r[0] for r in rows], np.int64)
    t_idx = np.array([r[1] for r in rows], np.int64)

    in_maps = []
    for b in range(B):
        xg = np.zeros((nrowc, D), np.float32)
        xg[:len(rows)] = enc[b][s_idx, t_idx]
        ug = np.zeros((nrowc,), np.float32)
        ug[:len(rows)] = tu[b][s_idx, t_idx]
        xs16 = xg.T.reshape(2, P, nrowc).transpose(1, 0, 2) * XS
        xub = np.broadcast_to((ug * XS).astype(BFNP), (4, nrowc))
        pred = enc[b, i][mid]
        predt = pred.T.reshape(2, P, M).transpose(1, 0, 2).copy()
        m = dict(shared)
        m["XT8"] = _fp8(xs16, 1.0)
        m["XTB"] = xs16.astype(BFNP)
        m["XTUB"] = np.ascontiguousarray(xub)
        m["PREDT"] = predt
        in_maps.append(m)
    return in_maps


def kernel(**inputs):
    ins = {k: np.asarray(v) for k, v in inputs.items()}
    plan = plan_compaction(ins)
    in_maps = make_in_maps(ins, plan)
    has_b2 = bool(np.any(np.asarray(ins["kb2"], np.float32))
                  or np.any(np.asarray(ins["vb2"], np.float32)))
    nc = _build(plan["i"], plan["nrowc"], plan["sec_starts"], has_b2)
    res = run_bass_kernel_spmd(nc, in_maps, core_ids=list(range(NCORES)))
    out = np.stack([res.results[c]["OUT"] for c in range(NCORES)])
    return out.astype(np.float32)


# revision 25
# speedup vs baseline: 1.1156x; 1.0584x over previous
"""Trainium2 Bass kernel for nn_AttentionalCopula (sparse_attention).

Sharding: data-parallel over batch (B=8 -> 8 cores); per-head K/V MLP stacks
computed locally per core. Weights replicated. No collectives.

Key optimizations over the v1 baseline:
  * Row compaction: the attention only gathers K/V rows from
    union_s({left} u ({arange} if s<i else {right})) -- ~1350 of 2048 rows.
    The MLP chains run only on those rows.
  * K chains in fp8 (float8e4) with MatmulPerfMode.DoubleRow: K_eff=256 per
    matmul at 0.5 cyc/col (4x bf16). V chains stay bf16 (V values feed the
    output directly; fp8 there fails the accuracy gate -- measured).
  * The u-row (feature 257) enters L1 psum via a K=1 bf16 rank-1 matmul
    (tile_position-paired across the two f-tiles).
  * Relu/copy work is round-robined across ACT/DVE/Pool engines.
  * l=0 K/V staged interleaved in DRAM so one indirect gather per slot
    fetches both; contiguous slots (n=1, s<i) use direct DMA.
"""

from contextlib import ExitStack

import ml_dtypes
import numpy as np

import concourse.bass as bass
import concourse.mybir as mybir
import concourse.tile as tile
from concourse import bacc
from concourse.bass_utils import run_bass_kernel_spmd
from concourse.masks import make_identity

B, S, T, D = 8, 8, 256, 256
H, DK = 8, 64
HD = H * DK            # 512
L = 2
F = 256
R = 512
M = 128
EPS = 1e-5
NCORES = 8
P = 128

F32 = mybir.dt.float32
F32R = mybir.dt.float32r
BF16 = mybir.dt.bfloat16
FP8 = mybir.dt.float8e4
I32 = mybir.dt.int32
E4NP = ml_dtypes.float8_e4m3
BFNP = ml_dtypes.bfloat16
DRMODE = mybir.MatmulPerfMode.DoubleRow

NCH = L * H            # 16 chains per kv side

# fp8 scale plan (K side):  X*16, W1*64 -> psum 1024*pre1 ; h1'=psum/64 (16*h1)
# W2*2 -> psum2 32*pre2 ; h2' = max(psum2 + 32*b2, 0) (32*h2)
# W3*64 -> psum3 2048*kv ; stag = psum3/2048
XS = 16.0
W1S = 64.0
H1DIV = 64.0
W2S = 2.0
B2KS = 32.0
W3S = 64.0
L3KDIV = 2048.0
# V side: X*16 (bf16), W1*64 (bf16) -> psum 1024*pre1 ; h1 = psum/1024
V1DIV = 1024.0

# PBIAS layout (per-partition f32 scalars)
PB_FFB1 = 0                      # 4 per l
PB_DEB1 = PB_FFB1 + 4 * L
PB_DEB2 = PB_DEB1 + 2
PB_B2KP = PB_DEB2 + 2            # +32*b2k per (c, gt)
PB_B2KN = PB_B2KP + 2 * NCH      # -32*b2k
PB_B2VP = PB_B2KN + 2 * NCH      # +b2v
PB_B2VN = PB_B2VP + 2 * NCH      # -b2v
PB_COLS = PB_B2VN + 2 * NCH

# BCAST layout (free-dim vectors, bf16, replicated across partitions)
BC_DSB = 0
BC_B3V8 = BC_DSB + HD            # S * vb3 per l
BC_FFB2 = BC_B3V8 + L * HD
BC_DEB3 = BC_FFB2 + L * HD
BC_LN1G = BC_DEB3 + R
BC_LN1B = BC_LN1G + L * HD
BC_LN2G = BC_LN1B + L * HD
BC_LN2B = BC_LN2G + L * HD
BC_COLS = BC_LN2B + L * HD


def _chunks(nrowc):
    out = []
    col = 0
    while col < nrowc:
        w = min(512, nrowc - col)
        out.append((col, w))
        col += w
    return out


class _RR:
    def __init__(self, pattern):
        self.pattern = pattern
        self.i = 0

    def next(self):
        e = self.pattern[self.i % len(self.pattern)]
        self.i += 1
        return e


def _emit(nc, tc, t, i_val, nrowc, sec_starts, has_b2):
    (XT8, XTB, XTUB, PREDT, DSW, W1K8, W2K8, W3K8, W1LB,
     W1V, W2V, W3V, FFW1, FFW2, DEW1, DEW2, DEW3,
     PBIAS, BCAST, IDX, OUT, KD0, VD0, KD1, VD1A, VD1B) = t

    NRT = nrowc // P
    chunks = _chunks(nrowc)
    relu_rr = _RR("AAD")      # Pool cannot read PSUM on TRN2
    copy_rr = _RR("AD")

    with ExitStack() as ctx:
        cp = ctx.enter_context(tc.tile_pool(name="const", bufs=1))
        wp = ctx.enter_context(tc.tile_pool(name="w", bufs=3))
        sp = ctx.enter_context(tc.tile_pool(name="stag", bufs=2))
        hp = ctx.enter_context(tc.tile_pool(name="h", bufs=2))
        gp0 = ctx.enter_context(tc.tile_pool(name="gath0", bufs=2))
        gp1 = ctx.enter_context(tc.tile_pool(name="gath1", bufs=2))
        ap = ctx.enter_context(tc.tile_pool(name="attn", bufs=1))
        pp = ctx.enter_context(tc.tile_pool(name="ps", bufs=5, space="PSUM"))
        pq3 = ctx.enter_context(tc.tile_pool(name="ps3", bufs=2, space="PSUM"))
        pqa = ctx.enter_context(tc.tile_pool(name="pa", bufs=1, space="PSUM"))

        # ---- resident loads; chain-0 critical path first ----
        xtub = cp.tile([97, nrowc], BF16)
        nc.scalar.dma_start(xtub[0:97:32, 0:512], XTUB.ap()[:, 0:512])
        nc.scalar.dma_start(xtub[0:97:32, 512:], XTUB.ap()[:, 512:])
        xt8 = cp.tile([P, 2, nrowc], FP8)
        nc.sync.dma_start(xt8[:, :, 0:256], XT8.ap()[:, :, 0:256])
        nc.scalar.dma_start(xt8[:, :, 256:512], XT8.ap()[:, :, 256:512])
        pbias = cp.tile([P, PB_COLS], F32)
        nc.gpsimd.dma_start(pbias[:], PBIAS.ap())
        for col, w in chunks:
            if col == 0:
                continue
            nc.scalar.dma_start(xt8[:, :, col:col + w], XT8.ap()[:, :, col:col + w])
        idx = cp.tile([P, 16], I32)
        nc.gpsimd.dma_start(idx[:], IDX.ap())
        xtb = cp.tile([P, 2, nrowc], BF16)
        for k, (col, w) in enumerate(chunks):
            q = nc.scalar if k % 2 == 0 else nc.gpsimd
            q.dma_start(xtb[:, :, col:col + w], XTB.ap()[:, :, col:col + w])
        predt = cp.tile([P, 2, M], F32R)
        nc.gpsimd.dma_start(predt[:], PREDT.ap())
        dsw = cp.tile([P, 2, HD], F32R)
        nc.gpsimd.dma_start(dsw[:], DSW.ap())
        bcast = cp.tile([P, BC_COLS], BF16)
        nc.gpsimd.dma_start(bcast[:], BCAST.ap())
        ffw1 = cp.tile([P, L, 4, HD], BF16)
        nc.gpsimd.dma_start(ffw1[:], FFW1.ap().rearrange("l p a h -> p l a h"))
        ffw2 = cp.tile([P, L, 4, HD], BF16)
        nc.gpsimd.dma_start(ffw2[:], FFW2.ap().rearrange("l p a h -> p l a h"))
        dew1 = cp.tile([P, 4, F], BF16)
        nc.gpsimd.dma_start(dew1[:], DEW1.ap())
        dew2 = cp.tile([P, 2, F], BF16)
        nc.gpsimd.dma_start(dew2[:], DEW2.ap())
        dew3 = cp.tile([P, 2, R], BF16)
        nc.gpsimd.dma_start(dew3[:], DEW3.ap())
        ident = cp.tile([P, P], F32)
        make_identity(nc, ident[:])
        epst = cp.tile([P, 1], F32)
        nc.vector.memset(epst[:], EPS)

        def relu_out(ps_ap, out_ap, scale=None, bias=None):
            e = relu_rr.next()
            if e == "A":
                if bias is None:
                    nc.scalar.activation(out=out_ap, in_=ps_ap,
                                         func=mybir.ActivationFunctionType.Relu,
                                         scale=scale)
                else:
                    nc.scalar.activation(out=out_ap, in_=ps_ap,
                                         func=mybir.ActivationFunctionType.Relu,
                                         bias=pbias[:, bias[0]:bias[0] + 1])
            else:
                eng = nc.vector if e == "D" else nc.gpsimd
                if bias is None:
                    eng.tensor_scalar(out=out_ap, in0=ps_ap,
                                      scalar1=float(scale), scalar2=0.0,
                                      op0=mybir.AluOpType.mult,
                                      op1=mybir.AluOpType.max)
                else:
                    eng.tensor_scalar(out=out_ap, in0=ps_ap,
                                      scalar1=pbias[:, bias[1]:bias[1] + 1],
                                      scalar2=pbias[:, bias[0]:bias[0] + 1],
                                      op0=mybir.AluOpType.max,
                                      op1=mybir.AluOpType.add)

        def copy_out(ps_ap, out_ap, scale):
            e = copy_rr.next()
            if e == "A":
                nc.scalar.activation(out=out_ap, in_=ps_ap,
                                     func=mybir.ActivationFunctionType.Copy,
                                     scale=scale)
            else:
                eng = nc.vector if e == "D" else nc.gpsimd
                eng.tensor_scalar(out=out_ap, in0=ps_ap,
                                  scalar1=float(scale), scalar2=0.0,
                                  op0=mybir.AluOpType.mult,
                                  op1=mybir.AluOpType.add)

        def k_chain(c, h, stag):
            """fp8 DoubleRow chain, software-pipelined:
            L1(ci) -> L2(ci-1) -> L3(ci-2) so relus hide under matmuls."""
            w1l = wp.tile([97, F], BF16, tag="w1lk")
            nc.sync.dma_start(w1l[0:97:32, :], W1LB.ap()[c])
            w1 = wp.tile([P, 2, F], FP8, tag="w1k")
            nc.sync.dma_start(w1[:], W1K8.ap()[c])
            w2 = wp.tile([P, 2, F], FP8, tag="w2k")
            nc.sync.dma_start(w2[:], W2K8.ap()[c])
            w3 = wp.tile([P, 2, DK], FP8, tag="w3k")
            nc.sync.dma_start(w3[:], W3K8.ap()[c])

            st = {}

            def l1(ci):
                col, cw = chunks[ci]
                h1t = hp.tile([P, 2, 512], FP8, tag="h1k")
                # rank-1 u-terms OPEN the full psum region (a start=False
                # accumulate spanning two DR-opened regions corrupts psum, so
                # open wide first, then let the DR mains accumulate densely)
                ps = []
                for ft in range(2):
                    fsl = slice(ft * P, (ft + 1) * P)
                    rg = 32 * ft
                    ps1 = pp.tile([P, 512], F32, tag="ps")
                    ps.append(ps1)
                    nc.tensor.matmul(ps1[:, 0:cw], w1l[rg:rg + 1, fsl],
                                     xtub[rg:rg + 1, col:col + cw],
                                     start=True, stop=False, tile_position=(rg, 0))
                for ft in range(2):
                    fsl = slice(ft * P, (ft + 1) * P)
                    for cc in range(0, cw, 256):
                        ccw = min(256, cw - cc)
                        csl = slice(col + cc, col + cc + ccw)
                        nc.tensor.matmul(ps[ft][:, cc:cc + ccw], w1[:, :, fsl],
                                         xt8[:, :, csl], start=False,
                                         stop=(cc + 256 >= cw),
                                         perf_mode=DRMODE, skip_group_check=True)
                for ft in range(2):
                    relu_out(ps[ft][:, 0:cw], h1t[:, ft, 0:cw], scale=1.0 / H1DIV)
                st[ci] = [h1t, None]

            def l2(ci):
                col, cw = chunks[ci]
                h1t = st[ci][0]
                h2t = hp.tile([P, 2, 512], FP8, tag="h2k")
                ps = []
                for gt in range(2):
                    gsl = slice(gt * P, (gt + 1) * P)
                    ps2 = pp.tile([P, 512], F32, tag="ps")
                    ps.append(ps2)
                    for cc in range(0, cw, 256):
                        ccw = min(256, cw - cc)
                        nc.tensor.matmul(ps2[:, cc:cc + ccw], w2[:, :, gsl],
                                         h1t[:, :, cc:cc + ccw], start=True,
                                         stop=True, perf_mode=DRMODE)
                for gt in range(2):
                    relu_out(ps[gt][:, 0:cw], h2t[:, gt, 0:cw], scale=1.0,
                             bias=((PB_B2KP + 2 * c + gt, PB_B2KN + 2 * c + gt)
                                   if has_b2 else None))
                st[ci][1] = h2t

            def l3(ci):
                col, cw = chunks[ci]
                nrti = cw // P
                h2t = st[ci][1]
                ps3 = pq3.tile([P, 256], F32, tag="ps3")
                for rti in range(nrti):
                    rsl = slice(rti * P, (rti + 1) * P)
                    nc.tensor.matmul(ps3[:, rti * DK:(rti + 1) * DK],
                                     h2t[:, :, rsl], w3[:, :, :],
                                     start=True, stop=True, perf_mode=DRMODE)
                copy_out(ps3[:, 0:nrti * DK].rearrange("p (r d) -> p r d", d=DK),
                         stag[:, col // P:col // P + nrti, h * DK:(h + 1) * DK],
                         1.0 / L3KDIV)

            _pipeline(l1, l2, l3)

        def _pipeline(l1, l2, l3):
            n = len(chunks)
            for ci in range(n):
                l1(ci)
                if ci >= 1:
                    l2(ci - 1)
                if ci >= 2:
                    l3(ci - 2)
            l2(n - 1)
            if n >= 2:
                l3(n - 2)
            l3(n - 1)

        def v_chain(c, h, stag):
            w1l = wp.tile([97, F], BF16, tag="w1lv")
            nc.sync.dma_start(w1l[0:97:32, :], W1LB.ap()[NCH + c])
            w1 = wp.tile([P, 2, F], BF16, tag="w1v")
            nc.sync.dma_start(w1[:], W1V.ap()[c])
            w2 = wp.tile([P, 2, F], BF16, tag="w2v")
            nc.sync.dma_start(w2[:], W2V.ap()[c])
            w3 = wp.tile([P, 2, DK], BF16, tag="w3v")
            nc.sync.dma_start(w3[:], W3V.ap()[c])

            st = {}

            def l1(ci):
                col, cw = chunks[ci]
                h1t = hp.tile([P, 2, 512], BF16, tag="h1v")
                ps = []
                for ft in range(2):
                    fsl = slice(ft * P, (ft + 1) * P)
                    rg = 32 * ft
                    ps1 = pp.tile([P, 512], F32, tag="ps")
                    ps.append(ps1)
                    nc.tensor.matmul(ps1[:, 0:cw], w1l[rg:rg + 1, fsl],
                                     xtub[rg:rg + 1, col:col + cw],
                                     start=True, stop=False, tile_position=(rg, 0))
                for ft in range(2):
                    fsl = slice(ft * P, (ft + 1) * P)
                    nc.tensor.matmul(ps[ft][:, 0:cw], w1[:, 0, fsl],
                                     xtb[:, 0, col:col + cw], start=False, stop=False,
                                     skip_group_check=True)
                    nc.tensor.matmul(ps[ft][:, 0:cw], w1[:, 1, fsl],
                                     xtb[:, 1, col:col + cw], start=False, stop=True,
                                     skip_group_check=True)
                for ft in range(2):
                    relu_out(ps[ft][:, 0:cw], h1t[:, ft, 0:cw], scale=1.0 / V1DIV)
                st[ci] = [h1t, None]

            def l2(ci):
                col, cw = chunks[ci]
                h1t = st[ci][0]
                h2t = hp.tile([P, 2, 512], BF16, tag="h2v")
                ps = []
                for gt in range(2):
                    gsl = slice(gt * P, (gt + 1) * P)
                    ps2 = pp.tile([P, 512], F32, tag="ps")
                    ps.append(ps2)
                    nc.tensor.matmul(ps2[:, 0:cw], w2[:, 0, gsl],
                                     h1t[:, 0, 0:cw], start=True, stop=False)
                    nc.tensor.matmul(ps2[:, 0:cw], w2[:, 1, gsl],
                                     h1t[:, 1, 0:cw], start=False, stop=True)
                for gt in range(2):
                    relu_out(ps[gt][:, 0:cw], h2t[:, gt, 0:cw], scale=1.0,
                             bias=((PB_B2VP + 2 * c + gt, PB_B2VN + 2 * c + gt)
                                   if has_b2 else None))
                st[ci][1] = h2t

            def l3(ci):
                col, cw = chunks[ci]
                nrti = cw // P
                h2t = st[ci][1]
                ps3 = pq3.tile([P, 256], F32, tag="ps3")
                for rti in range(nrti):
                    rsl = slice(rti * P, (rti + 1) * P)
                    nc.tensor.matmul(ps3[:, rti * DK:(rti + 1) * DK],
                                     h2t[:, 0, rsl], w3[:, 0, :],
                                     start=True, stop=False)
                    nc.tensor.matmul(ps3[:, rti * DK:(rti + 1) * DK],
                                     h2t[:, 1, rsl], w3[:, 1, :],
                                     start=False, stop=True)
                copy_out(ps3[:, 0:nrti * DK].rearrange("p (r d) -> p r d", d=DK),
                         stag[:, col // P:col // P + nrti, h * DK:(h + 1) * DK],
                         1.0)

            _pipeline(l1, l2, l3)

        def ds_project():
            ps = pqa.tile([P, HD], F32, tag="pa")
            nc.tensor.matmul(ps[:], predt[:, 0, :], dsw[:, 0, :], start=True, stop=False)
            nc.tensor.matmul(ps[:], predt[:, 1, :], dsw[:, 1, :], start=False, stop=True)
            av0 = ap.tile([P, HD], F32, tag="av")
            nc.vector.tensor_tensor(out=av0[:], in0=ps[:],
                                    in1=bcast[:, BC_DSB:BC_DSB + HD],
                                    op=mybir.AluOpType.add)
            return av0

        def layer_norm(src, gcol, bcol, out):
            stats = ap.tile([P, 6], F32, tag="lnstat")
            nc.vector.bn_stats(stats[:], src[:])
            mv = ap.tile([P, 2], F32, tag="lnmv")
            nc.vector.bn_aggr(mv[:], stats[:])
            rstd = ap.tile([P, 1], F32, tag="lnrstd")
            nc.scalar.activation(out=rstd[:], in_=mv[:, 1:2],
                                 func=mybir.ActivationFunctionType.Sqrt,
                                 bias=epst[:, 0:1])
            nc.vector.reciprocal(rstd[:], rstd[:])
            nc.vector.tensor_scalar(out=out[:], in0=src[:], scalar1=mv[:, 0:1],
                                    scalar2=rstd[:, 0:1],
                                    op0=mybir.AluOpType.subtract,
                                    op1=mybir.AluOpType.mult)
            nc.vector.tensor_tensor(out=out[:], in0=out[:],
                                    in1=bcast[:, gcol:gcol + HD],
                                    op=mybir.AluOpType.mult)
            nc.vector.tensor_tensor(out=out[:], in0=out[:],
                                    in1=bcast[:, bcol:bcol + HD],
                                    op=mybir.AluOpType.add)

        def transpose_to(src, dst):
            for kt in range(4):
                tp_ = pp.tile([P, P], F32, tag="ps")
                nc.tensor.transpose(tp_[:], src[:, kt * P:(kt + 1) * P], ident[:])
                nc.scalar.activation(out=dst[:, kt, :], in_=tp_[:],
                                     func=mybir.ActivationFunctionType.Copy)

        def gather1(src, gall):
            """l=1: one of KD1/VD1 into [P, 16, HD]."""
            for j in range(16):
                n, s = j // 8, j % 8
                if n == 1 and s < i_val:
                    sec = sec_starts[s]
                    nc.sync.dma_start(gall[:, j, :], src.ap()[sec:sec + P, :])
                else:
                    nc.gpsimd.indirect_dma_start(
                        out=gall[:, j, :], out_offset=None, in_=src.ap(),
                        in_offset=bass.IndirectOffsetOnAxis(ap=idx[:, j:j + 1], axis=0))

        def attn_phase1(l, av_in, wall, kg):
            """kg: callable js_slice -> [P, 4, HD] bf16 K rows."""
            avbf = ap.tile([P, HD], BF16, tag="avbf")
            nc.vector.tensor_copy(out=avbf[:], in_=av_in[:])
            scrb = ap.tile([P, 4, HD], BF16, tag="scrb")
            logits = ap.tile([P, 16, 8], BF16, tag="logits")
            avb = avbf[:, None, :].to_broadcast([P, 4, HD])
            for js in range(4):
                jsl = slice(js * 4, (js + 1) * 4)
                nc.vector.tensor_tensor(out=scrb[:], in0=kg(jsl),
                                        in1=avb, op=mybir.AluOpType.mult)
                with nc.allow_low_precision(reason="logits bf16; sigmoid smooths"):
                    nc.vector.tensor_reduce(
                        out=logits[:, jsl, :],
                        in_=scrb[:].rearrange("p a (h d) -> p (a h) d", d=DK),
                        axis=mybir.AxisListType.X, op=mybir.AluOpType.add)
            delta = ap.tile([P, 8, 8], BF16, tag="delta")
            nc.vector.tensor_tensor(out=delta[:], in0=logits[:, 0:8, :],
                                    in1=logits[:, 8:16, :],
                                    op=mybir.AluOpType.subtract)
            scale = DK ** (-0.5)
            dflat = delta[:].rearrange("p a b -> p (a b)")
            nc.scalar.activation(out=wall[:, 0:64], in_=dflat,
                                 func=mybir.ActivationFunctionType.Sigmoid,
                                 scale=scale)
            nc.scalar.activation(out=wall[:, 64:128], in_=dflat,
                                 func=mybir.ActivationFunctionType.Sigmoid,
                                 scale=-scale)

        def phase2_step(l, wall, vg, js, att, part, scr, half=None):
            # scr is [P, HD, 4] bf16: slot dim packed last so the reduce and
            # the accumulation run in the DVE fast (2x/4x) modes
            jsl = slice(js * 4, (js + 1) * 4)
            wv = wall[:].rearrange("p (j h) -> p j h", h=8)
            if half is None:
                hsl, nh = slice(0, HD), 8
            else:
                hsl, nh = slice(half * (HD // 2), (half + 1) * (HD // 2)), 4
            wvh = wv[:, jsl, half * 4:half * 4 + 4] if half is not None \
                else wv[:, jsl, :]
            nc.vector.tensor_tensor(
                out=scr[:, hsl, :].rearrange("p (h d) a -> p a h d", d=DK),
                in0=vg(jsl).rearrange("p a (h d) -> p a h d", d=DK),
                in1=wvh[:, :, :, None].to_broadcast([P, 4, nh, DK]),
                op=mybir.AluOpType.mult)
            dst = att if js == 0 else part
            with nc.allow_low_precision(reason="slot-sum in bf16"):
                nc.vector.tensor_reduce(
                    out=dst[:, hsl],
                    in_=scr[:, hsl, :],
                    axis=mybir.AxisListType.X, op=mybir.AluOpType.add)
            if js > 0:
                nc.vector.tensor_tensor(out=att[:, hsl], in0=att[:, hsl],
                                        in1=part[:, hsl],
                                        op=mybir.AluOpType.add)

        def attn_phase2_tail(l, avp, att):
            attf = ap.tile([P, HD], F32, tag="attf")
            nc.vector.tensor_tensor(out=attf[:], in0=att[:], in1=avp[:],
                                    op=mybir.AluOpType.add)
            xn = ap.tile([P, HD], F32, tag="xn")
            layer_norm(attf, BC_LN1G + l * HD, BC_LN1B + l * HD, xn)
            xT = ap.tile([P, 4, P], BF16, tag="xT")
            transpose_to(xn, xT)
            ff1 = ap.tile([P, 4, P], BF16, tag="ff1")
            for ft in range(4):
                psf = pp.tile([P, P], F32, tag="ps")
                for kt in range(4):
                    nc.tensor.matmul(psf[:], ffw1[:, l, kt, ft * P:(ft + 1) * P],
                                     xT[:, kt, :], start=(kt == 0), stop=(kt == 3))
                nc.scalar.activation(
                    out=ff1[:, ft, :], in_=psf[:],
                    func=mybir.ActivationFunctionType.Relu,
                    bias=pbias[:, PB_FFB1 + 4 * l + ft:PB_FFB1 + 4 * l + ft + 1])
            ps2 = pqa.tile([P, HD], F32, tag="pa")
            for kt in range(4):
                nc.tensor.matmul(ps2[:], ff1[:, kt, :], ffw2[:, l, kt, :],
                                 start=(kt == 0), stop=(kt == 3))
            ffx = ap.tile([P, HD], F32, tag="ffx")
            nc.vector.tensor_tensor(out=ffx[:], in0=ps2[:],
                                    in1=bcast[:, BC_FFB2 + l * HD:BC_FFB2 + (l + 1) * HD],
                                    op=mybir.AluOpType.add)
            nc.vector.tensor_tensor(out=ffx[:], in0=ffx[:], in1=xn[:],
                                    op=mybir.AluOpType.add)
            av_out = ap.tile([P, HD], F32, tag="av")
            layer_norm(ffx, BC_LN2G + l * HD, BC_LN2B + l * HD, av_out)
            return av_out

        def make_avp(l, av_in):
            avp = ap.tile([P, HD], F32, tag="avp")
            nc.vector.tensor_tensor(
                out=avp[:], in0=av_in[:],
                in1=bcast[:, BC_B3V8 + l * HD:BC_B3V8 + (l + 1) * HD],
                op=mybir.AluOpType.add)
            return avp

        # ================= schedule =================
        stag = sp.tile([P, NRT, HD], BF16, tag="stag")
        av = None
        for h in range(H):
            k_chain(0 * 8 + h, h, stag)
            if h == 0:
                av = ds_project()
        nc.sync.dma_start(
            KD0.ap().rearrange("(rt p) hd -> p rt hd", p=P), stag[:])
        k0gall = gp0.tile([P, 16, HD], BF16, tag="gall0")
        gather1(KD0, k0gall)

        stag = sp.tile([P, NRT, HD], BF16, tag="stag")
        for h in range(H):
            v_chain(0 * 8 + h, h, stag)
        nc.sync.dma_start(
            VD0.ap().rearrange("(rt p) hd -> p rt hd", p=P), stag[:])
        v0gall = gp0.tile([P, 16, HD], BF16, tag="gall0")
        gather1(VD0, v0gall)

        # l=1 K chains; l=0 attention interleaved (phase2 spread over chains)
        stag = sp.tile([P, NRT, HD], BF16, tag="stag")
        wall0 = ap.tile([P, 128], F32, tag="wall")
        att0 = ap.tile([P, HD], BF16, tag="att")
        part0 = ap.tile([P, HD], BF16, tag="part")
        scr0 = ap.tile([P, HD, 4], BF16, tag="scr")
        vg0 = lambda jsl: v0gall[:, jsl, :]
        avp0 = None
        for h in range(H):
            k_chain(1 * 8 + h, h, stag)
            if h == 1:
                attn_phase1(0, av, wall0, lambda jsl: k0gall[:, jsl, :])
                avp0 = make_avp(0, av)
            elif 2 <= h <= 5:
                phase2_step(0, wall0, vg0, h - 2, att0, part0, scr0)
        nc.sync.dma_start(
            KD1.ap().rearrange("(rt p) hd -> p rt hd", p=P), stag[:])
        k1gall = gp1.tile([P, 16, HD], BF16, tag="gall1")
        gather1(KD1, k1gall)
        av = attn_phase2_tail(0, avp0, att0)

        # l=1 V chains; l=1 phase1 interleaved. The stag is written to DRAM
        # in two head-halves so V gathers + weighted sums start while the
        # last four chains still compute.
        stag = sp.tile([P, NRT, HD], BF16, tag="stag")
        wall1 = ap.tile([P, 128], F32, tag="wall")
        att1 = ap.tile([P, HD], BF16, tag="att")
        part1 = ap.tile([P, HD], BF16, tag="part")
        scr1 = ap.tile([P, HD, 4], BF16, tag="scr")
        v1gall = gp1.tile([P, 16, HD], BF16, tag="gall1")
        avp1 = None

        def v1_half(half, src_t):
            hsl = slice(half * (HD // 2), (half + 1) * (HD // 2))
            nc.sync.dma_start(
                src_t.ap().rearrange("(rt p) hd -> p rt hd", p=P),
                stag[:, :, hsl])
            for js in range(4):
                for j in range(js * 4, js * 4 + 4):
                    n, s = j // 8, j % 8
                    if n == 1 and s < i_val:
                        nc.sync.dma_start(
                            v1gall[:, j, hsl],
                            src_t.ap()[sec_starts[s]:sec_starts[s] + P, :])
                    else:
                        nc.gpsimd.indirect_dma_start(
                            out=v1gall[:, j, hsl], out_offset=None,
                            in_=src_t.ap(),
                            in_offset=bass.IndirectOffsetOnAxis(
                                ap=idx[:, j:j + 1], axis=0))
                phase2_step(1, wall1, lambda jsl: v1gall[:, jsl, hsl], js,
                            att1, part1, scr1, half=half)

        for h in range(H):
            v_chain(1 * 8 + h, h, stag)
            if h == 1:
                attn_phase1(1, av, wall1, lambda jsl: k1gall[:, jsl, :])
                avp1 = make_avp(1, av)
            elif h == 4:
                v1_half(0, VD1A)
        v1_half(1, VD1B)
        av = attn_phase2_tail(1, avp1, att1)

        # ---- dist extractor ----
        avT = ap.tile([P, 4, P], BF16, tag="avT")
        transpose_to(av, avT)
        h1 = ap.tile([P, 2, P], BF16, tag="deh1")
        for ft in range(2):
            psd = pp.tile([P, P], F32, tag="ps")
            for kt in range(4):
                nc.tensor.matmul(psd[:], dew1[:, kt, ft * P:(ft + 1) * P],
                                 avT[:, kt, :], start=(kt == 0), stop=(kt == 3))
            nc.scalar.activation(out=h1[:, ft, :], in_=psd[:],
                                 func=mybir.ActivationFunctionType.Relu,
                                 bias=pbias[:, PB_DEB1 + ft:PB_DEB1 + ft + 1])
        h2 = ap.tile([P, 2, P], BF16, tag="deh2")
        for ft in range(2):
            psd = pp.tile([P, P], F32, tag="ps")
            for kt in range(2):
                nc.tensor.matmul(psd[:], dew2[:, kt, ft * P:(ft + 1) * P],
                                 h1[:, kt, :], start=(kt == 0), stop=(kt == 1))
            nc.scalar.activation(out=h2[:, ft, :], in_=psd[:],
                                 func=mybir.ActivationFunctionType.Relu,
                                 bias=pbias[:, PB_DEB2 + ft:PB_DEB2 + ft + 1])
        pso = pqa.tile([P, R], F32, tag="pa")
        for kt in range(2):
            nc.tensor.matmul(pso[:], h2[:, kt, :], dew3[:, kt, :],
                             start=(kt == 0), stop=(kt == 1))
        o = ap.tile([P, R], F32, tag="out")
        nc.vector.tensor_tensor(out=o[:], in0=pso[:],
                                in1=bcast[:, BC_DEB3:BC_DEB3 + R],
                                op=mybir.AluOpType.add)
        nc.sync.dma_start(OUT.ap()[:, 0:R // 2], o[:, 0:R // 2])
        nc.scalar.dma_start(OUT.ap()[:, R // 2:], o[:, R // 2:])


_BUILD_CACHE = {}


def _build(i_val, nrowc, sec_starts, has_b2=False):
    key = (i_val, nrowc, tuple(sec_starts), has_b2)
    if key in _BUILD_CACHE:
        return _BUILD_CACHE[key]
    nc = bacc.Bacc("TRN2", target_bir_lowering=False, debug=False)
    t = []
    t.append(nc.dram_tensor("XT8", [P, 2, nrowc], FP8, kind="ExternalInput"))
    t.append(nc.dram_tensor("XTB", [P, 2, nrowc], BF16, kind="ExternalInput"))
    t.append(nc.dram_tensor("XTUB", [4, nrowc], BF16, kind="ExternalInput"))
    t.append(nc.dram_tensor("PREDT", [P, 2, M], F32R, kind="ExternalInput"))
    t.append(nc.dram_tensor("DSW", [P, 2, HD], F32R, kind="ExternalInput"))
    t.append(nc.dram_tensor("W1K8", [NCH, P, 2, F], FP8, kind="ExternalInput"))
    t.append(nc.dram_tensor("W2K8", [NCH, P, 2, F], FP8, kind="ExternalInput"))
    t.append(nc.dram_tensor("W3K8", [NCH, P, 2, DK], FP8, kind="ExternalInput"))
    t.append(nc.dram_tensor("W1LB", [2 * NCH, 4, F], BF16, kind="ExternalInput"))
    t.append(nc.dram_tensor("W1V", [NCH, P, 2, F], BF16, kind="ExternalInput"))
    t.append(nc.dram_tensor("W2V", [NCH, P, 2, F], BF16, kind="ExternalInput"))
    t.append(nc.dram_tensor("W3V", [NCH, P, 2, DK], BF16, kind="ExternalInput"))
    t.append(nc.dram_tensor("FFW1", [L, P, 4, HD], BF16, kind="ExternalInput"))
    t.append(nc.dram_tensor("FFW2", [L, P, 4, HD], BF16, kind="ExternalInput"))
    t.append(nc.dram_tensor("DEW1", [P, 4, F], BF16, kind="ExternalInput"))
    t.append(nc.dram_tensor("DEW2", [P, 2, F], BF16, kind="ExternalInput"))
    t.append(nc.dram_tensor("DEW3", [P, 2, R], BF16, kind="ExternalInput"))
    t.append(nc.dram_tensor("PBIAS", [P, PB_COLS], F32, kind="ExternalInput"))
    t.append(nc.dram_tensor("BCAST", [P, BC_COLS], BF16, kind="ExternalInput"))
    t.append(nc.dram_tensor("IDX", [P, 16], I32, kind="ExternalInput"))
    t.append(nc.dram_tensor("OUT", [M, R], F32, kind="ExternalOutput"))
    t.append(nc.dram_tensor("KD0", [nrowc, HD], BF16))
    t.append(nc.dram_tensor("VD0", [nrowc, HD], BF16))
    t.append(nc.dram_tensor("KD1", [nrowc, HD], BF16))
    t.append(nc.dram_tensor("VD1A", [nrowc, HD // 2], BF16))
    t.append(nc.dram_tensor("VD1B", [nrowc, HD // 2], BF16))
    with tile.TileContext(nc) as tc:
        _emit(nc, tc, t, i_val, nrowc, sec_starts, has_b2)
    nc.compile()
    _BUILD_CACHE[key] = nc
    return nc


def _fp8(x, scale):
    return np.clip(np.asarray(x, np.float32) * scale, -240.0, 240.0).astype(E4NP)


def plan_compaction(ins):
    i = int(ins["i"])
    left = np.asarray(ins["left_idx"], np.int64)
    right = np.asarray(ins["right_idx"], np.int64)
    m_ar = np.arange(M, dtype=np.int64)
    tsets = []
    for s in range(S):
        tset = np.union1d(left, m_ar) if s < i else np.union1d(left, right)
        tsets.append(np.sort(tset))
    sec_starts = []
    rows = []
    rowmap = {}
    for s in range(S):
        sec_starts.append(len(rows))
        for tt in tsets[s]:
            rowmap[(s, int(tt))] = len(rows)
            rows.append((s, int(tt)))
    nraw = len(rows)
    nrowc = ((nraw + P - 1) // P) * P
    idxm = np.zeros((P, 16), np.int32)
    for s in range(S):
        for mm in range(M):
            idxm[mm, 0 * 8 + s] = rowmap[(s, int(left[mm]))]
            t1 = mm if s < i else int(right[mm])
            idxm[mm, 1 * 8 + s] = rowmap[(s, t1)]
    return {
        "i": i, "rows": rows, "nrowc": nrowc,
        "sec_starts": tuple(sec_starts[:i]), "idx": idxm,
    }


def _prep_shared(ins, plan):
    f32 = np.float32
    kW1, kW2, kW3 = ins["kW1"], ins["kW2"], ins["kW3"]
    vW1, vW2, vW3 = ins["vW1"], ins["vW2"], ins["vW3"]
    kb1, kb2 = ins["kb1"], ins["kb2"]
    vb1, vb2, vb3 = ins["vb1"], ins["vb2"], ins["vb3"]
    if np.any(np.asarray(kb1, np.float32)) or np.any(np.asarray(vb1, np.float32)):
        raise NotImplementedError("nonzero kb1/vb1 not folded (u-row rank-1 "
                                  "carries no bias term)")

    W1K8 = np.empty((NCH, P, 2, F), E4NP)
    W2K8 = np.empty((NCH, P, 2, F), E4NP)
    W3K8 = np.empty((NCH, P, 2, DK), E4NP)
    W1V = np.empty((NCH, P, 2, F), BFNP)
    W2V = np.empty((NCH, P, 2, F), BFNP)
    W3V = np.empty((NCH, P, 2, DK), BFNP)
    W1LB = np.empty((2 * NCH, 4, F), BFNP)
    PB = np.zeros((P, PB_COLS), f32)
    BC = np.zeros((BC_COLS,), f32)

    def pack2(w, ncols):
        return np.asarray(w, f32).reshape(2, P, ncols).transpose(1, 0, 2)

    for l in range(L):
        for h in range(H):
            c = l * 8 + h
            W1K8[c] = _fp8(pack2(kW1[l, h][:256], F), W1S)
            W2K8[c] = _fp8(pack2(kW2[l, h], F), W2S)
            W3K8[c] = _fp8(pack2(kW3[l, h], DK), W3S)
            W1LB[c] = np.broadcast_to(
                (np.asarray(kW1[l, h][256], f32) * W1S).astype(BFNP), (4, F))
            W1V[c] = (pack2(vW1[l, h][:256], F) * 64.0).astype(BFNP)
            W2V[c] = pack2(vW2[l, h], F).astype(BFNP)
            W3V[c] = pack2(vW3[l, h], DK).astype(BFNP)
            W1LB[NCH + c] = np.broadcast_to(
                (np.asarray(vW1[l, h][256], f32) * W1S).astype(BFNP), (4, F))
            for gt in range(2):
                gsl = slice(gt * P, (gt + 1) * P)
                PB[:, PB_B2KP + 2 * c + gt] = B2KS * np.asarray(kb2[l, h][gsl], f32)
                PB[:, PB_B2KN + 2 * c + gt] = -B2KS * np.asarray(kb2[l, h][gsl], f32)
                PB[:, PB_B2VP + 2 * c + gt] = np.asarray(vb2[l, h][gsl], f32)
                PB[:, PB_B2VN + 2 * c + gt] = -np.asarray(vb2[l, h][gsl], f32)

    DSW = np.asarray(ins["ds_W"], f32).reshape(2, P, HD).transpose(1, 0, 2).copy()
    BC[BC_DSB:BC_DSB + HD] = np.asarray(ins["ds_b"], f32)
    for l in range(L):
        BC[BC_B3V8 + l * HD:BC_B3V8 + (l + 1) * HD] = \
            S * np.asarray(vb3[l], f32).reshape(HD)

    FFW1 = np.empty((L, P, 4, HD), BFNP)
    FFW2 = np.empty((L, P, 4, HD), BFNP)
    for l in range(L):
        FFW1[l] = np.asarray(ins["ffW1"][l], f32).reshape(4, P, HD).transpose(1, 0, 2).astype(BFNP)
        FFW2[l] = np.asarray(ins["ffW2"][l], f32).reshape(4, P, HD).transpose(1, 0, 2).astype(BFNP)
        for ft in range(4):
            PB[:, PB_FFB1 + 4 * l + ft] = np.asarray(ins["ffb1"][l][ft * P:(ft + 1) * P], f32)
        BC[BC_FFB2 + l * HD:BC_FFB2 + (l + 1) * HD] = np.asarray(ins["ffb2"][l], f32)
        BC[BC_LN1G + l * HD:BC_LN1G + (l + 1) * HD] = np.asarray(ins["ln1_g"][l], f32)
        BC[BC_LN1B + l * HD:BC_LN1B + (l + 1) * HD] = np.asarray(ins["ln1_b"][l], f32)
        BC[BC_LN2G + l * HD:BC_LN2G + (l + 1) * HD] = np.asarray(ins["ln2_g"][l], f32)
        BC[BC_LN2B + l * HD:BC_LN2B + (l + 1) * HD] = np.asarray(ins["ln2_b"][l], f32)

    DEW1 = np.asarray(ins["deW1"], f32).reshape(4, P, F).transpose(1, 0, 2).astype(BFNP)
    DEW2 = np.asarray(ins["deW2"], f32).reshape(2, P, F).transpose(1, 0, 2).astype(BFNP)
    DEW3 = np.asarray(ins["deW3"], f32).reshape(2, P, R).transpose(1, 0, 2).astype(BFNP)
    for ft in range(2):
        PB[:, PB_DEB1 + ft] = np.asarray(ins["deb1"][ft * P:(ft + 1) * P], f32)
        PB[:, PB_DEB2 + ft] = np.asarray(ins["deb2"][ft * P:(ft + 1) * P], f32)
    BC[BC_DEB3:BC_DEB3 + R] = np.asarray(ins["deb3"], f32)

    BCAST = np.broadcast_to(BC.astype(BFNP), (P, BC_COLS)).copy()

    return {
        "W1K8": W1K8, "W2K8": W2K8, "W3K8": W3K8, "W1LB": W1LB,
        "W1V": W1V, "W2V": W2V, "W3V": W3V,
        "DSW": DSW, "FFW1": FFW1, "FFW2": FFW2,
        "DEW1": DEW1, "DEW2": DEW2, "DEW3": DEW3,
        "PBIAS": PB, "BCAST": BCAST, "IDX": plan["idx"],
    }


def make_in_maps(ins, plan=None):
    if plan is None:
        plan = plan_compaction(ins)
    shared = _prep_shared(ins, plan)
    enc = np.asarray(ins["encoded"], np.float32)
    tu = np.asarray(ins["true_u"], np.float32)
    mid = np.asarray(ins["mid_idx"], np.int64)
    i = plan["i"]
    nrowc = plan["nrowc"]
    rows = plan["rows"]
    s_idx = np.array([r[0] for r in rows], np.int64)
    t_idx = np.array([r[1] for r in rows], np.int64)

    in_maps = []
    for b in range(B):
        xg = np.zeros((nrowc, D), np.float32)
        xg[:len(rows)] = enc[b][s_idx, t_idx]
        ug = np.zeros((nrowc,), np.float32)
        ug[:len(rows)] = tu[b][s_idx, t_idx]
        xs16 = xg.T.reshape(2, P, nrowc).transpose(1, 0, 2) * XS
        xub = np.broadcast_to((ug * XS).astype(BFNP), (4, nrowc))
        pred = enc[b, i][mid]
        predt = pred.T.reshape(2, P, M).transpose(1, 0, 2).copy()
        m = dict(shared)
        m["XT8"] = _fp8(xs16, 1.0)
        m["XTB"] = xs16.astype(BFNP)
        m["XTUB"] = np.ascontiguousarray(xub)
        m["PREDT"] = predt
        in_maps.append(m)
    return in_maps


def kernel(**inputs):
    ins = {k: np.asarray(v) for k, v in inputs.items()}
    plan = plan_compaction(ins)
    in_maps = make_in_maps(ins, plan)
    has_b2 = bool(np.any(np.asarray(ins["kb2"], np.float32))
                  or np.any(np.asarray(ins["vb2"], np.float32)))
    nc = _build(plan["i"], plan["nrowc"], plan["sec_starts"], has_b2)
    res = run_bass_kernel_spmd(nc, in_maps, core_ids=list(range(NCORES)))
    out = np.stack([res.results[c]["OUT"] for c in range(NCORES)])
    return out.astype(np.float32)
